# revision 16
# baseline (speedup 1.0000x reference)
"""Multi-head attention (RoPE, causal) Trainium2 Bass kernel, 8-core SPMD.

Problem: B=2, S=2048, D=1024, H=16, DK=64, fp32, causal mask.

Sharding: core c handles batch b = c//4 and head group hg = c%4 (4 heads).
Each core computes Q/K/V projections for its 4 heads (column-sliced weights),
RoPE, causal attention, and a partial output projection (row-sliced Wo).
Host sums the 4 partial outputs per batch and adds the output bias.

Layout strategy (no on-device transposes):
  x^T [D, S] is precomputed on host; Q^T/K^T computed as [dk, S] tiles
  (weights stationary, x^T moving); scores computed transposed [k, q]
  (K^T stationary, Q^T moving); PV uses V in natural layout [k, dk+1]
  (stationary) with exp(scores^T) moving, accumulating attn^T [dk(+1), q];
  the ones column of V accumulates the softmax denominator. Normalization
  multiplies attn^T rows by broadcast 1/denom. Output projection uses
  attn^T as stationary and Wo as moving, producing natural [s, D] partials.

All matmul operands are float32r (TF32-like fast mode: 1 cycle/row at
moving-dim >= 256 vs 4 cycles/row for fp32): ~1e-4 relative L2 per matmul.
"""
import sys
sys.path.insert(0, "/opt/trn_rl_repo")
import math
import numpy as np

B, S, D, H, DK = 2, 2048, 1024, 16, 64
NCORES = 8
HPC = H // (NCORES // B)     # 4 heads per core
DHC = HPC * DK               # 256 attn dims per core
NPAIR = HPC // 2             # 2 head pairs per core
KC = D // 128                # 8 contraction chunks
NSB = S // 128               # 16 s-blocks / k-blocks
NQC = S // 512               # 4 q-chunks of 512

_BUILD_CACHE = {}


def _build(causal: bool):
    import concourse.tile as tile
    from concourse import bacc, mybir

    f32, f32r = mybir.dt.float32, mybir.dt.float32r
    MULT, ADD = mybir.AluOpType.mult, mybir.AluOpType.add
    EXP = mybir.ActivationFunctionType.Exp

    nc = bacc.Bacc(target_bir_lowering=False, trn_type="TRN2", debug=False)

    xT_d = nc.dram_tensor("xT", [D, S], f32r, kind="ExternalInput")
    wq_d = nc.dram_tensor("wq", [D, DHC], f32r, kind="ExternalInput")
    wk_d = nc.dram_tensor("wk", [D, DHC], f32r, kind="ExternalInput")
    wv_d = nc.dram_tensor("wv", [D, DHC], f32r, kind="ExternalInput")
    wo_d = nc.dram_tensor("wo", [DHC, D], f32r, kind="ExternalInput")
    bqk_d = nc.dram_tensor("bqk", [2, DHC], f32r, kind="ExternalInput")
    bv_d = nc.dram_tensor("bv", [1, DHC], f32r, kind="ExternalInput")
    ones_d = nc.dram_tensor("ones", [1, 512], f32r, kind="ExternalInput")
    psig_d = nc.dram_tensor("psig", [128, 128], f32r, kind="ExternalInput")
    rope_d = nc.dram_tensor("rope", [4, 128, S], f32, kind="ExternalInput")
    mdiag_d = nc.dram_tensor("mdiag", [128, 128], f32, kind="ExternalInput")
    out_d = nc.dram_tensor("out", [D, S], f32, kind="ExternalOutput")
    if not causal:
        maskT_d = nc.dram_tensor("maskT", [S, S], f32, kind="ExternalInput")

    with tile.TileContext(nc) as tc:
        with tc.tile_pool(name="const", bufs=1) as const_p, \
             tc.tile_pool(name="persist", bufs=1) as pers_p, \
             tc.tile_pool(name="ph2sb", bufs=1) as ph2_sb, \
             tc.tile_pool(name="ph3sb", bufs=1) as ph3_sb:

            # ---------- constants ----------
            ones_t = const_p.tile([1, 512], f32r, tag="ones")
            nc.sync.dma_start(out=ones_t, in_=ones_d[:])
            psig_t = const_p.tile([128, 128], f32r, tag="psig")
            nc.sync.dma_start(out=psig_t, in_=psig_d[:])
            bq_t = const_p.tile([1, DHC], f32r, tag="bq")
            nc.sync.dma_start(out=bq_t, in_=bqk_d[0:1, :])
            bk_t = const_p.tile([1, DHC], f32r, tag="bk")
            nc.sync.dma_start(out=bk_t, in_=bqk_d[1:2, :])
            bv_t = const_p.tile([1, DHC], f32r, tag="bv")
            nc.sync.dma_start(out=bv_t, in_=bv_d[:])
            mdiag_t = const_p.tile([128, 128], f32, tag="mdiag")
            nc.sync.dma_start(out=mdiag_t, in_=mdiag_d[:])

            # ---------- persistent activations ----------
            qt_pair = [pers_p.tile([128, S], f32r, tag=f"qt{p}", name=f"qt{p}") for p in range(NPAIR)]
            kt_pair = [pers_p.tile([128, S], f32r, tag=f"kt{p}", name=f"kt{p}") for p in range(NPAIR)]
            v_sb = [pers_p.tile([128, HPC, DK + 1], f32r, tag=f"v{i}", name=f"v{i}") for i in range(NSB)]
            attnT_sb = [pers_p.tile([128, S], f32r, tag=f"at{p}", name=f"at{p}") for p in range(NPAIR)]

            # =========================================================
            # Phase 1: projections + RoPE + V assembly
            # =========================================================
            with tc.tile_pool(name="ph1sb", bufs=1) as ph1_sb, \
                 tc.tile_pool(name="ph1ps", bufs=1, space="PSUM") as ph1_ps:

                def load_xq(qc, tiles_only=False):
                    ql, qh = 512 * qc, 512 * (qc + 1)
                    ts = [ph1_sb.tile([128, 512], f32r, tag="xq", bufs=10, name=f"xq{kc}_{qc}")
                          for kc in range(KC)]
                    if not tiles_only:
                        for kc in range(KC):
                            nc.sync.dma_start(out=ts[kc], in_=xT_d[128 * kc:128 * (kc + 1), ql:qh])
                    return ts

                # q-chunks processed descending so that phase 2 (which walks
                # k-blocks descending) can start as soon as the tail chunk of
                # Q^T/K^T/V is ready.  First-chunk x slices and weights are
                # DMA'd interleaved per k-chunk so the first matmuls start
                # as soon as possible.
                qc_order = list(reversed(range(NQC)))
                w_t = {t_i: [ph1_sb.tile([128, DHC], f32r, tag=f"w{t_i}_{kc}", name=f"w{t_i}_{kc}")
                             for kc in range(KC)] for t_i in (0, 1)}
                wv_t = [ph1_sb.tile([128, DHC], f32r, tag=f"wv{kc}", name=f"wv{kc}")
                        for kc in range(KC)]
                def load_rope(qc):
                    ql = 512 * qc
                    ts = [ph1_sb.tile([128, 512], f32, tag="rope", bufs=6, name=f"rope{i}_{qc}")
                          for i in range(4)]
                    for i in range(4):
                        nc.sync.dma_start(out=ts[i], in_=rope_d[i][:, ql:ql + 512])
                    return ts

                xq_next = load_xq(qc_order[0], tiles_only=True)
                q3l = 512 * qc_order[0]
                for kc in range(KC):
                    nc.sync.dma_start(out=w_t[0][kc], in_=wq_d[128 * kc:128 * (kc + 1), :])
                    nc.sync.dma_start(out=w_t[1][kc], in_=wk_d[128 * kc:128 * (kc + 1), :])
                    nc.sync.dma_start(out=xq_next[kc], in_=xT_d[128 * kc:128 * (kc + 1), q3l:q3l + 512])
                    nc.sync.dma_start(out=wv_t[kc], in_=wv_d[128 * kc:128 * (kc + 1), :])
                rope_next = load_rope(qc_order[0])

                for qi, qc in enumerate(qc_order):
                    ql, qh = 512 * qc, 512 * (qc + 1)
                    xq = xq_next
                    rope_s = rope_next
                    pps = {}
                    for t_i in (0, 1):
                        for p in range(NPAIR):
                            pps[t_i, p] = ph1_ps.tile([128, 512], f32, tag="qtp",
                                                      bufs=4, name=f"pp{t_i}_{p}_{qc}")
                    for kc in range(KC):
                        for t_i in (0, 1):
                            for p in range(NPAIR):
                                nc.tensor.matmul(pps[t_i, p],
                                                 w_t[t_i][kc][:, 128 * p:128 * (p + 1)],
                                                 xq[kc], start=(kc == 0), stop=False)
                    if qi + 1 < NQC:
                        xq_next = load_xq(qc_order[qi + 1])
                        rope_next = load_rope(qc_order[qi + 1])
                    for t_i in (0, 1):
                        cos_t, sin_t = rope_s[2 * t_i], rope_s[2 * t_i + 1]
                        dst_pair = qt_pair if t_i == 0 else kt_pair
                        bias_t = bq_t if t_i == 0 else bk_t
                        for p in range(NPAIR):
                            pp = pps[t_i, p]
                            nc.tensor.matmul(pp, bias_t[:, 128 * p:128 * (p + 1)],
                                             ones_t, start=False, stop=True)
                            # RoPE: dst = pp*cos + Psig @ (pp*sin_sig)
                            u_t = ph1_sb.tile([128, 512], f32r, tag="u", bufs=2)
                            nc.vector.tensor_tensor(out=u_t, in0=pp, in1=sin_t, op=MULT)
                            us = ph1_ps.tile([128, 512], f32, tag="usp", bufs=2)
                            nc.tensor.matmul(us, psig_t, u_t, start=True, stop=True)
                            dst = dst_pair[p][:, ql:qh]
                            nc.vector.tensor_tensor(out=dst, in0=pp, in1=cos_t, op=MULT)
                            nc.vector.tensor_tensor(out=dst, in0=us, in1=dst.bitcast(f32), op=ADD)
                    # V projection for the 4 s-blocks covered by this q-chunk
                    for r in range(4):
                        si = 4 * qc + r
                        vp = ph1_ps.tile([128, DHC + HPC], f32, tag="vp", bufs=2)
                        for kc in range(KC):
                            nc.tensor.matmul(vp[:, 0:DHC], xq[kc][:, 128 * r:128 * (r + 1)],
                                             wv_t[kc], start=(kc == 0), stop=False)
                        nc.tensor.matmul(vp[:, 0:DHC], ones_t[:, 0:128], bv_t,
                                         start=False, stop=True)
                        nc.tensor.matmul(vp[:, DHC:DHC + HPC], ones_t[:, 0:128],
                                         ones_t[:, 0:HPC], start=True, stop=True)
                        nc.scalar.copy(out=v_sb[si][:, :, 0:DK],
                                       in_=vp[:, 0:DHC].rearrange("p (h d) -> p h d", h=HPC))
                        nc.scalar.copy(out=v_sb[si][:, :, DK:DK + 1],
                                       in_=vp[:, DHC:DHC + HPC].rearrange("p (h o) -> p h o", h=HPC))

            # =========================================================
            # Phase 2: attention per head
            # =========================================================
            HALF = S // 2
            with tc.tile_pool(name="ph2ps", bufs=1, space="PSUM") as ph2_ps:
                # Per (pair, q-half): the two heads of the pair run as two
                # interleaved dependency chains (separate scores psum + attn
                # accumulator each) so PE and ACT stay busy simultaneously.
                # k-blocks walk descending (matches phase-1 production order);
                # PV trails one item behind QK^T/exp.
                def emit_pv(h, at_ps, hlo, it, pT):
                    j, base, w, a0 = it
                    a = a0
                    while a < w:
                        bnd = min((a // 512 + 1) * 512, w)
                        sbank = (base + a) // 512
                        jmax = min(NSB - 1, 4 * sbank + 3) if causal else NSB - 1
                        nc.tensor.matmul(at_ps[:, base + a - hlo:base + bnd - hlo],
                                         v_sb[j][:, h, :], pT[:, a:bnd],
                                         start=(j == jmax), stop=(j == 0))
                        a = bnd

                for p in range(NPAIR):
                    for half in (1, 0):
                        hlo, hhi = HALF * half, HALF * (half + 1)
                        at_ps = [ph2_ps.tile([DK + 1, HALF], f32, tag="atp",
                                             bufs=2, name=f"atp{p}_{half}_{hh}")
                                 for hh in range(2)]
                        items = []
                        for j in reversed(range(NSB)):
                            qlo = max(128 * j, hlo) if causal else hlo
                            if qlo >= hhi:
                                continue
                            base = (qlo // 512) * 512
                            first = True
                            while base < hhi:
                                w = min(1024, hhi - base)
                                a0 = (qlo - base) if first else 0
                                items.append((j, base, w, a0))
                                base += w
                                first = False
                        pend = []
                        for it in items:
                            j, base, w, a0 = it
                            scs = []
                            for hh in range(2):
                                off = 64 * hh
                                sc = ph2_ps.tile([128, 1024], f32, tag="sc",
                                                 bufs=2, name=f"sc{hh}")
                                a = a0
                                while a < w:
                                    bnd = min((a // 512 + 1) * 512, w)
                                    nc.tensor.matmul(
                                        sc[:, a:bnd],
                                        kt_pair[p][off:off + 64, 128 * j:128 * (j + 1)],
                                        qt_pair[p][off:off + 64, base + a:base + bnd],
                                        start=True, stop=True)
                                    a = bnd
                                scs.append(sc)
                            if pend:
                                for (pit, phh, ppT) in pend:
                                    emit_pv(2 * p + phh, at_ps[phh], hlo, pit, ppT)
                                pend = []
                            for hh in range(2):
                                sc = scs[hh]
                                if not causal:
                                    mt = ph2_sb.tile([128, 1024], f32, tag="mt", bufs=3)
                                    nc.sync.dma_start(
                                        out=mt[:, a0:w],
                                        in_=maskT_d[128 * j:128 * (j + 1), base + a0:base + w])
                                    nc.vector.tensor_tensor(
                                        out=sc[:, a0:w], in0=sc[:, a0:w],
                                        in1=mt[:, a0:w], op=ADD)
                                pT = ph2_sb.tile([128, 1024], f32r, tag="pT", bufs=4)
                                nc.scalar.activation(out=pT[:, a0:w], in_=sc[:, a0:w], func=EXP)
                                if causal and base <= 128 * j < base + w:
                                    dc = 128 * j - base
                                    nc.vector.tensor_tensor(
                                        out=pT[:, dc:dc + 128],
                                        in0=pT[:, dc:dc + 128].bitcast(f32),
                                        in1=mdiag_t, op=MULT)
                                pend.append((it, hh, pT))
                        for (pit, phh, ppT) in pend:
                            emit_pv(2 * p + phh, at_ps[phh], hlo, pit, ppT)
                        for hh in range(2):
                            off = 64 * hh
                            rec = ph2_sb.tile([1, HALF], f32, tag="rec", bufs=2)
                            nc.vector.reciprocal(rec, at_ps[hh][DK:DK + 1, :])
                            bc = ph2_sb.tile([64, HALF], f32, tag="bc", bufs=2)
                            nc.gpsimd.partition_broadcast(bc, rec)
                            nc.vector.tensor_tensor(out=attnT_sb[p][off:off + 64, hlo:hhi],
                                                    in0=at_ps[hh][0:DK, :], in1=bc, op=MULT)

            # =========================================================
            # Phase 3: output projection (partial; host sums cores + bias)
            # =========================================================
            # output projection computed transposed: out^T[dout, s] so the
            # stationary operand (Wo chunk) is reused across the whole s sweep
            # (one weight load per (dout-block, chunk) instead of per matmul).
            # The host transposes the [D, S] partial back.
            with tc.tile_pool(name="ph3ps", bufs=1, space="PSUM") as ph3_ps:
                wo_t = [ph3_sb.tile([128, D], f32r, tag=f"wo{ch}", name=f"wo{ch}") for ch in range(NPAIR)]
                for ch in range(NPAIR):
                    nc.sync.dma_start(out=wo_t[ch], in_=wo_d[128 * ch:128 * (ch + 1), :])
                for do in range(D // 128):          # 8 dout blocks
                    ops = [ph3_ps.tile([128, 512], f32, tag="op", bufs=8, name=f"op{do}_{sc_}")
                           for sc_ in range(NQC)]
                    for ch in range(NPAIR):
                        for sc_ in range(NQC):
                            nc.tensor.matmul(ops[sc_],
                                             wo_t[ch][:, 128 * do:128 * (do + 1)],
                                             attnT_sb[ch][:, 512 * sc_:512 * (sc_ + 1)],
                                             start=(ch == 0), stop=(ch == NPAIR - 1))
                    for sc_ in range(NQC):
                        ob = ph3_sb.tile([128, 512], f32, tag="ob", bufs=8)
                        if sc_ % 2 == 0:
                            nc.vector.tensor_copy(ob, ops[sc_])
                        else:
                            nc.scalar.copy(out=ob, in_=ops[sc_])
                        nc.sync.dma_start(out=out_d[128 * do:128 * (do + 1), 512 * sc_:512 * (sc_ + 1)],
                                          in_=ob)

    nc.compile()
    return nc


def _rope_tables():
    half = DK // 2
    freqs = (10000.0 ** (-2.0 / DK * np.arange(half, dtype=np.float32))).astype(np.float64)
    ang = np.outer(np.arange(S, dtype=np.float64), freqs)           # [S, 32]
    cos1 = np.cos(ang).T.astype(np.float32)                          # [32, S]
    sin1 = np.sin(ang).T.astype(np.float32)
    c64 = np.concatenate([cos1, cos1], axis=0)                       # [64, S]
    ssig64 = np.concatenate([sin1, -sin1], axis=0)                   # s-tilde(sigma(p))
    c128 = np.concatenate([c64, c64], axis=0)
    ssig128 = np.concatenate([ssig64, ssig64], axis=0)
    scale = np.float32(1.0 / math.sqrt(DK))
    return np.stack([c128 * scale, ssig128 * scale, c128, ssig128]).astype(np.float32)


def _psig():
    p64 = np.zeros((64, 64), np.float32)
    p64[np.arange(32) + 32, np.arange(32)] = 1.0
    p64[np.arange(32), np.arange(32) + 32] = 1.0
    p = np.zeros((128, 128), np.float32)
    p[0:64, 0:64] = p64
    p[64:128, 64:128] = p64
    return p


def kernel(x, mask, Wq, bq, Wk, bk, Wv, bv, Wo, bo):
    from concourse.bass_utils import run_bass_kernel_spmd

    x = np.asarray(x, dtype=np.float32)
    mask = np.asarray(mask)
    Wq, bq = np.asarray(Wq, np.float32), np.asarray(bq, np.float32)
    Wk, bk = np.asarray(Wk, np.float32), np.asarray(bk, np.float32)
    Wv, bv = np.asarray(Wv, np.float32), np.asarray(bv, np.float32)
    Wo, bo = np.asarray(Wo, np.float32), np.asarray(bo, np.float32)

    causal_ref = np.triu(np.ones((S, S), dtype=bool), k=1)
    m2 = np.broadcast_to(mask, (B, 1, S, S))[:, 0]
    causal = all(np.array_equal(m2[b], causal_ref) for b in range(B))

    key = causal
    if key not in _BUILD_CACHE:
        _BUILD_CACHE[key] = _build(causal)
    nc = _BUILD_CACHE[key]

    rope = _rope_tables()
    psig = _psig()
    ones = np.ones((1, 512), np.float32)
    # multiplicative 0/1 mask for the diagonal block (applied to exp(scores))
    mdiag = np.where(np.arange(128)[:, None] > np.arange(128)[None, :],
                     np.float32(0.0), np.float32(1.0)).astype(np.float32)

    in_maps = []
    for c in range(NCORES):
        b, hg = c // (NCORES // B), c % (NCORES // B)
        cs = slice(DHC * hg, DHC * (hg + 1))
        im = {
            "xT": np.ascontiguousarray(x[b].T),
            "wq": np.ascontiguousarray(Wq[:, cs]),
            "wk": np.ascontiguousarray(Wk[:, cs]),
            "wv": np.ascontiguousarray(Wv[:, cs]),
            "wo": np.ascontiguousarray(Wo[cs, :]),
            "bqk": np.ascontiguousarray(np.stack([bq[cs], bk[cs]])),
            "bv": np.ascontiguousarray(bv[cs][None, :]),
            "ones": ones, "psig": psig, "rope": rope, "mdiag": mdiag,
        }
        if not causal:
            madd = np.where(m2[b], np.float32(-1e30), np.float32(0.0))
            im["maskT"] = np.ascontiguousarray(madd.T)
        in_maps.append(im)

    res = run_bass_kernel_spmd(nc, in_maps, core_ids=list(range(NCORES)))
    out = np.zeros((B, S, D), np.float32)
    for c in range(NCORES):
        out[c // (NCORES // B)] += res.results[c]["out"].T
    out += bo[None, None, :]
    return out


# revision 20
# speedup vs baseline: 1.0959x; 1.0959x over previous
"""Multi-head attention (RoPE, causal) Trainium2 Bass kernel, 8-core SPMD.

Problem: B=2, S=2048, D=1024, H=16, DK=64, fp32, causal mask.

Sharding: core c handles batch b = c//4 and head group hg = c%4 (4 heads).
Each core computes Q/K/V projections for its 4 heads (column-sliced weights),
RoPE, causal attention, and a partial output projection (row-sliced Wo).
Host sums the 4 partial outputs per batch and adds the output bias.

Layout strategy (no on-device transposes):
  x^T [D, S] is precomputed on host; Q^T/K^T computed as [dk, S] tiles
  (weights stationary, x^T moving); scores computed transposed [k, q]
  (K^T stationary, Q^T moving); PV uses V in natural layout [k, dk+1]
  (stationary) with exp(scores^T) moving, accumulating attn^T [dk(+1), q];
  the ones column of V accumulates the softmax denominator. Normalization
  multiplies attn^T rows by broadcast 1/denom. Output projection uses
  attn^T as stationary and Wo as moving, producing natural [s, D] partials.

All matmul operands are float32r (TF32-like fast mode: 1 cycle/row at
moving-dim >= 256 vs 4 cycles/row for fp32): ~1e-4 relative L2 per matmul.
"""
import sys
sys.path.insert(0, "/opt/trn_rl_repo")
import math
import numpy as np

B, S, D, H, DK = 2, 2048, 1024, 16, 64
NCORES = 8
HPC = H // (NCORES // B)     # 4 heads per core
DHC = HPC * DK               # 256 attn dims per core
NPAIR = HPC // 2             # 2 head pairs per core
KC = D // 128                # 8 contraction chunks
NSB = S // 128               # 16 s-blocks / k-blocks
NQC = S // 512               # 4 q-chunks of 512

_BUILD_CACHE = {}


def _build(causal: bool):
    import concourse.tile as tile
    from concourse import bacc, mybir

    f32, f32r = mybir.dt.float32, mybir.dt.float32r
    MULT, ADD = mybir.AluOpType.mult, mybir.AluOpType.add
    EXP = mybir.ActivationFunctionType.Exp

    nc = bacc.Bacc(target_bir_lowering=False, trn_type="TRN2", debug=False)

    xT_d = nc.dram_tensor("xT", [D, S], f32r, kind="ExternalInput")
    wq_d = nc.dram_tensor("wq", [D, DHC], f32r, kind="ExternalInput")
    wk_d = nc.dram_tensor("wk", [D, DHC], f32r, kind="ExternalInput")
    wv_d = nc.dram_tensor("wv", [D, DHC], f32r, kind="ExternalInput")
    wo_d = nc.dram_tensor("wo", [DHC, D], f32r, kind="ExternalInput")
    bqk_d = nc.dram_tensor("bqk", [2, DHC], f32r, kind="ExternalInput")
    bv_d = nc.dram_tensor("bv", [1, DHC], f32r, kind="ExternalInput")
    ones_d = nc.dram_tensor("ones", [1, 512], f32r, kind="ExternalInput")
    psig_d = nc.dram_tensor("psig", [128, 128], f32r, kind="ExternalInput")
    rope_d = nc.dram_tensor("rope", [4, 128, S], f32, kind="ExternalInput")
    mdiag_d = nc.dram_tensor("mdiag", [128, 128], f32, kind="ExternalInput")
    out_d = nc.dram_tensor("out", [D, S], f32, kind="ExternalOutput")
    if not causal:
        maskT_d = nc.dram_tensor("maskT", [S, S], f32, kind="ExternalInput")

    with tile.TileContext(nc) as tc:
        with tc.tile_pool(name="const", bufs=1) as const_p, \
             tc.tile_pool(name="persist", bufs=1) as pers_p, \
             tc.tile_pool(name="ph2sb", bufs=1) as ph2_sb, \
             tc.tile_pool(name="ph3sb", bufs=1) as ph3_sb:

            # ---------- constants ----------
            ones_t = const_p.tile([1, 512], f32r, tag="ones")
            nc.sync.dma_start(out=ones_t, in_=ones_d[:])
            psig_t = const_p.tile([128, 128], f32r, tag="psig")
            nc.sync.dma_start(out=psig_t, in_=psig_d[:])
            # per-pair bias columns [128, 1] (partition = head-pair dims)
            bcol = {}
            for t_i in (0, 1):
                for p in range(NPAIR):
                    bt = const_p.tile([128, 1], f32r, tag=f"bc{t_i}{p}", name=f"bc{t_i}{p}")
                    src_row = bqk_d[t_i:t_i + 1, 128 * p:128 * (p + 1)]
                    nc.sync.dma_start(out=bt, in_=src_row.rearrange("o d -> d o"))
                    bcol[t_i, p] = bt.bitcast(f32)
            bv_t = const_p.tile([1, DHC], f32r, tag="bv")
            nc.sync.dma_start(out=bv_t, in_=bv_d[:])
            mdiag_t = const_p.tile([128, 128], f32, tag="mdiag")
            nc.sync.dma_start(out=mdiag_t, in_=mdiag_d[:])

            # ---------- persistent activations ----------
            qt_pair = [pers_p.tile([128, S], f32r, tag=f"qt{p}", name=f"qt{p}") for p in range(NPAIR)]
            kt_pair = [pers_p.tile([128, S], f32r, tag=f"kt{p}", name=f"kt{p}") for p in range(NPAIR)]
            v_sb = [pers_p.tile([128, HPC, DK + 1], f32r, tag=f"v{i}", name=f"v{i}") for i in range(NSB)]
            attnT_sb = [pers_p.tile([128, S], f32r, tag=f"at{p}", name=f"at{p}") for p in range(NPAIR)]

            # =========================================================
            # Phase 1: projections + RoPE + V assembly
            # =========================================================
            with tc.tile_pool(name="ph1sb", bufs=1) as ph1_sb, \
                 tc.tile_pool(name="ph1ps", bufs=1, space="PSUM") as ph1_ps:

                def load_xq(qc, tiles_only=False):
                    ql, qh = 512 * qc, 512 * (qc + 1)
                    ts = [ph1_sb.tile([128, 512], f32r, tag="xq", bufs=10, name=f"xq{kc}_{qc}")
                          for kc in range(KC)]
                    if not tiles_only:
                        for kc in range(KC):
                            nc.sync.dma_start(out=ts[kc], in_=xT_d[128 * kc:128 * (kc + 1), ql:qh])
                    return ts

                # q-chunks processed descending so that phase 2 (which walks
                # k-blocks descending) can start as soon as the tail chunk of
                # Q^T/K^T/V is ready.  First-chunk x slices and weights are
                # DMA'd interleaved per k-chunk so the first matmuls start
                # as soon as possible.
                qc_order = list(reversed(range(NQC)))
                w_t = {t_i: [ph1_sb.tile([128, DHC], f32r, tag=f"w{t_i}_{kc}", name=f"w{t_i}_{kc}")
                             for kc in range(KC)] for t_i in (0, 1)}
                wv_t = [ph1_sb.tile([128, DHC], f32r, tag=f"wv{kc}", name=f"wv{kc}")
                        for kc in range(KC)]
                def load_rope(qc):
                    ql = 512 * qc
                    ts = [ph1_sb.tile([128, 512], f32, tag="rope", bufs=6, name=f"rope{i}_{qc}")
                          for i in range(4)]
                    for i in range(4):
                        nc.sync.dma_start(out=ts[i], in_=rope_d[i][:, ql:ql + 512])
                    return ts

                xq_next = load_xq(qc_order[0], tiles_only=True)
                q3l = 512 * qc_order[0]
                for kc in range(KC):
                    nc.sync.dma_start(out=w_t[0][kc], in_=wq_d[128 * kc:128 * (kc + 1), :])
                    nc.sync.dma_start(out=w_t[1][kc], in_=wk_d[128 * kc:128 * (kc + 1), :])
                    nc.sync.dma_start(out=xq_next[kc], in_=xT_d[128 * kc:128 * (kc + 1), q3l:q3l + 512])
                    nc.sync.dma_start(out=wv_t[kc], in_=wv_d[128 * kc:128 * (kc + 1), :])
                rope_next = load_rope(qc_order[0])

                for qi, qc in enumerate(qc_order):
                    ql, qh = 512 * qc, 512 * (qc + 1)
                    xq = xq_next
                    rope_s = rope_next
                    pps = {}
                    for t_i in (0, 1):
                        for p in range(NPAIR):
                            pps[t_i, p] = ph1_ps.tile([128, 512], f32, tag="qtp",
                                                      bufs=4, name=f"pp{t_i}_{p}_{qc}")
                    for kc in range(KC):
                        for t_i in (0, 1):
                            for p in range(NPAIR):
                                nc.tensor.matmul(pps[t_i, p],
                                                 w_t[t_i][kc][:, 128 * p:128 * (p + 1)],
                                                 xq[kc], start=(kc == 0), stop=(kc == KC - 1))
                    if qi + 1 < NQC:
                        xq_next = load_xq(qc_order[qi + 1])
                        rope_next = load_rope(qc_order[qi + 1])
                    for t_i in (0, 1):
                        cos_t, sin_t = rope_s[2 * t_i], rope_s[2 * t_i + 1]
                        dst_pair = qt_pair if t_i == 0 else kt_pair
                        for p in range(NPAIR):
                            pp = pps[t_i, p]
                            # RoPE with bias folded in:
                            #   dst = (pp+b)*cos + Psig @ ((pp+b)*sin_sig)
                            u_t = ph1_sb.tile([128, 512], f32r, tag="u", bufs=2)
                            nc.vector.scalar_tensor_tensor(
                                out=u_t, in0=pp, scalar=bcol[t_i, p], in1=sin_t,
                                op0=ADD, op1=MULT)
                            us = ph1_ps.tile([128, 512], f32, tag="usp", bufs=2)
                            nc.tensor.matmul(us, psig_t, u_t, start=True, stop=True)
                            dst = dst_pair[p][:, ql:qh]
                            nc.vector.scalar_tensor_tensor(
                                out=dst, in0=pp, scalar=bcol[t_i, p], in1=cos_t,
                                op0=ADD, op1=MULT)
                            nc.vector.tensor_tensor(out=dst, in0=us, in1=dst.bitcast(f32), op=ADD)
                    # V projection for the 4 s-blocks covered by this q-chunk
                    for r in range(4):
                        si = 4 * qc + r
                        vp = ph1_ps.tile([128, DHC + HPC], f32, tag="vp", bufs=2)
                        for kc in range(KC):
                            nc.tensor.matmul(vp[:, 0:DHC], xq[kc][:, 128 * r:128 * (r + 1)],
                                             wv_t[kc], start=(kc == 0), stop=False)
                        nc.tensor.matmul(vp[:, 0:DHC], ones_t[:, 0:128], bv_t,
                                         start=False, stop=True)
                        nc.tensor.matmul(vp[:, DHC:DHC + HPC], ones_t[:, 0:128],
                                         ones_t[:, 0:HPC], start=True, stop=True)
                        nc.scalar.copy(out=v_sb[si][:, :, 0:DK],
                                       in_=vp[:, 0:DHC].rearrange("p (h d) -> p h d", h=HPC))
                        nc.scalar.copy(out=v_sb[si][:, :, DK:DK + 1],
                                       in_=vp[:, DHC:DHC + HPC].rearrange("p (h o) -> p h o", h=HPC))

            # =========================================================
            # Phase 2: attention per head
            # =========================================================
            HALF = S // 2
            with tc.tile_pool(name="ph2ps", bufs=1, space="PSUM") as ph2_ps:
                # Per (pair, q-half): the two heads of the pair run as two
                # interleaved dependency chains (separate scores psum + attn
                # accumulator each) so PE and ACT stay busy simultaneously.
                # k-blocks walk descending (matches phase-1 production order);
                # PV trails one item behind QK^T/exp.
                def emit_pv(h, at_ps, hlo, it, pT):
                    j, base, w, a0 = it
                    a = a0
                    while a < w:
                        bnd = min((a // 512 + 1) * 512, w)
                        sbank = (base + a) // 512
                        jmax = min(NSB - 1, 4 * sbank + 3) if causal else NSB - 1
                        nc.tensor.matmul(at_ps[:, base + a - hlo:base + bnd - hlo],
                                         v_sb[j][:, h, :], pT[:, a:bnd],
                                         start=(j == jmax), stop=(j == 0))
                        a = bnd

                for p in range(NPAIR):
                    for half in (1, 0):
                        hlo, hhi = HALF * half, HALF * (half + 1)
                        at_ps = [ph2_ps.tile([DK + 1, HALF], f32, tag="atp",
                                             bufs=2, name=f"atp{p}_{half}_{hh}")
                                 for hh in range(2)]
                        items = []
                        for j in reversed(range(NSB)):
                            qlo = max(128 * j, hlo) if causal else hlo
                            if qlo >= hhi:
                                continue
                            base = (qlo // 512) * 512
                            first = True
                            while base < hhi:
                                w = min(1024, hhi - base)
                                a0 = (qlo - base) if first else 0
                                items.append((j, base, w, a0))
                                base += w
                                first = False
                        pend = []
                        for it in items:
                            j, base, w, a0 = it
                            scs = []
                            for hh in range(2):
                                off = 64 * hh
                                sc = ph2_ps.tile([128, 1024], f32, tag="sc",
                                                 bufs=2, name=f"sc{hh}")
                                a = a0
                                while a < w:
                                    bnd = min((a // 512 + 1) * 512, w)
                                    nc.tensor.matmul(
                                        sc[:, a:bnd],
                                        kt_pair[p][off:off + 64, 128 * j:128 * (j + 1)],
                                        qt_pair[p][off:off + 64, base + a:base + bnd],
                                        start=True, stop=True)
                                    a = bnd
                                scs.append(sc)
                            if pend:
                                for (pit, phh, ppT) in pend:
                                    emit_pv(2 * p + phh, at_ps[phh], hlo, pit, ppT)
                                pend = []
                            for hh in range(2):
                                sc = scs[hh]
                                if not causal:
                                    mt = ph2_sb.tile([128, 1024], f32, tag="mt", bufs=3)
                                    nc.sync.dma_start(
                                        out=mt[:, a0:w],
                                        in_=maskT_d[128 * j:128 * (j + 1), base + a0:base + w])
                                    nc.vector.tensor_tensor(
                                        out=sc[:, a0:w], in0=sc[:, a0:w],
                                        in1=mt[:, a0:w], op=ADD)
                                pT = ph2_sb.tile([128, 1024], f32r, tag="pT", bufs=4)
                                nc.scalar.activation(out=pT[:, a0:w], in_=sc[:, a0:w], func=EXP)
                                if causal and base <= 128 * j < base + w:
                                    dc = 128 * j - base
                                    nc.vector.tensor_tensor(
                                        out=pT[:, dc:dc + 128],
                                        in0=pT[:, dc:dc + 128].bitcast(f32),
                                        in1=mdiag_t, op=MULT)
                                pend.append((it, hh, pT))
                        for (pit, phh, ppT) in pend:
                            emit_pv(2 * p + phh, at_ps[phh], hlo, pit, ppT)
                        for hh in range(2):
                            off = 64 * hh
                            rec = ph2_sb.tile([1, HALF], f32, tag="rec", bufs=2)
                            nc.vector.reciprocal(rec, at_ps[hh][DK:DK + 1, :])
                            bc = ph2_sb.tile([64, HALF], f32, tag="bc", bufs=2)
                            nc.gpsimd.partition_broadcast(bc, rec)
                            nc.vector.tensor_tensor(out=attnT_sb[p][off:off + 64, hlo:hhi],
                                                    in0=at_ps[hh][0:DK, :], in1=bc, op=MULT)

            # =========================================================
            # Phase 3: output projection (partial; host sums cores + bias)
            # =========================================================
            # output projection computed transposed: out^T[dout, s] so the
            # stationary operand (Wo chunk) is reused across the whole s sweep
            # (one weight load per (dout-block, chunk) instead of per matmul).
            # The host transposes the [D, S] partial back.
            with tc.tile_pool(name="ph3ps", bufs=1, space="PSUM") as ph3_ps:
                wo_t = [ph3_sb.tile([128, D], f32r, tag=f"wo{ch}", name=f"wo{ch}") for ch in range(NPAIR)]
                for ch in range(NPAIR):
                    nc.sync.dma_start(out=wo_t[ch], in_=wo_d[128 * ch:128 * (ch + 1), :])
                for do in range(D // 128):          # 8 dout blocks
                    ops = [ph3_ps.tile([128, 512], f32, tag="op", bufs=8, name=f"op{do}_{sc_}")
                           for sc_ in range(NQC)]
                    for ch in range(NPAIR):
                        for sc_ in range(NQC):
                            nc.tensor.matmul(ops[sc_],
                                             wo_t[ch][:, 128 * do:128 * (do + 1)],
                                             attnT_sb[ch][:, 512 * sc_:512 * (sc_ + 1)],
                                             start=(ch == 0), stop=(ch == NPAIR - 1))
                    for sc_ in range(NQC):
                        ob = ph3_sb.tile([128, 512], f32, tag="ob", bufs=8)
                        if sc_ % 2 == 0:
                            nc.vector.tensor_copy(ob, ops[sc_])
                        else:
                            nc.scalar.copy(out=ob, in_=ops[sc_])
                        nc.sync.dma_start(out=out_d[128 * do:128 * (do + 1), 512 * sc_:512 * (sc_ + 1)],
                                          in_=ob)

    nc.compile()
    return nc


def _rope_tables():
    half = DK // 2
    freqs = (10000.0 ** (-2.0 / DK * np.arange(half, dtype=np.float32))).astype(np.float64)
    ang = np.outer(np.arange(S, dtype=np.float64), freqs)           # [S, 32]
    cos1 = np.cos(ang).T.astype(np.float32)                          # [32, S]
    sin1 = np.sin(ang).T.astype(np.float32)
    c64 = np.concatenate([cos1, cos1], axis=0)                       # [64, S]
    ssig64 = np.concatenate([sin1, -sin1], axis=0)                   # s-tilde(sigma(p))
    c128 = np.concatenate([c64, c64], axis=0)
    ssig128 = np.concatenate([ssig64, ssig64], axis=0)
    scale = np.float32(1.0 / math.sqrt(DK))
    return np.stack([c128 * scale, ssig128 * scale, c128, ssig128]).astype(np.float32)


def _psig():
    p64 = np.zeros((64, 64), np.float32)
    p64[np.arange(32) + 32, np.arange(32)] = 1.0
    p64[np.arange(32), np.arange(32) + 32] = 1.0
    p = np.zeros((128, 128), np.float32)
    p[0:64, 0:64] = p64
    p[64:128, 64:128] = p64
    return p


def kernel(x, mask, Wq, bq, Wk, bk, Wv, bv, Wo, bo):
    from concourse.bass_utils import run_bass_kernel_spmd

    x = np.asarray(x, dtype=np.float32)
    mask = np.asarray(mask)
    Wq, bq = np.asarray(Wq, np.float32), np.asarray(bq, np.float32)
    Wk, bk = np.asarray(Wk, np.float32), np.asarray(bk, np.float32)
    Wv, bv = np.asarray(Wv, np.float32), np.asarray(bv, np.float32)
    Wo, bo = np.asarray(Wo, np.float32), np.asarray(bo, np.float32)

    causal_ref = np.triu(np.ones((S, S), dtype=bool), k=1)
    m2 = np.broadcast_to(mask, (B, 1, S, S))[:, 0]
    causal = all(np.array_equal(m2[b], causal_ref) for b in range(B))

    key = causal
    if key not in _BUILD_CACHE:
        _BUILD_CACHE[key] = _build(causal)
    nc = _BUILD_CACHE[key]

    rope = _rope_tables()
    psig = _psig()
    ones = np.ones((1, 512), np.float32)
    # multiplicative 0/1 mask for the diagonal block (applied to exp(scores))
    mdiag = np.where(np.arange(128)[:, None] > np.arange(128)[None, :],
                     np.float32(0.0), np.float32(1.0)).astype(np.float32)

    in_maps = []
    for c in range(NCORES):
        b, hg = c // (NCORES // B), c % (NCORES // B)
        cs = slice(DHC * hg, DHC * (hg + 1))
        im = {
            "xT": np.ascontiguousarray(x[b].T),
            "wq": np.ascontiguousarray(Wq[:, cs]),
            "wk": np.ascontiguousarray(Wk[:, cs]),
            "wv": np.ascontiguousarray(Wv[:, cs]),
            "wo": np.ascontiguousarray(Wo[cs, :]),
            "bqk": np.ascontiguousarray(np.stack([bq[cs], bk[cs]])),
            "bv": np.ascontiguousarray(bv[cs][None, :]),
            "ones": ones, "psig": psig, "rope": rope, "mdiag": mdiag,
        }
        if not causal:
            madd = np.where(m2[b], np.float32(-1e30), np.float32(0.0))
            im["maskT"] = np.ascontiguousarray(madd.T)
        in_maps.append(im)

    res = run_bass_kernel_spmd(nc, in_maps, core_ids=list(range(NCORES)))
    out = np.zeros((B, S, D), np.float32)
    for c in range(NCORES):
        out[c // (NCORES // B)] += res.results[c]["out"].T
    out += bo[None, None, :]
    return out


# revision 21
# speedup vs baseline: 1.2783x; 1.1664x over previous
"""Multi-head attention (RoPE, causal) Trainium2 Bass kernel, 8-core SPMD.

Problem: B=2, S=2048, D=1024, H=16, DK=64, fp32, causal mask.

Sharding: core c handles batch b = c//4 and head group hg = c%4 (4 heads).
Each core computes Q/K/V projections for its 4 heads (column-sliced weights),
RoPE, causal attention, and a partial output projection (row-sliced Wo).
Host sums the 4 partial outputs per batch and adds the output bias.

Layout strategy (no on-device transposes):
  x^T [D, S] is precomputed on host; Q^T/K^T computed as [dk, S] tiles
  (weights stationary, x^T moving); scores computed transposed [k, q]
  (K^T stationary, Q^T moving); PV uses V in natural layout [k, dk+1]
  (stationary) with exp(scores^T) moving, accumulating attn^T [dk(+1), q];
  the ones column of V accumulates the softmax denominator. Normalization
  multiplies attn^T rows by broadcast 1/denom. Output projection uses
  attn^T as stationary and Wo as moving, producing natural [s, D] partials.

All matmul operands are float32r (TF32-like fast mode: 1 cycle/row at
moving-dim >= 256 vs 4 cycles/row for fp32): ~1e-4 relative L2 per matmul.
"""
import sys
sys.path.insert(0, "/opt/trn_rl_repo")
import math
import numpy as np

B, S, D, H, DK = 2, 2048, 1024, 16, 64
NCORES = 8
HPC = H // (NCORES // B)     # 4 heads per core
DHC = HPC * DK               # 256 attn dims per core
NPAIR = HPC // 2             # 2 head pairs per core
KC = D // 128                # 8 contraction chunks
NSB = S // 128               # 16 s-blocks / k-blocks
NQC = S // 512               # 4 q-chunks of 512

_BUILD_CACHE = {}


def _build(causal: bool):
    import concourse.tile as tile
    from concourse import bacc, mybir

    f32, f32r = mybir.dt.float32, mybir.dt.float32r
    MULT, ADD = mybir.AluOpType.mult, mybir.AluOpType.add
    EXP = mybir.ActivationFunctionType.Exp

    nc = bacc.Bacc(target_bir_lowering=False, trn_type="TRN2", debug=False)

    xT_d = nc.dram_tensor("xT", [D, S], f32r, kind="ExternalInput")
    wq_d = nc.dram_tensor("wq", [D, DHC], f32r, kind="ExternalInput")
    wk_d = nc.dram_tensor("wk", [D, DHC], f32r, kind="ExternalInput")
    wv_d = nc.dram_tensor("wv", [D, DHC], f32r, kind="ExternalInput")
    wo_d = nc.dram_tensor("wo", [DHC, D], f32r, kind="ExternalInput")
    bqk_d = nc.dram_tensor("bqk", [2, DHC], f32r, kind="ExternalInput")
    bv_d = nc.dram_tensor("bv", [1, DHC], f32r, kind="ExternalInput")
    ones_d = nc.dram_tensor("ones", [1, 512], f32r, kind="ExternalInput")
    psig_d = nc.dram_tensor("psig", [128, 128], f32r, kind="ExternalInput")
    rope_d = nc.dram_tensor("rope", [4, 128, S], f32, kind="ExternalInput")
    mdiag_d = nc.dram_tensor("mdiag", [128, 128], f32, kind="ExternalInput")
    out_d = nc.dram_tensor("out", [D, S], f32, kind="ExternalOutput")
    if not causal:
        maskT_d = nc.dram_tensor("maskT", [S, S], f32, kind="ExternalInput")

    with tile.TileContext(nc) as tc:
        with tc.tile_pool(name="const", bufs=1) as const_p, \
             tc.tile_pool(name="persist", bufs=1) as pers_p, \
             tc.tile_pool(name="ph2sb", bufs=1) as ph2_sb, \
             tc.tile_pool(name="ph3sb", bufs=1) as ph3_sb:

            # ---------- constants ----------
            ones_t = const_p.tile([1, 512], f32r, tag="ones")
            nc.sync.dma_start(out=ones_t, in_=ones_d[:])
            psig_t = const_p.tile([128, 128], f32r, tag="psig")
            nc.sync.dma_start(out=psig_t, in_=psig_d[:])
            # per-pair bias columns [128, 1] (partition = head-pair dims)
            bcol = {}
            for t_i in (0, 1):
                for p in range(NPAIR):
                    bt = const_p.tile([128, 1], f32r, tag=f"bc{t_i}{p}", name=f"bc{t_i}{p}")
                    src_row = bqk_d[t_i:t_i + 1, 128 * p:128 * (p + 1)]
                    nc.sync.dma_start(out=bt, in_=src_row.rearrange("o d -> d o"))
                    bcol[t_i, p] = bt.bitcast(f32)
            bv_t = const_p.tile([1, DHC], f32r, tag="bv")
            nc.sync.dma_start(out=bv_t, in_=bv_d[:])
            mdiag_t = const_p.tile([128, 128], f32, tag="mdiag")
            nc.sync.dma_start(out=mdiag_t, in_=mdiag_d[:])

            # ---------- persistent activations ----------
            qt_pair = [pers_p.tile([128, S], f32r, tag=f"qt{p}", name=f"qt{p}") for p in range(NPAIR)]
            kt_pair = [pers_p.tile([128, S], f32r, tag=f"kt{p}", name=f"kt{p}") for p in range(NPAIR)]
            v_sb = [pers_p.tile([128, HPC, DK + 1], f32r, tag=f"v{i}", name=f"v{i}") for i in range(NSB)]
            attnT_sb = [pers_p.tile([128, S], f32r, tag=f"at{p}", name=f"at{p}") for p in range(NPAIR)]

            # =========================================================
            # Phase 1: projections + RoPE + V assembly
            # =========================================================
            with tc.tile_pool(name="ph1sb", bufs=1) as ph1_sb, \
                 tc.tile_pool(name="ph1ps", bufs=1, space="PSUM") as ph1_ps:

                def load_xq(qc, tiles_only=False):
                    ql, qh = 512 * qc, 512 * (qc + 1)
                    ts = [ph1_sb.tile([128, 512], f32r, tag="xq", bufs=10, name=f"xq{kc}_{qc}")
                          for kc in range(KC)]
                    if not tiles_only:
                        for kc in range(KC):
                            nc.sync.dma_start(out=ts[kc], in_=xT_d[128 * kc:128 * (kc + 1), ql:qh])
                    return ts

                # q-chunks processed descending so that phase 2 (which walks
                # k-blocks descending) can start as soon as the tail chunk of
                # Q^T/K^T/V is ready.  First-chunk x slices and weights are
                # DMA'd interleaved per k-chunk so the first matmuls start
                # as soon as possible.
                qc_order = list(reversed(range(NQC)))
                w_t = {t_i: [ph1_sb.tile([128, DHC], f32r, tag=f"w{t_i}_{kc}", name=f"w{t_i}_{kc}")
                             for kc in range(KC)] for t_i in (0, 1)}
                wv_t = [ph1_sb.tile([128, DHC], f32r, tag=f"wv{kc}", name=f"wv{kc}")
                        for kc in range(KC)]
                def load_rope(qc):
                    ql = 512 * qc
                    ts = [ph1_sb.tile([128, 512], f32, tag="rope", bufs=6, name=f"rope{i}_{qc}")
                          for i in range(4)]
                    for i in range(4):
                        nc.sync.dma_start(out=ts[i], in_=rope_d[i][:, ql:ql + 512])
                    return ts

                xq_next = load_xq(qc_order[0], tiles_only=True)
                q3l = 512 * qc_order[0]
                for kc in range(KC):
                    nc.sync.dma_start(out=w_t[0][kc], in_=wq_d[128 * kc:128 * (kc + 1), :])
                    nc.sync.dma_start(out=w_t[1][kc], in_=wk_d[128 * kc:128 * (kc + 1), :])
                    nc.sync.dma_start(out=xq_next[kc], in_=xT_d[128 * kc:128 * (kc + 1), q3l:q3l + 512])
                    nc.sync.dma_start(out=wv_t[kc], in_=wv_d[128 * kc:128 * (kc + 1), :])
                rope_next = load_rope(qc_order[0])

                for qi, qc in enumerate(qc_order):
                    ql, qh = 512 * qc, 512 * (qc + 1)
                    xq = xq_next
                    rope_s = rope_next
                    pps = {}
                    for t_i in (0, 1):
                        for p in range(NPAIR):
                            pps[t_i, p] = ph1_ps.tile([128, 512], f32, tag="qtp",
                                                      bufs=4, name=f"pp{t_i}_{p}_{qc}")
                    for kc in range(KC):
                        for t_i in (0, 1):
                            for p in range(NPAIR):
                                nc.tensor.matmul(pps[t_i, p],
                                                 w_t[t_i][kc][:, 128 * p:128 * (p + 1)],
                                                 xq[kc], start=(kc == 0), stop=(kc == KC - 1))
                    if qi + 1 < NQC:
                        xq_next = load_xq(qc_order[qi + 1])
                        rope_next = load_rope(qc_order[qi + 1])
                    for t_i in (0, 1):
                        cos_t, sin_t = rope_s[2 * t_i], rope_s[2 * t_i + 1]
                        dst_pair = qt_pair if t_i == 0 else kt_pair
                        for p in range(NPAIR):
                            pp = pps[t_i, p]
                            # RoPE with bias folded in:
                            #   dst = (pp+b)*cos + Psig @ ((pp+b)*sin_sig)
                            u_t = ph1_sb.tile([128, 512], f32r, tag="u", bufs=2)
                            nc.vector.scalar_tensor_tensor(
                                out=u_t, in0=pp, scalar=bcol[t_i, p], in1=sin_t,
                                op0=ADD, op1=MULT)
                            us = ph1_ps.tile([128, 512], f32, tag="usp", bufs=2)
                            nc.tensor.matmul(us, psig_t, u_t, start=True, stop=True)
                            dst = dst_pair[p][:, ql:qh]
                            nc.vector.scalar_tensor_tensor(
                                out=dst, in0=pp, scalar=bcol[t_i, p], in1=cos_t,
                                op0=ADD, op1=MULT)
                            nc.vector.tensor_tensor(out=dst, in0=us, in1=dst.bitcast(f32), op=ADD)
                    # V projection for the 4 s-blocks covered by this q-chunk
                    for r in range(4):
                        si = 4 * qc + r
                        vp = ph1_ps.tile([128, DHC + HPC], f32, tag="vp", bufs=2)
                        for kc in range(KC):
                            nc.tensor.matmul(vp[:, 0:DHC], xq[kc][:, 128 * r:128 * (r + 1)],
                                             wv_t[kc], start=(kc == 0), stop=False)
                        nc.tensor.matmul(vp[:, 0:DHC], ones_t[:, 0:128], bv_t,
                                         start=False, stop=True)
                        nc.tensor.matmul(vp[:, DHC:DHC + HPC], ones_t[:, 0:128],
                                         ones_t[:, 0:HPC], start=True, stop=True)
                        nc.scalar.copy(out=v_sb[si][:, :, 0:DK],
                                       in_=vp[:, 0:DHC].rearrange("p (h d) -> p h d", h=HPC))
                        nc.scalar.copy(out=v_sb[si][:, :, DK:DK + 1],
                                       in_=vp[:, DHC:DHC + HPC].rearrange("p (h o) -> p h o", h=HPC))

            # =========================================================
            # Phase 2: attention per head
            # =========================================================
            HALF = S // 2
            with tc.tile_pool(name="ph2ps", bufs=1, space="PSUM") as ph2_ps:
                # Per (pair, q-half): the two heads of the pair run as two
                # interleaved dependency chains (separate scores psum + attn
                # accumulator each) so PE and ACT stay busy simultaneously.
                # k-blocks walk descending (matches phase-1 production order);
                # PV trails one item behind QK^T/exp.
                def emit_pv(h, at_ps, hlo, it, pT):
                    j, base, w, a0 = it
                    a = a0
                    while a < w:
                        bnd = min((a // 512 + 1) * 512, w)
                        sbank = (base + a) // 512
                        jmax = min(NSB - 1, 4 * sbank + 3) if causal else NSB - 1
                        nc.tensor.matmul(at_ps[:, base + a - hlo:base + bnd - hlo],
                                         v_sb[j][:, h, :], pT[:, a:bnd],
                                         start=(j == jmax), stop=(j == 0))
                        a = bnd

                for p in range(NPAIR):
                    for half in (1, 0):
                        hlo, hhi = HALF * half, HALF * (half + 1)
                        at_ps = [ph2_ps.tile([DK + 1, HALF], f32, tag="atp",
                                             bufs=2, name=f"atp{p}_{half}_{hh}")
                                 for hh in range(2)]
                        items = []
                        for j in reversed(range(NSB)):
                            qlo = max(128 * j, hlo) if causal else hlo
                            if qlo >= hhi:
                                continue
                            base = (qlo // 512) * 512
                            first = True
                            while base < hhi:
                                w = min(1024, hhi - base)
                                a0 = (qlo - base) if first else 0
                                items.append((j, base, w, a0))
                                base += w
                                first = False
                        pend = []
                        for it in items:
                            j, base, w, a0 = it
                            scs = []
                            for hh in range(2):
                                off = 64 * hh
                                sc = ph2_ps.tile([128, 1024], f32, tag="sc",
                                                 bufs=2, name=f"sc{hh}")
                                a = a0
                                while a < w:
                                    bnd = min((a // 512 + 1) * 512, w)
                                    nc.tensor.matmul(
                                        sc[:, a:bnd],
                                        kt_pair[p][off:off + 64, 128 * j:128 * (j + 1)],
                                        qt_pair[p][off:off + 64, base + a:base + bnd],
                                        start=True, stop=True)
                                    a = bnd
                                scs.append(sc)
                            if pend:
                                for (pit, phh, ppT) in pend:
                                    emit_pv(2 * p + phh, at_ps[phh], hlo, pit, ppT)
                                pend = []
                            for hh in range(2):
                                sc = scs[hh]
                                if not causal:
                                    mt = ph2_sb.tile([128, 1024], f32, tag="mt", bufs=3)
                                    nc.sync.dma_start(
                                        out=mt[:, a0:w],
                                        in_=maskT_d[128 * j:128 * (j + 1), base + a0:base + w])
                                    nc.vector.tensor_tensor(
                                        out=sc[:, a0:w], in0=sc[:, a0:w],
                                        in1=mt[:, a0:w], op=ADD)
                                pT = ph2_sb.tile([128, 1024], f32r, tag="pT", bufs=4)
                                nc.scalar.activation(out=pT[:, a0:w], in_=sc[:, a0:w], func=EXP)
                                if causal and base <= 128 * j < base + w:
                                    dc = 128 * j - base
                                    nc.vector.tensor_tensor(
                                        out=pT[:, dc:dc + 128],
                                        in0=pT[:, dc:dc + 128].bitcast(f32),
                                        in1=mdiag_t, op=MULT)
                                pend.append((it, hh, pT))
                        for (pit, phh, ppT) in pend:
                            emit_pv(2 * p + phh, at_ps[phh], hlo, pit, ppT)
                        for hh in range(2):
                            off = 64 * hh
                            rec = ph2_sb.tile([1, HALF], f32, tag="rec", bufs=2)
                            nc.vector.reciprocal(rec, at_ps[hh][DK:DK + 1, :])
                            bc = ph2_sb.tile([64, HALF], f32, tag="bc", bufs=2)
                            nc.gpsimd.partition_broadcast(bc, rec)
                            nc.vector.tensor_tensor(out=attnT_sb[p][off:off + 64, hlo:hhi],
                                                    in0=at_ps[hh][0:DK, :], in1=bc, op=MULT)

            # =========================================================
            # Phase 3: output projection (partial; host sums cores + bias)
            # =========================================================
            # output projection computed transposed: out^T[dout, s] so the
            # stationary operand (Wo chunk) is reused across the whole s sweep
            # (one weight load per (dout-block, chunk) instead of per matmul).
            # The host transposes the [D, S] partial back.
            with tc.tile_pool(name="ph3ps", bufs=1, space="PSUM") as ph3_ps:
                wo_t = [ph3_sb.tile([128, D], f32r, tag=f"wo{ch}", name=f"wo{ch}") for ch in range(NPAIR)]
                for ch in range(NPAIR):
                    nc.sync.dma_start(out=wo_t[ch], in_=wo_d[128 * ch:128 * (ch + 1), :])
                for do in range(D // 128):          # 8 dout blocks
                    ops = [ph3_ps.tile([128, 512], f32, tag="op", bufs=8, name=f"op{do}_{sc_}")
                           for sc_ in range(NQC)]
                    for ch in range(NPAIR):
                        for sc_ in range(NQC):
                            nc.tensor.matmul(ops[sc_],
                                             wo_t[ch][:, 128 * do:128 * (do + 1)],
                                             attnT_sb[ch][:, 512 * sc_:512 * (sc_ + 1)],
                                             start=(ch == 0), stop=(ch == NPAIR - 1))
                    for sc_ in range(NQC):
                        ob = ph3_sb.tile([128, 512], f32, tag="ob", bufs=8)
                        if sc_ % 2 == 0:
                            nc.vector.tensor_copy(ob, ops[sc_])
                        else:
                            nc.scalar.copy(out=ob, in_=ops[sc_])
                        nc.sync.dma_start(out=out_d[128 * do:128 * (do + 1), 512 * sc_:512 * (sc_ + 1)],
                                          in_=ob)

    nc.compile()
    return nc


def _rope_tables():
    half = DK // 2
    freqs = (10000.0 ** (-2.0 / DK * np.arange(half, dtype=np.float32))).astype(np.float64)
    ang = np.outer(np.arange(S, dtype=np.float64), freqs)           # [S, 32]
    cos1 = np.cos(ang).T.astype(np.float32)                          # [32, S]
    sin1 = np.sin(ang).T.astype(np.float32)
    c64 = np.concatenate([cos1, cos1], axis=0)                       # [64, S]
    ssig64 = np.concatenate([sin1, -sin1], axis=0)                   # s-tilde(sigma(p))
    c128 = np.concatenate([c64, c64], axis=0)
    ssig128 = np.concatenate([ssig64, ssig64], axis=0)
    scale = np.float32(1.0 / math.sqrt(DK))
    return np.stack([c128 * scale, ssig128 * scale, c128, ssig128]).astype(np.float32)


def _psig():
    p64 = np.zeros((64, 64), np.float32)
    p64[np.arange(32) + 32, np.arange(32)] = 1.0
    p64[np.arange(32), np.arange(32) + 32] = 1.0
    p = np.zeros((128, 128), np.float32)
    p[0:64, 0:64] = p64
    p[64:128, 64:128] = p64
    return p


def _make_runner(nc, n_cores=NCORES):
    """Compile the SPMD program once into a reusable jitted shard_map callable
    (same execution path as bass_utils.run_bass_kernel_spmd under axon)."""
    import jax
    from jax.sharding import Mesh, PartitionSpec
    from jax.experimental.shard_map import shard_map
    from concourse import bass2jax, mybir
    from concourse.bass2jax import _bass_exec_p, install_neuronx_cc_hook

    install_neuronx_cc_hook()
    partition_name = nc.partition_id_tensor.name if nc.partition_id_tensor else None
    in_names, out_names, out_avals, zero_outs = [], [], [], []
    for alloc in nc.m.functions[0].allocations:
        if not isinstance(alloc, mybir.MemoryLocationSet):
            continue
        name = alloc.memorylocations[0].name
        if alloc.kind == "ExternalInput":
            if name != partition_name:
                in_names.append(name)
        elif alloc.kind == "ExternalOutput":
            out_names.append(name)
            shape = tuple(alloc.tensor_shape)
            dtype = mybir.dt.np(alloc.dtype)
            out_avals.append(jax.core.ShapedArray(shape, dtype))
            zero_outs.append(np.zeros(shape, dtype))
    n_params = len(in_names)
    all_in = in_names + out_names
    if partition_name is not None:
        all_in.append(partition_name)

    def _body(*args):
        operands = list(args)
        if partition_name is not None:
            operands.append(bass2jax.partition_id_tensor())
        outs = _bass_exec_p.bind(
            *operands, out_avals=tuple(out_avals), in_names=tuple(all_in),
            out_names=tuple(out_names), lowering_input_output_aliases=(),
            sim_require_finite=True, sim_require_nnan=True, nc=nc)
        return tuple(outs)

    devices = jax.devices()[:n_cores]
    mesh = Mesh(np.asarray(devices), ("core",))
    specs = (PartitionSpec("core"),) * (n_params + len(out_names))
    out_specs = (PartitionSpec("core"),) * len(out_names)
    fn = jax.jit(shard_map(_body, mesh=mesh, in_specs=specs,
                           out_specs=out_specs, check_rep=False),
                 keep_unused=True)
    concat_zeros = [np.zeros((n_cores * z.shape[0], *z.shape[1:]), z.dtype)
                    for z in zero_outs]

    def run(in_maps):
        concat_in = [np.concatenate([np.asarray(in_maps[c][k]) for c in range(n_cores)],
                                    axis=0) for k in in_names]
        outs = fn(*concat_in, *concat_zeros)
        o = np.asarray(outs[out_names.index("out")])
        return o.reshape(n_cores, *zero_outs[out_names.index("out")].shape)

    return run


def kernel(x, mask, Wq, bq, Wk, bk, Wv, bv, Wo, bo):
    x = np.asarray(x, dtype=np.float32)
    mask = np.asarray(mask)
    Wq, bq = np.asarray(Wq, np.float32), np.asarray(bq, np.float32)
    Wk, bk = np.asarray(Wk, np.float32), np.asarray(bk, np.float32)
    Wv, bv = np.asarray(Wv, np.float32), np.asarray(bv, np.float32)
    Wo, bo = np.asarray(Wo, np.float32), np.asarray(bo, np.float32)

    causal_ref = np.triu(np.ones((S, S), dtype=bool), k=1)
    m2 = np.broadcast_to(mask, (B, 1, S, S))[:, 0]
    causal = all(np.array_equal(m2[b], causal_ref) for b in range(B))

    if causal not in _BUILD_CACHE:
        nc = _build(causal)
        _BUILD_CACHE[causal] = (nc, _make_runner(nc))
    nc, run = _BUILD_CACHE[causal]

    rope = _rope_tables()
    psig = _psig()
    ones = np.ones((1, 512), np.float32)
    # multiplicative 0/1 mask for the diagonal block (applied to exp(scores))
    mdiag = np.where(np.arange(128)[:, None] > np.arange(128)[None, :],
                     np.float32(0.0), np.float32(1.0)).astype(np.float32)

    xT = [np.ascontiguousarray(x[b].T) for b in range(B)]
    maskT = None
    if not causal:
        maskT = [np.ascontiguousarray(
            np.where(m2[b], np.float32(-1e30), np.float32(0.0)).T) for b in range(B)]

    in_maps = []
    for c in range(NCORES):
        b, hg = c // (NCORES // B), c % (NCORES // B)
        cs = slice(DHC * hg, DHC * (hg + 1))
        im = {
            "xT": xT[b],
            "wq": np.ascontiguousarray(Wq[:, cs]),
            "wk": np.ascontiguousarray(Wk[:, cs]),
            "wv": np.ascontiguousarray(Wv[:, cs]),
            "wo": np.ascontiguousarray(Wo[cs, :]),
            "bqk": np.ascontiguousarray(np.stack([bq[cs], bk[cs]])),
            "bv": np.ascontiguousarray(bv[cs][None, :]),
            "ones": ones, "psig": psig, "rope": rope, "mdiag": mdiag,
        }
        if not causal:
            im["maskT"] = maskT[b]
        in_maps.append(im)

    partials = run(in_maps)
    out = np.zeros((B, S, D), np.float32)
    for c in range(NCORES):
        out[c // (NCORES // B)] += partials[c].T
    out += bo[None, None, :]
    return out


# revision 22
# speedup vs baseline: 1.3035x; 1.0197x over previous
"""Multi-head attention (RoPE, causal) Trainium2 Bass kernel, 8-core SPMD.

Problem: B=2, S=2048, D=1024, H=16, DK=64, fp32, causal mask.

Sharding: core c handles batch b = c//4 and head group hg = c%4 (4 heads).
Each core computes Q/K/V projections for its 4 heads (column-sliced weights),
RoPE, causal attention, and a partial output projection (row-sliced Wo).
Host sums the 4 partial outputs per batch and adds the output bias.

Layout strategy (no on-device transposes):
  x^T [D, S] is precomputed on host; Q^T/K^T computed as [dk, S] tiles
  (weights stationary, x^T moving); scores computed transposed [k, q]
  (K^T stationary, Q^T moving); PV uses V in natural layout [k, dk+1]
  (stationary) with exp(scores^T) moving, accumulating attn^T [dk(+1), q];
  the ones column of V accumulates the softmax denominator. Normalization
  multiplies attn^T rows by broadcast 1/denom. Output projection uses
  attn^T as stationary and Wo as moving, producing natural [s, D] partials.

All matmul operands are float32r (TF32-like fast mode: 1 cycle/row at
moving-dim >= 256 vs 4 cycles/row for fp32): ~1e-4 relative L2 per matmul.
"""
import sys
sys.path.insert(0, "/opt/trn_rl_repo")
import math
import numpy as np

B, S, D, H, DK = 2, 2048, 1024, 16, 64
NCORES = 8
HPC = H // (NCORES // B)     # 4 heads per core
DHC = HPC * DK               # 256 attn dims per core
NPAIR = HPC // 2             # 2 head pairs per core
KC = D // 128                # 8 contraction chunks
NSB = S // 128               # 16 s-blocks / k-blocks
NQC = S // 512               # 4 q-chunks of 512

_BUILD_CACHE = {}


def _build(causal: bool):
    import concourse.tile as tile
    from concourse import bacc, mybir

    f32, f32r = mybir.dt.float32, mybir.dt.float32r
    MULT, ADD = mybir.AluOpType.mult, mybir.AluOpType.add
    EXP = mybir.ActivationFunctionType.Exp

    nc = bacc.Bacc(target_bir_lowering=False, trn_type="TRN2", debug=False)

    xT_d = nc.dram_tensor("xT", [D, S], f32r, kind="ExternalInput")
    wq_d = nc.dram_tensor("wq", [D, DHC], f32r, kind="ExternalInput")
    wk_d = nc.dram_tensor("wk", [D, DHC], f32r, kind="ExternalInput")
    wv_d = nc.dram_tensor("wv", [D, DHC], f32r, kind="ExternalInput")
    wo_d = nc.dram_tensor("wo", [DHC, D], f32r, kind="ExternalInput")
    bqk_d = nc.dram_tensor("bqk", [2, DHC], f32r, kind="ExternalInput")
    bv_d = nc.dram_tensor("bv", [1, DHC], f32r, kind="ExternalInput")
    ones_d = nc.dram_tensor("ones", [1, 512], f32r, kind="ExternalInput")
    psig_d = nc.dram_tensor("psig", [128, 128], f32r, kind="ExternalInput")
    rope_d = nc.dram_tensor("rope", [2, 64, S], f32, kind="ExternalInput")
    mdiag_d = nc.dram_tensor("mdiag", [128, 128], f32, kind="ExternalInput")
    out_d = nc.dram_tensor("out", [D, S], f32, kind="ExternalOutput")
    if not causal:
        maskT_d = nc.dram_tensor("maskT", [S, S], f32, kind="ExternalInput")

    with tile.TileContext(nc) as tc:
        with tc.tile_pool(name="const", bufs=1) as const_p, \
             tc.tile_pool(name="persist", bufs=1) as pers_p, \
             tc.tile_pool(name="ph2sb", bufs=1) as ph2_sb, \
             tc.tile_pool(name="ph3sb", bufs=1) as ph3_sb:

            # ---------- constants ----------
            ones_t = const_p.tile([1, 512], f32r, tag="ones")
            nc.sync.dma_start(out=ones_t, in_=ones_d[:])
            psig_t = const_p.tile([128, 128], f32r, tag="psig")
            nc.sync.dma_start(out=psig_t, in_=psig_d[:])
            # per-pair bias columns [128, 1] (partition = head-pair dims)
            bcol = {}
            for t_i in (0, 1):
                for p in range(NPAIR):
                    bt = const_p.tile([128, 1], f32r, tag=f"bc{t_i}{p}", name=f"bc{t_i}{p}")
                    src_row = bqk_d[t_i:t_i + 1, 128 * p:128 * (p + 1)]
                    nc.sync.dma_start(out=bt, in_=src_row.rearrange("o d -> d o"))
                    bcol[t_i, p] = bt.bitcast(f32)
            bv_t = const_p.tile([1, DHC], f32r, tag="bv")
            nc.sync.dma_start(out=bv_t, in_=bv_d[:])
            mdiag_t = const_p.tile([128, 128], f32, tag="mdiag")
            nc.sync.dma_start(out=mdiag_t, in_=mdiag_d[:])

            # ---------- persistent activations ----------
            qt_pair = [pers_p.tile([128, S], f32r, tag=f"qt{p}", name=f"qt{p}") for p in range(NPAIR)]
            kt_pair = [pers_p.tile([128, S], f32r, tag=f"kt{p}", name=f"kt{p}") for p in range(NPAIR)]
            v_sb = [pers_p.tile([128, HPC, DK + 1], f32r, tag=f"v{i}", name=f"v{i}") for i in range(NSB)]
            attnT_sb = [pers_p.tile([128, S], f32r, tag=f"at{p}", name=f"at{p}") for p in range(NPAIR)]

            # =========================================================
            # Phase 1: projections + RoPE + V assembly
            # =========================================================
            with tc.tile_pool(name="ph1sb", bufs=1) as ph1_sb, \
                 tc.tile_pool(name="ph1ps", bufs=1, space="PSUM") as ph1_ps:

                def load_xq(qc, tiles_only=False):
                    ql, qh = 512 * qc, 512 * (qc + 1)
                    ts = [ph1_sb.tile([128, 512], f32r, tag="xq", bufs=10, name=f"xq{kc}_{qc}")
                          for kc in range(KC)]
                    if not tiles_only:
                        for kc in range(KC):
                            nc.sync.dma_start(out=ts[kc], in_=xT_d[128 * kc:128 * (kc + 1), ql:qh])
                    return ts

                # q-chunks processed descending so that phase 2 (which walks
                # k-blocks descending) can start as soon as the tail chunk of
                # Q^T/K^T/V is ready.  First-chunk x slices and weights are
                # DMA'd interleaved per k-chunk so the first matmuls start
                # as soon as possible.
                qc_order = list(reversed(range(NQC)))
                w_t = {t_i: [ph1_sb.tile([128, DHC], f32r, tag=f"w{t_i}_{kc}", name=f"w{t_i}_{kc}")
                             for kc in range(KC)] for t_i in (0, 1)}
                wv_t = [ph1_sb.tile([128, DHC], f32r, tag=f"wv{kc}", name=f"wv{kc}")
                        for kc in range(KC)]
                def load_rope(qc):
                    # [64, 512] source pair-stacked to 128 partitions via two DMAs
                    ql = 512 * qc
                    ts = [ph1_sb.tile([128, 512], f32, tag="rope", bufs=4, name=f"rope{i}_{qc}")
                          for i in range(2)]
                    for i in range(2):
                        nc.sync.dma_start(out=ts[i][0:64, :], in_=rope_d[i][:, ql:ql + 512])
                        nc.sync.dma_start(out=ts[i][64:128, :], in_=rope_d[i][:, ql:ql + 512])
                    return ts

                xq_next = load_xq(qc_order[0], tiles_only=True)
                q3l = 512 * qc_order[0]
                for kc in range(KC):
                    nc.sync.dma_start(out=w_t[0][kc], in_=wq_d[128 * kc:128 * (kc + 1), :])
                    nc.sync.dma_start(out=w_t[1][kc], in_=wk_d[128 * kc:128 * (kc + 1), :])
                    nc.sync.dma_start(out=xq_next[kc], in_=xT_d[128 * kc:128 * (kc + 1), q3l:q3l + 512])
                    nc.sync.dma_start(out=wv_t[kc], in_=wv_d[128 * kc:128 * (kc + 1), :])
                rope_next = load_rope(qc_order[0])

                for qi, qc in enumerate(qc_order):
                    ql, qh = 512 * qc, 512 * (qc + 1)
                    xq = xq_next
                    rope_s = rope_next
                    pps = {}
                    for t_i in (0, 1):
                        for p in range(NPAIR):
                            pps[t_i, p] = ph1_ps.tile([128, 512], f32, tag="qtp",
                                                      bufs=4, name=f"pp{t_i}_{p}_{qc}")
                    for kc in range(KC):
                        for t_i in (0, 1):
                            for p in range(NPAIR):
                                nc.tensor.matmul(pps[t_i, p],
                                                 w_t[t_i][kc][:, 128 * p:128 * (p + 1)],
                                                 xq[kc], start=(kc == 0), stop=(kc == KC - 1))
                    if qi + 1 < NQC:
                        xq_next = load_xq(qc_order[qi + 1])
                        rope_next = load_rope(qc_order[qi + 1])
                    cos_t, sin_t = rope_s
                    for t_i in (0, 1):
                        dst_pair = qt_pair if t_i == 0 else kt_pair
                        for p in range(NPAIR):
                            pp = pps[t_i, p]
                            # RoPE with bias folded in:
                            #   dst = (pp+b)*cos + Psig @ ((pp+b)*sin_sig)
                            u_t = ph1_sb.tile([128, 512], f32r, tag="u", bufs=2)
                            nc.vector.scalar_tensor_tensor(
                                out=u_t, in0=pp, scalar=bcol[t_i, p], in1=sin_t,
                                op0=ADD, op1=MULT)
                            us = ph1_ps.tile([128, 512], f32, tag="usp", bufs=2)
                            nc.tensor.matmul(us, psig_t, u_t, start=True, stop=True)
                            dst = dst_pair[p][:, ql:qh]
                            nc.vector.scalar_tensor_tensor(
                                out=dst, in0=pp, scalar=bcol[t_i, p], in1=cos_t,
                                op0=ADD, op1=MULT)
                            nc.vector.tensor_tensor(out=dst, in0=us, in1=dst.bitcast(f32), op=ADD)
                    # V projection for the 4 s-blocks covered by this q-chunk
                    for r in range(4):
                        si = 4 * qc + r
                        vp = ph1_ps.tile([128, DHC + HPC], f32, tag="vp", bufs=2)
                        for kc in range(KC):
                            nc.tensor.matmul(vp[:, 0:DHC], xq[kc][:, 128 * r:128 * (r + 1)],
                                             wv_t[kc], start=(kc == 0), stop=False)
                        nc.tensor.matmul(vp[:, 0:DHC], ones_t[:, 0:128], bv_t,
                                         start=False, stop=True)
                        nc.tensor.matmul(vp[:, DHC:DHC + HPC], ones_t[:, 0:128],
                                         ones_t[:, 0:HPC], start=True, stop=True)
                        nc.scalar.copy(out=v_sb[si][:, :, 0:DK],
                                       in_=vp[:, 0:DHC].rearrange("p (h d) -> p h d", h=HPC))
                        nc.scalar.copy(out=v_sb[si][:, :, DK:DK + 1],
                                       in_=vp[:, DHC:DHC + HPC].rearrange("p (h o) -> p h o", h=HPC))

            # =========================================================
            # Phase 2: attention per head
            # =========================================================
            HALF = S // 2
            with tc.tile_pool(name="ph2ps", bufs=1, space="PSUM") as ph2_ps:
                # Per (pair, q-half): the two heads of the pair run as two
                # interleaved dependency chains (separate scores psum + attn
                # accumulator each) so PE and ACT stay busy simultaneously.
                # k-blocks walk descending (matches phase-1 production order);
                # PV trails one item behind QK^T/exp.
                def emit_pv(h, at_ps, hlo, it, pT):
                    j, base, w, a0 = it
                    a = a0
                    while a < w:
                        bnd = min((a // 512 + 1) * 512, w)
                        sbank = (base + a) // 512
                        jmax = min(NSB - 1, 4 * sbank + 3) if causal else NSB - 1
                        nc.tensor.matmul(at_ps[:, base + a - hlo:base + bnd - hlo],
                                         v_sb[j][:, h, :], pT[:, a:bnd],
                                         start=(j == jmax), stop=(j == 0))
                        a = bnd

                for p in range(NPAIR):
                    for half in (1, 0):
                        hlo, hhi = HALF * half, HALF * (half + 1)
                        at_ps = [ph2_ps.tile([DK + 1, HALF], f32, tag="atp",
                                             bufs=2, name=f"atp{p}_{half}_{hh}")
                                 for hh in range(2)]
                        items = []
                        for j in reversed(range(NSB)):
                            qlo = max(128 * j, hlo) if causal else hlo
                            if qlo >= hhi:
                                continue
                            base = (qlo // 512) * 512
                            first = True
                            while base < hhi:
                                w = min(1024, hhi - base)
                                a0 = (qlo - base) if first else 0
                                items.append((j, base, w, a0))
                                base += w
                                first = False
                        pend = []
                        for it in items:
                            j, base, w, a0 = it
                            scs = []
                            for hh in range(2):
                                off = 64 * hh
                                sc = ph2_ps.tile([128, 1024], f32, tag="sc",
                                                 bufs=2, name=f"sc{hh}")
                                a = a0
                                while a < w:
                                    bnd = min((a // 512 + 1) * 512, w)
                                    nc.tensor.matmul(
                                        sc[:, a:bnd],
                                        kt_pair[p][off:off + 64, 128 * j:128 * (j + 1)],
                                        qt_pair[p][off:off + 64, base + a:base + bnd],
                                        start=True, stop=True)
                                    a = bnd
                                scs.append(sc)
                            if pend:
                                for (pit, phh, ppT) in pend:
                                    emit_pv(2 * p + phh, at_ps[phh], hlo, pit, ppT)
                                pend = []
                            for hh in range(2):
                                sc = scs[hh]
                                if not causal:
                                    mt = ph2_sb.tile([128, 1024], f32, tag="mt", bufs=3)
                                    nc.sync.dma_start(
                                        out=mt[:, a0:w],
                                        in_=maskT_d[128 * j:128 * (j + 1), base + a0:base + w])
                                    nc.vector.tensor_tensor(
                                        out=sc[:, a0:w], in0=sc[:, a0:w],
                                        in1=mt[:, a0:w], op=ADD)
                                pT = ph2_sb.tile([128, 1024], f32r, tag="pT", bufs=4)
                                nc.scalar.activation(out=pT[:, a0:w], in_=sc[:, a0:w], func=EXP)
                                if causal and base <= 128 * j < base + w:
                                    dc = 128 * j - base
                                    nc.vector.tensor_tensor(
                                        out=pT[:, dc:dc + 128],
                                        in0=pT[:, dc:dc + 128].bitcast(f32),
                                        in1=mdiag_t, op=MULT)
                                pend.append((it, hh, pT))
                        for (pit, phh, ppT) in pend:
                            emit_pv(2 * p + phh, at_ps[phh], hlo, pit, ppT)
                        for hh in range(2):
                            off = 64 * hh
                            rec = ph2_sb.tile([1, HALF], f32, tag="rec", bufs=2)
                            nc.vector.reciprocal(rec, at_ps[hh][DK:DK + 1, :])
                            bc = ph2_sb.tile([64, HALF], f32, tag="bc", bufs=2)
                            nc.gpsimd.partition_broadcast(bc, rec)
                            nc.vector.tensor_tensor(out=attnT_sb[p][off:off + 64, hlo:hhi],
                                                    in0=at_ps[hh][0:DK, :], in1=bc, op=MULT)

            # =========================================================
            # Phase 3: output projection (partial; host sums cores + bias)
            # =========================================================
            # output projection computed transposed: out^T[dout, s] so the
            # stationary operand (Wo chunk) is reused across the whole s sweep
            # (one weight load per (dout-block, chunk) instead of per matmul).
            # The host transposes the [D, S] partial back.
            with tc.tile_pool(name="ph3ps", bufs=1, space="PSUM") as ph3_ps:
                wo_t = [ph3_sb.tile([128, D], f32r, tag=f"wo{ch}", name=f"wo{ch}") for ch in range(NPAIR)]
                for ch in range(NPAIR):
                    nc.sync.dma_start(out=wo_t[ch], in_=wo_d[128 * ch:128 * (ch + 1), :])
                for do in range(D // 128):          # 8 dout blocks
                    ops = [ph3_ps.tile([128, 512], f32, tag="op", bufs=8, name=f"op{do}_{sc_}")
                           for sc_ in range(NQC)]
                    for ch in range(NPAIR):
                        for sc_ in range(NQC):
                            nc.tensor.matmul(ops[sc_],
                                             wo_t[ch][:, 128 * do:128 * (do + 1)],
                                             attnT_sb[ch][:, 512 * sc_:512 * (sc_ + 1)],
                                             start=(ch == 0), stop=(ch == NPAIR - 1))
                    for sc_ in range(NQC):
                        ob = ph3_sb.tile([128, 512], f32, tag="ob", bufs=8)
                        if sc_ % 2 == 0:
                            nc.vector.tensor_copy(ob, ops[sc_])
                        else:
                            nc.scalar.copy(out=ob, in_=ops[sc_])
                        nc.sync.dma_start(out=out_d[128 * do:128 * (do + 1), 512 * sc_:512 * (sc_ + 1)],
                                          in_=ob)

    nc.compile()
    return nc


def _rope_tables():
    # [2, 64, S]: cos and sigma-permuted signed sin, one head's worth; the
    # device pair-stacks to 128 partitions. The 1/sqrt(dk) score scale is
    # folded into Wq/bq on the host, so Q and K share these tables.
    half = DK // 2
    freqs = (10000.0 ** (-2.0 / DK * np.arange(half, dtype=np.float32))).astype(np.float64)
    ang = np.outer(np.arange(S, dtype=np.float64), freqs)           # [S, 32]
    cos1 = np.cos(ang).T.astype(np.float32)                          # [32, S]
    sin1 = np.sin(ang).T.astype(np.float32)
    c64 = np.concatenate([cos1, cos1], axis=0)                       # [64, S]
    ssig64 = np.concatenate([sin1, -sin1], axis=0)                   # s-tilde(sigma(p))
    return np.stack([c64, ssig64]).astype(np.float32)


def _psig():
    p64 = np.zeros((64, 64), np.float32)
    p64[np.arange(32) + 32, np.arange(32)] = 1.0
    p64[np.arange(32), np.arange(32) + 32] = 1.0
    p = np.zeros((128, 128), np.float32)
    p[0:64, 0:64] = p64
    p[64:128, 64:128] = p64
    return p


def _make_runner(nc, n_cores=NCORES):
    """Compile the SPMD program once into a reusable jitted shard_map callable
    (same execution path as bass_utils.run_bass_kernel_spmd under axon)."""
    import jax
    from jax.sharding import Mesh, PartitionSpec
    from jax.experimental.shard_map import shard_map
    from concourse import bass2jax, mybir
    from concourse.bass2jax import _bass_exec_p, install_neuronx_cc_hook

    install_neuronx_cc_hook()
    partition_name = nc.partition_id_tensor.name if nc.partition_id_tensor else None
    in_names, out_names, out_avals, zero_outs = [], [], [], []
    for alloc in nc.m.functions[0].allocations:
        if not isinstance(alloc, mybir.MemoryLocationSet):
            continue
        name = alloc.memorylocations[0].name
        if alloc.kind == "ExternalInput":
            if name != partition_name:
                in_names.append(name)
        elif alloc.kind == "ExternalOutput":
            out_names.append(name)
            shape = tuple(alloc.tensor_shape)
            dtype = mybir.dt.np(alloc.dtype)
            out_avals.append(jax.core.ShapedArray(shape, dtype))
            zero_outs.append(np.zeros(shape, dtype))
    n_params = len(in_names)
    all_in = in_names + out_names
    if partition_name is not None:
        all_in.append(partition_name)

    def _body(*args):
        operands = list(args)
        if partition_name is not None:
            operands.append(bass2jax.partition_id_tensor())
        outs = _bass_exec_p.bind(
            *operands, out_avals=tuple(out_avals), in_names=tuple(all_in),
            out_names=tuple(out_names), lowering_input_output_aliases=(),
            sim_require_finite=True, sim_require_nnan=True, nc=nc)
        return tuple(outs)

    devices = jax.devices()[:n_cores]
    mesh = Mesh(np.asarray(devices), ("core",))
    specs = (PartitionSpec("core"),) * (n_params + len(out_names))
    out_specs = (PartitionSpec("core"),) * len(out_names)
    fn = jax.jit(shard_map(_body, mesh=mesh, in_specs=specs,
                           out_specs=out_specs, check_rep=False),
                 keep_unused=True)
    concat_zeros = [np.zeros((n_cores * z.shape[0], *z.shape[1:]), z.dtype)
                    for z in zero_outs]

    def run(in_maps):
        concat_in = [np.concatenate([np.asarray(in_maps[c][k]) for c in range(n_cores)],
                                    axis=0) for k in in_names]
        outs = fn(*concat_in, *concat_zeros)
        o = np.asarray(outs[out_names.index("out")])
        return o.reshape(n_cores, *zero_outs[out_names.index("out")].shape)

    return run


def kernel(x, mask, Wq, bq, Wk, bk, Wv, bv, Wo, bo):
    x = np.asarray(x, dtype=np.float32)
    mask = np.asarray(mask)
    Wq, bq = np.asarray(Wq, np.float32), np.asarray(bq, np.float32)
    Wk, bk = np.asarray(Wk, np.float32), np.asarray(bk, np.float32)
    Wv, bv = np.asarray(Wv, np.float32), np.asarray(bv, np.float32)
    Wo, bo = np.asarray(Wo, np.float32), np.asarray(bo, np.float32)

    causal_ref = np.triu(np.ones((S, S), dtype=bool), k=1)
    m2 = np.broadcast_to(mask, (B, 1, S, S))[:, 0]
    causal = all(np.array_equal(m2[b], causal_ref) for b in range(B))

    if causal not in _BUILD_CACHE:
        nc = _build(causal)
        _BUILD_CACHE[causal] = (nc, _make_runner(nc))
    nc, run = _BUILD_CACHE[causal]

    rope = _rope_tables()
    psig = _psig()
    ones = np.ones((1, 512), np.float32)
    # multiplicative 0/1 mask for the diagonal block (applied to exp(scores))
    mdiag = np.where(np.arange(128)[:, None] > np.arange(128)[None, :],
                     np.float32(0.0), np.float32(1.0)).astype(np.float32)

    xT = [np.ascontiguousarray(x[b].T) for b in range(B)]
    maskT = None
    if not causal:
        maskT = [np.ascontiguousarray(
            np.where(m2[b], np.float32(-1e30), np.float32(0.0)).T) for b in range(B)]

    in_maps = []
    for c in range(NCORES):
        b, hg = c // (NCORES // B), c % (NCORES // B)
        cs = slice(DHC * hg, DHC * (hg + 1))
        im = {
            "xT": xT[b],
            "wq": np.ascontiguousarray(Wq[:, cs] * np.float32(1.0 / math.sqrt(DK))),
            "wk": np.ascontiguousarray(Wk[:, cs]),
            "wv": np.ascontiguousarray(Wv[:, cs]),
            "wo": np.ascontiguousarray(Wo[cs, :]),
            "bqk": np.ascontiguousarray(np.stack([bq[cs] * np.float32(1.0 / math.sqrt(DK)), bk[cs]])),
            "bv": np.ascontiguousarray(bv[cs][None, :]),
            "ones": ones, "psig": psig, "rope": rope, "mdiag": mdiag,
        }
        if not causal:
            im["maskT"] = maskT[b]
        in_maps.append(im)

    partials = run(in_maps)
    out = np.zeros((B, S, D), np.float32)
    for c in range(NCORES):
        out[c // (NCORES // B)] += partials[c].T
    out += bo[None, None, :]
    return out


# revision 23
# speedup vs baseline: 20831.3848x; 15981.3966x over previous
"""Multi-head attention (RoPE, causal) Trainium2 Bass kernel, 8-core SPMD.

Problem: B=2, S=2048, D=1024, H=16, DK=64, fp32, causal mask.

Sharding: core c handles batch b = c//4 and head group hg = c%4 (4 heads).
Each core computes Q/K/V projections for its 4 heads (column-sliced weights),
RoPE, causal attention, and a partial output projection (row-sliced Wo).
Host sums the 4 partial outputs per batch and adds the output bias.

Layout strategy (no on-device transposes):
  x^T [D, S] is precomputed on host; Q^T/K^T computed as [dk, S] tiles
  (weights stationary, x^T moving); scores computed transposed [k, q]
  (K^T stationary, Q^T moving); PV uses V in natural layout [k, dk+1]
  (stationary) with exp(scores^T) moving, accumulating attn^T [dk(+1), q];
  the ones column of V accumulates the softmax denominator. Normalization
  multiplies attn^T rows by broadcast 1/denom. Output projection uses
  attn^T as stationary and Wo as moving, producing natural [s, D] partials.

All matmul operands are float32r (TF32-like fast mode: 1 cycle/row at
moving-dim >= 256 vs 4 cycles/row for fp32): ~1e-4 relative L2 per matmul.
"""
import sys
sys.path.insert(0, "/opt/trn_rl_repo")
import math
import numpy as np

B, S, D, H, DK = 2, 2048, 1024, 16, 64
NCORES = 8
HPC = H // (NCORES // B)     # 4 heads per core
DHC = HPC * DK               # 256 attn dims per core
NPAIR = HPC // 2             # 2 head pairs per core
KC = D // 128                # 8 contraction chunks
NSB = S // 128               # 16 s-blocks / k-blocks
NQC = S // 512               # 4 q-chunks of 512

_BUILD_CACHE = {}


def _build(causal: bool):
    import concourse.tile as tile
    from concourse import bacc, mybir

    f32, f32r = mybir.dt.float32, mybir.dt.float32r
    MULT, ADD = mybir.AluOpType.mult, mybir.AluOpType.add
    EXP = mybir.ActivationFunctionType.Exp

    nc = bacc.Bacc(target_bir_lowering=False, trn_type="TRN2", debug=False)

    xT_d = nc.dram_tensor("xT", [D, S], f32r, kind="ExternalInput")
    wq_d = nc.dram_tensor("wq", [D, DHC], f32r, kind="ExternalInput")
    wk_d = nc.dram_tensor("wk", [D, DHC], f32r, kind="ExternalInput")
    wv_d = nc.dram_tensor("wv", [D, DHC], f32r, kind="ExternalInput")
    wo_d = nc.dram_tensor("wo", [DHC, D], f32r, kind="ExternalInput")
    bqk_d = nc.dram_tensor("bqk", [2, DHC], f32r, kind="ExternalInput")
    bv_d = nc.dram_tensor("bv", [1, DHC], f32r, kind="ExternalInput")
    ones_d = nc.dram_tensor("ones", [1, 512], f32r, kind="ExternalInput")
    psig_d = nc.dram_tensor("psig", [128, 128], f32r, kind="ExternalInput")
    rope_d = nc.dram_tensor("rope", [2, 64, S], f32, kind="ExternalInput")
    mdiag_d = nc.dram_tensor("mdiag", [128, 128], f32, kind="ExternalInput")
    out_d = nc.dram_tensor("out", [D, S], f32, kind="ExternalOutput")
    if not causal:
        maskT_d = nc.dram_tensor("maskT", [S, S], f32, kind="ExternalInput")

    with tile.TileContext(nc) as tc:
        with tc.tile_pool(name="const", bufs=1) as const_p, \
             tc.tile_pool(name="persist", bufs=1) as pers_p, \
             tc.tile_pool(name="ph2sb", bufs=1) as ph2_sb, \
             tc.tile_pool(name="ph3sb", bufs=1) as ph3_sb:

            # ---------- constants ----------
            ones_t = const_p.tile([1, 512], f32r, tag="ones")
            nc.sync.dma_start(out=ones_t, in_=ones_d[:])
            psig_t = const_p.tile([128, 128], f32r, tag="psig")
            nc.sync.dma_start(out=psig_t, in_=psig_d[:])
            # per-pair bias columns [128, 1] (partition = head-pair dims)
            bcol = {}
            for t_i in (0, 1):
                for p in range(NPAIR):
                    bt = const_p.tile([128, 1], f32r, tag=f"bc{t_i}{p}", name=f"bc{t_i}{p}")
                    src_row = bqk_d[t_i:t_i + 1, 128 * p:128 * (p + 1)]
                    nc.sync.dma_start(out=bt, in_=src_row.rearrange("o d -> d o"))
                    bcol[t_i, p] = bt.bitcast(f32)
            bv_t = const_p.tile([1, DHC], f32r, tag="bv")
            nc.sync.dma_start(out=bv_t, in_=bv_d[:])
            mdiag_t = const_p.tile([128, 128], f32, tag="mdiag")
            nc.sync.dma_start(out=mdiag_t, in_=mdiag_d[:])

            # ---------- persistent activations ----------
            qt_pair = [pers_p.tile([128, S], f32r, tag=f"qt{p}", name=f"qt{p}") for p in range(NPAIR)]
            kt_pair = [pers_p.tile([128, S], f32r, tag=f"kt{p}", name=f"kt{p}") for p in range(NPAIR)]
            v_sb = [pers_p.tile([128, HPC, DK + 1], f32r, tag=f"v{i}", name=f"v{i}") for i in range(NSB)]
            attnT_sb = [pers_p.tile([128, S], f32r, tag=f"at{p}", name=f"at{p}") for p in range(NPAIR)]

            # =========================================================
            # Phase 1: projections + RoPE + V assembly
            # =========================================================
            with tc.tile_pool(name="ph1sb", bufs=1) as ph1_sb, \
                 tc.tile_pool(name="ph1ps", bufs=1, space="PSUM") as ph1_ps:

                def load_xq(qc, tiles_only=False):
                    ql, qh = 512 * qc, 512 * (qc + 1)
                    ts = [ph1_sb.tile([128, 512], f32r, tag="xq", bufs=10, name=f"xq{kc}_{qc}")
                          for kc in range(KC)]
                    if not tiles_only:
                        for kc in range(KC):
                            nc.sync.dma_start(out=ts[kc], in_=xT_d[128 * kc:128 * (kc + 1), ql:qh])
                    return ts

                # q-chunks processed descending so that phase 2 (which walks
                # k-blocks descending) can start as soon as the tail chunk of
                # Q^T/K^T/V is ready.  First-chunk x slices and weights are
                # DMA'd interleaved per k-chunk so the first matmuls start
                # as soon as possible.
                qc_order = list(reversed(range(NQC)))
                w_t = {t_i: [ph1_sb.tile([128, DHC], f32r, tag=f"w{t_i}_{kc}", name=f"w{t_i}_{kc}")
                             for kc in range(KC)] for t_i in (0, 1)}
                wv_t = [ph1_sb.tile([128, DHC], f32r, tag=f"wv{kc}", name=f"wv{kc}")
                        for kc in range(KC)]
                def load_rope(qc):
                    # [64, 512] source pair-stacked to 128 partitions via two DMAs
                    ql = 512 * qc
                    ts = [ph1_sb.tile([128, 512], f32, tag="rope", bufs=4, name=f"rope{i}_{qc}")
                          for i in range(2)]
                    for i in range(2):
                        nc.sync.dma_start(out=ts[i][0:64, :], in_=rope_d[i][:, ql:ql + 512])
                        nc.sync.dma_start(out=ts[i][64:128, :], in_=rope_d[i][:, ql:ql + 512])
                    return ts

                xq_next = load_xq(qc_order[0], tiles_only=True)
                q3l = 512 * qc_order[0]
                for kc in range(KC):
                    nc.sync.dma_start(out=w_t[0][kc], in_=wq_d[128 * kc:128 * (kc + 1), :])
                    nc.sync.dma_start(out=w_t[1][kc], in_=wk_d[128 * kc:128 * (kc + 1), :])
                    nc.sync.dma_start(out=xq_next[kc], in_=xT_d[128 * kc:128 * (kc + 1), q3l:q3l + 512])
                    nc.sync.dma_start(out=wv_t[kc], in_=wv_d[128 * kc:128 * (kc + 1), :])
                rope_next = load_rope(qc_order[0])

                for qi, qc in enumerate(qc_order):
                    ql, qh = 512 * qc, 512 * (qc + 1)
                    xq = xq_next
                    rope_s = rope_next
                    pps = {}
                    for t_i in (0, 1):
                        for p in range(NPAIR):
                            pps[t_i, p] = ph1_ps.tile([128, 512], f32, tag="qtp",
                                                      bufs=4, name=f"pp{t_i}_{p}_{qc}")
                    for kc in range(KC):
                        for t_i in (0, 1):
                            for p in range(NPAIR):
                                nc.tensor.matmul(pps[t_i, p],
                                                 w_t[t_i][kc][:, 128 * p:128 * (p + 1)],
                                                 xq[kc], start=(kc == 0), stop=(kc == KC - 1))
                    if qi + 1 < NQC:
                        xq_next = load_xq(qc_order[qi + 1])
                        rope_next = load_rope(qc_order[qi + 1])
                    cos_t, sin_t = rope_s
                    for t_i in (0, 1):
                        dst_pair = qt_pair if t_i == 0 else kt_pair
                        for p in range(NPAIR):
                            pp = pps[t_i, p]
                            # RoPE with bias folded in:
                            #   dst = (pp+b)*cos + Psig @ ((pp+b)*sin_sig)
                            u_t = ph1_sb.tile([128, 512], f32r, tag="u", bufs=2)
                            nc.vector.scalar_tensor_tensor(
                                out=u_t, in0=pp, scalar=bcol[t_i, p], in1=sin_t,
                                op0=ADD, op1=MULT)
                            us = ph1_ps.tile([128, 512], f32, tag="usp", bufs=2)
                            nc.tensor.matmul(us, psig_t, u_t, start=True, stop=True)
                            dst = dst_pair[p][:, ql:qh]
                            nc.vector.scalar_tensor_tensor(
                                out=dst, in0=pp, scalar=bcol[t_i, p], in1=cos_t,
                                op0=ADD, op1=MULT)
                            nc.vector.tensor_tensor(out=dst, in0=us, in1=dst.bitcast(f32), op=ADD)
                    # V projection for the 4 s-blocks covered by this q-chunk
                    for r in range(4):
                        si = 4 * qc + r
                        vp = ph1_ps.tile([128, DHC + HPC], f32, tag="vp", bufs=2)
                        for kc in range(KC):
                            nc.tensor.matmul(vp[:, 0:DHC], xq[kc][:, 128 * r:128 * (r + 1)],
                                             wv_t[kc], start=(kc == 0), stop=False)
                        nc.tensor.matmul(vp[:, 0:DHC], ones_t[:, 0:128], bv_t,
                                         start=False, stop=True)
                        nc.tensor.matmul(vp[:, DHC:DHC + HPC], ones_t[:, 0:128],
                                         ones_t[:, 0:HPC], start=True, stop=True)
                        nc.scalar.copy(out=v_sb[si][:, :, 0:DK],
                                       in_=vp[:, 0:DHC].rearrange("p (h d) -> p h d", h=HPC))
                        nc.scalar.copy(out=v_sb[si][:, :, DK:DK + 1],
                                       in_=vp[:, DHC:DHC + HPC].rearrange("p (h o) -> p h o", h=HPC))

            # =========================================================
            # Phase 2: attention per head
            # =========================================================
            HALF = S // 2
            with tc.tile_pool(name="ph2ps", bufs=1, space="PSUM") as ph2_ps:
                # Per (pair, q-half): the two heads of the pair run as two
                # interleaved dependency chains (separate scores psum + attn
                # accumulator each) so PE and ACT stay busy simultaneously.
                # k-blocks walk descending (matches phase-1 production order);
                # PV trails one item behind QK^T/exp.
                def emit_pv(h, at_ps, hlo, it, pT):
                    j, base, w, a0 = it
                    a = a0
                    while a < w:
                        bnd = min((a // 512 + 1) * 512, w)
                        sbank = (base + a) // 512
                        jmax = min(NSB - 1, 4 * sbank + 3) if causal else NSB - 1
                        nc.tensor.matmul(at_ps[:, base + a - hlo:base + bnd - hlo],
                                         v_sb[j][:, h, :], pT[:, a:bnd],
                                         start=(j == jmax), stop=(j == 0))
                        a = bnd

                for p in range(NPAIR):
                    for half in (1, 0):
                        hlo, hhi = HALF * half, HALF * (half + 1)
                        at_ps = [ph2_ps.tile([DK + 1, HALF], f32, tag="atp",
                                             bufs=2, name=f"atp{p}_{half}_{hh}")
                                 for hh in range(2)]
                        items = []
                        for j in reversed(range(NSB)):
                            qlo = max(128 * j, hlo) if causal else hlo
                            if qlo >= hhi:
                                continue
                            base = (qlo // 512) * 512
                            first = True
                            while base < hhi:
                                w = min(1024, hhi - base)
                                a0 = (qlo - base) if first else 0
                                items.append((j, base, w, a0))
                                base += w
                                first = False
                        pend = []
                        for it in items:
                            j, base, w, a0 = it
                            scs = []
                            for hh in range(2):
                                off = 64 * hh
                                sc = ph2_ps.tile([128, 1024], f32, tag="sc",
                                                 bufs=2, name=f"sc{hh}")
                                a = a0
                                while a < w:
                                    bnd = min((a // 512 + 1) * 512, w)
                                    nc.tensor.matmul(
                                        sc[:, a:bnd],
                                        kt_pair[p][off:off + 64, 128 * j:128 * (j + 1)],
                                        qt_pair[p][off:off + 64, base + a:base + bnd],
                                        start=True, stop=True)
                                    a = bnd
                                scs.append(sc)
                            if pend:
                                for (pit, phh, ppT) in pend:
                                    emit_pv(2 * p + phh, at_ps[phh], hlo, pit, ppT)
                                pend = []
                            for hh in range(2):
                                sc = scs[hh]
                                if not causal:
                                    mt = ph2_sb.tile([128, 1024], f32, tag="mt", bufs=3)
                                    nc.sync.dma_start(
                                        out=mt[:, a0:w],
                                        in_=maskT_d[128 * j:128 * (j + 1), base + a0:base + w])
                                    nc.vector.tensor_tensor(
                                        out=sc[:, a0:w], in0=sc[:, a0:w],
                                        in1=mt[:, a0:w], op=ADD)
                                pT = ph2_sb.tile([128, 1024], f32r, tag="pT", bufs=4)
                                nc.scalar.activation(out=pT[:, a0:w], in_=sc[:, a0:w], func=EXP)
                                if causal and base <= 128 * j < base + w:
                                    dc = 128 * j - base
                                    nc.vector.tensor_tensor(
                                        out=pT[:, dc:dc + 128],
                                        in0=pT[:, dc:dc + 128].bitcast(f32),
                                        in1=mdiag_t, op=MULT)
                                pend.append((it, hh, pT))
                        for (pit, phh, ppT) in pend:
                            emit_pv(2 * p + phh, at_ps[phh], hlo, pit, ppT)
                        for hh in range(2):
                            off = 64 * hh
                            rec = ph2_sb.tile([1, HALF], f32, tag="rec", bufs=2)
                            nc.vector.reciprocal(rec, at_ps[hh][DK:DK + 1, :])
                            bc = ph2_sb.tile([64, HALF], f32, tag="bc", bufs=2)
                            nc.gpsimd.partition_broadcast(bc, rec)
                            nc.vector.tensor_tensor(out=attnT_sb[p][off:off + 64, hlo:hhi],
                                                    in0=at_ps[hh][0:DK, :], in1=bc, op=MULT)

            # =========================================================
            # Phase 3: output projection (partial; host sums cores + bias)
            # =========================================================
            # output projection computed transposed: out^T[dout, s] so the
            # stationary operand (Wo chunk) is reused across the whole s sweep
            # (one weight load per (dout-block, chunk) instead of per matmul).
            # The host transposes the [D, S] partial back.
            with tc.tile_pool(name="ph3ps", bufs=1, space="PSUM") as ph3_ps:
                wo_t = [ph3_sb.tile([128, D], f32r, tag=f"wo{ch}", name=f"wo{ch}") for ch in range(NPAIR)]
                for ch in range(NPAIR):
                    nc.sync.dma_start(out=wo_t[ch], in_=wo_d[128 * ch:128 * (ch + 1), :])
                for do in range(D // 128):          # 8 dout blocks
                    ops = [ph3_ps.tile([128, 512], f32, tag="op", bufs=8, name=f"op{do}_{sc_}")
                           for sc_ in range(NQC)]
                    for ch in range(NPAIR):
                        for sc_ in range(NQC):
                            nc.tensor.matmul(ops[sc_],
                                             wo_t[ch][:, 128 * do:128 * (do + 1)],
                                             attnT_sb[ch][:, 512 * sc_:512 * (sc_ + 1)],
                                             start=(ch == 0), stop=(ch == NPAIR - 1))
                    for sc_ in range(NQC):
                        ob = ph3_sb.tile([128, 512], f32, tag="ob", bufs=8)
                        if sc_ % 2 == 0:
                            nc.vector.tensor_copy(ob, ops[sc_])
                        else:
                            nc.scalar.copy(out=ob, in_=ops[sc_])
                        nc.sync.dma_start(out=out_d[128 * do:128 * (do + 1), 512 * sc_:512 * (sc_ + 1)],
                                          in_=ob)

    nc.compile()
    return nc


def _rope_tables():
    # [2, 64, S]: cos and sigma-permuted signed sin, one head's worth; the
    # device pair-stacks to 128 partitions. The 1/sqrt(dk) score scale is
    # folded into Wq/bq on the host, so Q and K share these tables.
    half = DK // 2
    freqs = (10000.0 ** (-2.0 / DK * np.arange(half, dtype=np.float32))).astype(np.float64)
    ang = np.outer(np.arange(S, dtype=np.float64), freqs)           # [S, 32]
    cos1 = np.cos(ang).T.astype(np.float32)                          # [32, S]
    sin1 = np.sin(ang).T.astype(np.float32)
    c64 = np.concatenate([cos1, cos1], axis=0)                       # [64, S]
    ssig64 = np.concatenate([sin1, -sin1], axis=0)                   # s-tilde(sigma(p))
    return np.stack([c64, ssig64]).astype(np.float32)


def _psig():
    p64 = np.zeros((64, 64), np.float32)
    p64[np.arange(32) + 32, np.arange(32)] = 1.0
    p64[np.arange(32), np.arange(32) + 32] = 1.0
    p = np.zeros((128, 128), np.float32)
    p[0:64, 0:64] = p64
    p[64:128, 64:128] = p64
    return p


def _make_runner(nc, n_cores=NCORES):
    """Compile the SPMD program once into a reusable jitted shard_map callable
    (same execution path as bass_utils.run_bass_kernel_spmd under axon)."""
    import jax
    from jax.sharding import Mesh, PartitionSpec
    from jax.experimental.shard_map import shard_map
    from concourse import bass2jax, mybir
    from concourse.bass2jax import _bass_exec_p, install_neuronx_cc_hook

    install_neuronx_cc_hook()
    partition_name = nc.partition_id_tensor.name if nc.partition_id_tensor else None
    in_names, out_names, out_avals, zero_outs = [], [], [], []
    for alloc in nc.m.functions[0].allocations:
        if not isinstance(alloc, mybir.MemoryLocationSet):
            continue
        name = alloc.memorylocations[0].name
        if alloc.kind == "ExternalInput":
            if name != partition_name:
                in_names.append(name)
        elif alloc.kind == "ExternalOutput":
            out_names.append(name)
            shape = tuple(alloc.tensor_shape)
            dtype = mybir.dt.np(alloc.dtype)
            out_avals.append(jax.core.ShapedArray(shape, dtype))
            zero_outs.append(np.zeros(shape, dtype))
    n_params = len(in_names)
    all_in = in_names + out_names
    if partition_name is not None:
        all_in.append(partition_name)

    def _body(*args):
        operands = list(args)
        if partition_name is not None:
            operands.append(bass2jax.partition_id_tensor())
        outs = _bass_exec_p.bind(
            *operands, out_avals=tuple(out_avals), in_names=tuple(all_in),
            out_names=tuple(out_names), lowering_input_output_aliases=(),
            sim_require_finite=True, sim_require_nnan=True, nc=nc)
        return tuple(outs)

    devices = jax.devices()[:n_cores]
    mesh = Mesh(np.asarray(devices), ("core",))
    specs = (PartitionSpec("core"),) * (n_params + len(out_names))
    out_specs = (PartitionSpec("core"),) * len(out_names)
    fn = jax.jit(shard_map(_body, mesh=mesh, in_specs=specs,
                           out_specs=out_specs, check_rep=False),
                 keep_unused=True)
    concat_zeros = [np.zeros((n_cores * z.shape[0], *z.shape[1:]), z.dtype)
                    for z in zero_outs]

    def run(in_maps):
        concat_in = [np.concatenate([np.asarray(in_maps[c][k]) for c in range(n_cores)],
                                    axis=0) for k in in_names]
        outs = fn(*concat_in, *concat_zeros)
        o = np.asarray(outs[out_names.index("out")])
        return o.reshape(n_cores, *zero_outs[out_names.index("out")].shape)

    return run


def kernel(x, mask, Wq, bq, Wk, bk, Wv, bv, Wo, bo):
    x = np.asarray(x, dtype=np.float32)
    mask = np.asarray(mask)
    Wq, bq = np.asarray(Wq, np.float32), np.asarray(bq, np.float32)
    Wk, bk = np.asarray(Wk, np.float32), np.asarray(bk, np.float32)
    Wv, bv = np.asarray(Wv, np.float32), np.asarray(bv, np.float32)
    Wo, bo = np.asarray(Wo, np.float32), np.asarray(bo, np.float32)

    causal_ref = np.triu(np.ones((S, S), dtype=bool), k=1)
    m2 = np.broadcast_to(mask, (B, 1, S, S))[:, 0]
    causal = all(np.array_equal(m2[b], causal_ref) for b in range(B))

    if causal not in _BUILD_CACHE:
        nc = _build(causal)
        _BUILD_CACHE[causal] = (nc, _make_runner(nc))
    nc, run = _BUILD_CACHE[causal]

    rope = _rope_tables()
    psig = _psig()
    ones = np.ones((1, 512), np.float32)
    # multiplicative 0/1 mask for the diagonal block (applied to exp(scores))
    mdiag = np.where(np.arange(128)[:, None] > np.arange(128)[None, :],
                     np.float32(0.0), np.float32(1.0)).astype(np.float32)

    xT = [np.ascontiguousarray(x[b].T) for b in range(B)]
    maskT = None
    if not causal:
        maskT = [np.ascontiguousarray(
            np.where(m2[b], np.float32(-1e30), np.float32(0.0)).T) for b in range(B)]

    in_maps = []
    for c in range(NCORES):
        b, hg = c // (NCORES // B), c % (NCORES // B)
        cs = slice(DHC * hg, DHC * (hg + 1))
        im = {
            "xT": xT[b],
            "wq": np.ascontiguousarray(Wq[:, cs] * np.float32(1.0 / math.sqrt(DK))),
            "wk": np.ascontiguousarray(Wk[:, cs]),
            "wv": np.ascontiguousarray(Wv[:, cs]),
            "wo": np.ascontiguousarray(Wo[cs, :]),
            "bqk": np.ascontiguousarray(np.stack([bq[cs] * np.float32(1.0 / math.sqrt(DK)), bk[cs]])),
            "bv": np.ascontiguousarray(bv[cs][None, :]),
            "ones": ones, "psig": psig, "rope": rope, "mdiag": mdiag,
        }
        if not causal:
            im["maskT"] = maskT[b]
        in_maps.append(im)

    try:
        partials = run(in_maps)
    except Exception:
        # fallback: canonical SPMD runner (recompiles per call)
        from concourse.bass_utils import run_bass_kernel_spmd
        res = run_bass_kernel_spmd(nc, in_maps, core_ids=list(range(NCORES)))
        partials = np.stack([res.results[c]["out"] for c in range(NCORES)])
    out = np.zeros((B, S, D), np.float32)
    for c in range(NCORES):
        out[c // (NCORES // B)] += partials[c].T
    out += bo[None, None, :]
    return out


# revision 25
# speedup vs baseline: 21255.4069x; 1.0204x over previous
"""Multi-head attention (RoPE, causal) Trainium2 Bass kernel, 8-core SPMD.

Problem: B=2, S=2048, D=1024, H=16, DK=64, fp32, causal mask.

Sharding: core c handles batch b = c//4 and head group hg = c%4 (4 heads).
Each core computes Q/K/V projections for its 4 heads (column-sliced weights),
RoPE, causal attention, and a partial output projection (row-sliced Wo).
Host sums the 4 partial outputs per batch and adds the output bias.

Layout strategy (no on-device transposes):
  x^T [D, S] is precomputed on host; Q^T/K^T computed as [dk, S] tiles
  (weights stationary, x^T moving); scores computed transposed [k, q]
  (K^T stationary, Q^T moving); PV uses V in natural layout [k, dk+1]
  (stationary) with exp(scores^T) moving, accumulating attn^T [dk(+1), q];
  the ones column of V accumulates the softmax denominator. Normalization
  multiplies attn^T rows by broadcast 1/denom. Output projection uses
  attn^T as stationary and Wo as moving, producing natural [s, D] partials.

All matmul operands are float32r (TF32-like fast mode: 1 cycle/row at
moving-dim >= 256 vs 4 cycles/row for fp32): ~1e-4 relative L2 per matmul.
"""
import sys
sys.path.insert(0, "/opt/trn_rl_repo")
import math
import numpy as np

B, S, D, H, DK = 2, 2048, 1024, 16, 64
NCORES = 8
HPC = H // (NCORES // B)     # 4 heads per core
DHC = HPC * DK               # 256 attn dims per core
NPAIR = HPC // 2             # 2 head pairs per core
KC = D // 128                # 8 contraction chunks
NSB = S // 128               # 16 s-blocks / k-blocks
NQC = S // 512               # 4 q-chunks of 512

_BUILD_CACHE = {}


def _build(causal: bool):
    import concourse.tile as tile
    from concourse import bacc, mybir

    f32, f32r = mybir.dt.float32, mybir.dt.float32r
    MULT, ADD = mybir.AluOpType.mult, mybir.AluOpType.add
    EXP = mybir.ActivationFunctionType.Exp

    nc = bacc.Bacc(target_bir_lowering=False, trn_type="TRN2", debug=False)

    xT_d = nc.dram_tensor("xT", [D, S], f32r, kind="ExternalInput")
    wq_d = nc.dram_tensor("wq", [D, DHC], f32r, kind="ExternalInput")
    wk_d = nc.dram_tensor("wk", [D, DHC], f32r, kind="ExternalInput")
    wv_d = nc.dram_tensor("wv", [D, DHC], f32r, kind="ExternalInput")
    wo_d = nc.dram_tensor("wo", [DHC, D], f32r, kind="ExternalInput")
    bqk_d = nc.dram_tensor("bqk", [2, DHC], f32r, kind="ExternalInput")
    bv_d = nc.dram_tensor("bv", [1, DHC], f32r, kind="ExternalInput")
    ones_d = nc.dram_tensor("ones", [1, 512], f32r, kind="ExternalInput")
    psig_d = nc.dram_tensor("psig", [128, 128], f32r, kind="ExternalInput")
    rope_d = nc.dram_tensor("rope", [2, 64, S], f32, kind="ExternalInput")
    mdiag_d = nc.dram_tensor("mdiag", [128, 128], f32, kind="ExternalInput")
    out_d = nc.dram_tensor("out", [D, S], f32, kind="ExternalOutput")
    if not causal:
        maskT_d = nc.dram_tensor("maskT", [S, S], f32, kind="ExternalInput")

    with tile.TileContext(nc) as tc:
        with tc.tile_pool(name="const", bufs=1) as const_p, \
             tc.tile_pool(name="persist", bufs=1) as pers_p, \
             tc.tile_pool(name="ph2sb", bufs=1) as ph2_sb, \
             tc.tile_pool(name="ph3sb", bufs=1) as ph3_sb:

            # ---------- constants (tiles now; DMAs deferred until after the
            # first-chunk weight/x preload so the first matmuls start early) ----------
            const_dmas = []
            ones_t = const_p.tile([1, 512], f32r, tag="ones")
            const_dmas.append((ones_t, ones_d[:]))
            psig_t = const_p.tile([128, 128], f32r, tag="psig")
            const_dmas.append((psig_t, psig_d[:]))
            # per-pair bias columns [128, 1] (partition = head-pair dims)
            bcol = {}
            for t_i in (0, 1):
                for p in range(NPAIR):
                    bt = const_p.tile([128, 1], f32r, tag=f"bc{t_i}{p}", name=f"bc{t_i}{p}")
                    src_row = bqk_d[t_i:t_i + 1, 128 * p:128 * (p + 1)]
                    const_dmas.append((bt, src_row.rearrange("o d -> d o")))
                    bcol[t_i, p] = bt.bitcast(f32)
            bv_t = const_p.tile([1, DHC], f32r, tag="bv")
            const_dmas.append((bv_t, bv_d[:]))
            mdiag_t = const_p.tile([128, 128], f32, tag="mdiag")
            const_dmas.append((mdiag_t, mdiag_d[:]))

            # ---------- persistent activations ----------
            qt_pair = [pers_p.tile([128, S], f32r, tag=f"qt{p}", name=f"qt{p}") for p in range(NPAIR)]
            kt_pair = [pers_p.tile([128, S], f32r, tag=f"kt{p}", name=f"kt{p}") for p in range(NPAIR)]
            v_sb = [pers_p.tile([128, HPC, DK + 1], f32r, tag=f"v{i}", name=f"v{i}") for i in range(NSB)]
            attnT_sb = [pers_p.tile([128, S], f32r, tag=f"at{p}", name=f"at{p}") for p in range(NPAIR)]

            # =========================================================
            # Phase 1: projections + RoPE + V assembly
            # =========================================================
            with tc.tile_pool(name="ph1sb", bufs=1) as ph1_sb, \
                 tc.tile_pool(name="ph1ps", bufs=1, space="PSUM") as ph1_ps:

                def load_xq(qc, tiles_only=False):
                    ql, qh = 512 * qc, 512 * (qc + 1)
                    ts = [ph1_sb.tile([128, 512], f32r, tag="xq", bufs=12, name=f"xq{kc}_{qc}")
                          for kc in range(KC)]
                    if not tiles_only:
                        for kc in range(KC):
                            nc.sync.dma_start(out=ts[kc], in_=xT_d[128 * kc:128 * (kc + 1), ql:qh])
                    return ts

                # q-chunks processed descending so that phase 2 (which walks
                # k-blocks descending) can start as soon as the tail chunk of
                # Q^T/K^T/V is ready.  First-chunk x slices and weights are
                # DMA'd interleaved per k-chunk so the first matmuls start
                # as soon as possible.
                qc_order = list(reversed(range(NQC)))
                w_t = {t_i: [ph1_sb.tile([128, DHC], f32r, tag=f"w{t_i}_{kc}", name=f"w{t_i}_{kc}")
                             for kc in range(KC)] for t_i in (0, 1)}
                wv_t = [ph1_sb.tile([128, DHC], f32r, tag=f"wv{kc}", name=f"wv{kc}")
                        for kc in range(KC)]
                def load_rope(qc):
                    # [64, 512] source pair-stacked to 128 partitions via two DMAs
                    ql = 512 * qc
                    ts = [ph1_sb.tile([128, 512], f32, tag="rope", bufs=4, name=f"rope{i}_{qc}")
                          for i in range(2)]
                    for i in range(2):
                        nc.sync.dma_start(out=ts[i][0:64, :], in_=rope_d[i][:, ql:ql + 512])
                        nc.sync.dma_start(out=ts[i][64:128, :], in_=rope_d[i][:, ql:ql + 512])
                    return ts

                xq_next = load_xq(qc_order[0], tiles_only=True)
                q3l = 512 * qc_order[0]
                for kc in range(KC):
                    nc.sync.dma_start(out=w_t[0][kc], in_=wq_d[128 * kc:128 * (kc + 1), :])
                    nc.sync.dma_start(out=w_t[1][kc], in_=wk_d[128 * kc:128 * (kc + 1), :])
                    nc.sync.dma_start(out=xq_next[kc], in_=xT_d[128 * kc:128 * (kc + 1), q3l:q3l + 512])
                for kc in range(KC):
                    nc.sync.dma_start(out=wv_t[kc], in_=wv_d[128 * kc:128 * (kc + 1), :])
                for t_, src_ in const_dmas:
                    nc.sync.dma_start(out=t_, in_=src_)
                rope_next = load_rope(qc_order[0])

                for qi, qc in enumerate(qc_order):
                    ql, qh = 512 * qc, 512 * (qc + 1)
                    xq = xq_next
                    rope_s = rope_next
                    pps = {}
                    for t_i in (0, 1):
                        for p in range(NPAIR):
                            pps[t_i, p] = ph1_ps.tile([128, 512], f32, tag="qtp",
                                                      bufs=4, name=f"pp{t_i}_{p}_{qc}")
                    for kc in range(KC):
                        for t_i in (0, 1):
                            for p in range(NPAIR):
                                nc.tensor.matmul(pps[t_i, p],
                                                 w_t[t_i][kc][:, 128 * p:128 * (p + 1)],
                                                 xq[kc], start=(kc == 0), stop=(kc == KC - 1))
                    if qi + 1 < NQC:
                        xq_next = load_xq(qc_order[qi + 1])
                        rope_next = load_rope(qc_order[qi + 1])
                    cos_t, sin_t = rope_s
                    for t_i in (0, 1):
                        dst_pair = qt_pair if t_i == 0 else kt_pair
                        for p in range(NPAIR):
                            pp = pps[t_i, p]
                            # RoPE with bias folded in:
                            #   dst = (pp+b)*cos + Psig @ ((pp+b)*sin_sig)
                            u_t = ph1_sb.tile([128, 512], f32r, tag="u", bufs=3)
                            nc.vector.scalar_tensor_tensor(
                                out=u_t, in0=pp, scalar=bcol[t_i, p], in1=sin_t,
                                op0=ADD, op1=MULT)
                            us = ph1_ps.tile([128, 512], f32, tag="usp", bufs=2)
                            nc.tensor.matmul(us, psig_t, u_t, start=True, stop=True)
                            dst = dst_pair[p][:, ql:qh]
                            nc.vector.scalar_tensor_tensor(
                                out=dst, in0=pp, scalar=bcol[t_i, p], in1=cos_t,
                                op0=ADD, op1=MULT)
                            nc.vector.tensor_tensor(out=dst, in0=us, in1=dst.bitcast(f32), op=ADD)
                    # V projection for the 4 s-blocks covered by this q-chunk
                    for r in range(4):
                        si = 4 * qc + r
                        vp = ph1_ps.tile([128, DHC + HPC], f32, tag="vp", bufs=2)
                        for kc in range(KC):
                            nc.tensor.matmul(vp[:, 0:DHC], xq[kc][:, 128 * r:128 * (r + 1)],
                                             wv_t[kc], start=(kc == 0), stop=False)
                        nc.tensor.matmul(vp[:, 0:DHC], ones_t[:, 0:128], bv_t,
                                         start=False, stop=True)
                        nc.tensor.matmul(vp[:, DHC:DHC + HPC], ones_t[:, 0:128],
                                         ones_t[:, 0:HPC], start=True, stop=True)
                        nc.scalar.copy(out=v_sb[si][:, :, 0:DK],
                                       in_=vp[:, 0:DHC].rearrange("p (h d) -> p h d", h=HPC))
                        nc.scalar.copy(out=v_sb[si][:, :, DK:DK + 1],
                                       in_=vp[:, DHC:DHC + HPC].rearrange("p (h o) -> p h o", h=HPC))

            # =========================================================
            # Phase 2: attention per head
            # =========================================================
            HALF = S // 2
            with tc.tile_pool(name="ph2ps", bufs=1, space="PSUM") as ph2_ps:
                # Per (pair, q-half): the two heads of the pair run as two
                # interleaved dependency chains (separate scores psum + attn
                # accumulator each) so PE and ACT stay busy simultaneously.
                # k-blocks walk descending (matches phase-1 production order);
                # PV trails one item behind QK^T/exp.
                def emit_pv(h, at_ps, hlo, it, pT):
                    j, base, w, a0 = it
                    a = a0
                    while a < w:
                        bnd = min((a // 512 + 1) * 512, w)
                        sbank = (base + a) // 512
                        jmax = min(NSB - 1, 4 * sbank + 3) if causal else NSB - 1
                        nc.tensor.matmul(at_ps[:, base + a - hlo:base + bnd - hlo],
                                         v_sb[j][:, h, :], pT[:, a:bnd],
                                         start=(j == jmax), stop=(j == 0))
                        a = bnd

                for p in range(NPAIR):
                    for half in (1, 0):
                        hlo, hhi = HALF * half, HALF * (half + 1)
                        at_ps = [ph2_ps.tile([DK + 1, HALF], f32, tag="atp",
                                             bufs=2, name=f"atp{p}_{half}_{hh}")
                                 for hh in range(2)]
                        items = []
                        for j in reversed(range(NSB)):
                            qlo = max(128 * j, hlo) if causal else hlo
                            if qlo >= hhi:
                                continue
                            base = (qlo // 512) * 512
                            first = True
                            while base < hhi:
                                w = min(1024, hhi - base)
                                a0 = (qlo - base) if first else 0
                                items.append((j, base, w, a0))
                                base += w
                                first = False
                        pend = []
                        for it in items:
                            j, base, w, a0 = it
                            scs = []
                            for hh in range(2):
                                off = 64 * hh
                                sc = ph2_ps.tile([128, 1024], f32, tag="sc",
                                                 bufs=2, name=f"sc{hh}")
                                a = a0
                                while a < w:
                                    bnd = min((a // 512 + 1) * 512, w)
                                    nc.tensor.matmul(
                                        sc[:, a:bnd],
                                        kt_pair[p][off:off + 64, 128 * j:128 * (j + 1)],
                                        qt_pair[p][off:off + 64, base + a:base + bnd],
                                        start=True, stop=True)
                                    a = bnd
                                scs.append(sc)
                            if pend:
                                for (pit, phh, ppT) in pend:
                                    emit_pv(2 * p + phh, at_ps[phh], hlo, pit, ppT)
                                pend = []
                            for hh in range(2):
                                sc = scs[hh]
                                if not causal:
                                    mt = ph2_sb.tile([128, 1024], f32, tag="mt", bufs=3)
                                    nc.sync.dma_start(
                                        out=mt[:, a0:w],
                                        in_=maskT_d[128 * j:128 * (j + 1), base + a0:base + w])
                                    nc.vector.tensor_tensor(
                                        out=sc[:, a0:w], in0=sc[:, a0:w],
                                        in1=mt[:, a0:w], op=ADD)
                                pT = ph2_sb.tile([128, 1024], f32r, tag="pT", bufs=6)
                                nc.scalar.activation(out=pT[:, a0:w], in_=sc[:, a0:w], func=EXP)
                                if causal and base <= 128 * j < base + w:
                                    dc = 128 * j - base
                                    nc.vector.tensor_tensor(
                                        out=pT[:, dc:dc + 128],
                                        in0=pT[:, dc:dc + 128].bitcast(f32),
                                        in1=mdiag_t, op=MULT)
                                pend.append((it, hh, pT))
                        for (pit, phh, ppT) in pend:
                            emit_pv(2 * p + phh, at_ps[phh], hlo, pit, ppT)
                        for hh in range(2):
                            off = 64 * hh
                            rec = ph2_sb.tile([1, HALF], f32, tag="rec", bufs=3)
                            nc.vector.reciprocal(rec, at_ps[hh][DK:DK + 1, :])
                            bc = ph2_sb.tile([64, HALF], f32, tag="bc", bufs=3)
                            nc.gpsimd.partition_broadcast(bc, rec)
                            nc.vector.tensor_tensor(out=attnT_sb[p][off:off + 64, hlo:hhi],
                                                    in0=at_ps[hh][0:DK, :], in1=bc, op=MULT)

            # =========================================================
            # Phase 3: output projection (partial; host sums cores + bias)
            # =========================================================
            # output projection computed transposed: out^T[dout, s] so the
            # stationary operand (Wo chunk) is reused across the whole s sweep
            # (one weight load per (dout-block, chunk) instead of per matmul).
            # The host transposes the [D, S] partial back.
            with tc.tile_pool(name="ph3ps", bufs=1, space="PSUM") as ph3_ps:
                wo_t = [ph3_sb.tile([128, D], f32r, tag=f"wo{ch}", name=f"wo{ch}") for ch in range(NPAIR)]
                for ch in range(NPAIR):
                    nc.sync.dma_start(out=wo_t[ch], in_=wo_d[128 * ch:128 * (ch + 1), :])
                for do in range(D // 128):          # 8 dout blocks
                    ops = [ph3_ps.tile([128, 512], f32, tag="op", bufs=8, name=f"op{do}_{sc_}")
                           for sc_ in range(NQC)]
                    for ch in range(NPAIR):
                        for sc_ in range(NQC):
                            nc.tensor.matmul(ops[sc_],
                                             wo_t[ch][:, 128 * do:128 * (do + 1)],
                                             attnT_sb[ch][:, 512 * sc_:512 * (sc_ + 1)],
                                             start=(ch == 0), stop=(ch == NPAIR - 1))
                    for sc_ in range(NQC):
                        ob = ph3_sb.tile([128, 512], f32, tag="ob", bufs=8)
                        if sc_ % 2 == 0:
                            nc.vector.tensor_copy(ob, ops[sc_])
                        else:
                            nc.scalar.copy(out=ob, in_=ops[sc_])
                        nc.sync.dma_start(out=out_d[128 * do:128 * (do + 1), 512 * sc_:512 * (sc_ + 1)],
                                          in_=ob)

    nc.compile()
    return nc


def _rope_tables():
    # [2, 64, S]: cos and sigma-permuted signed sin, one head's worth; the
    # device pair-stacks to 128 partitions. The 1/sqrt(dk) score scale is
    # folded into Wq/bq on the host, so Q and K share these tables.
    half = DK // 2
    freqs = (10000.0 ** (-2.0 / DK * np.arange(half, dtype=np.float32))).astype(np.float64)
    ang = np.outer(np.arange(S, dtype=np.float64), freqs)           # [S, 32]
    cos1 = np.cos(ang).T.astype(np.float32)                          # [32, S]
    sin1 = np.sin(ang).T.astype(np.float32)
    c64 = np.concatenate([cos1, cos1], axis=0)                       # [64, S]
    ssig64 = np.concatenate([sin1, -sin1], axis=0)                   # s-tilde(sigma(p))
    return np.stack([c64, ssig64]).astype(np.float32)


def _psig():
    p64 = np.zeros((64, 64), np.float32)
    p64[np.arange(32) + 32, np.arange(32)] = 1.0
    p64[np.arange(32), np.arange(32) + 32] = 1.0
    p = np.zeros((128, 128), np.float32)
    p[0:64, 0:64] = p64
    p[64:128, 64:128] = p64
    return p


def _make_runner(nc, n_cores=NCORES):
    """Compile the SPMD program once into a reusable jitted shard_map callable
    (same execution path as bass_utils.run_bass_kernel_spmd under axon)."""
    import jax
    from jax.sharding import Mesh, PartitionSpec
    from jax.experimental.shard_map import shard_map
    from concourse import bass2jax, mybir
    from concourse.bass2jax import _bass_exec_p, install_neuronx_cc_hook

    install_neuronx_cc_hook()
    partition_name = nc.partition_id_tensor.name if nc.partition_id_tensor else None
    in_names, out_names, out_avals, zero_outs = [], [], [], []
    for alloc in nc.m.functions[0].allocations:
        if not isinstance(alloc, mybir.MemoryLocationSet):
            continue
        name = alloc.memorylocations[0].name
        if alloc.kind == "ExternalInput":
            if name != partition_name:
                in_names.append(name)
        elif alloc.kind == "ExternalOutput":
            out_names.append(name)
            shape = tuple(alloc.tensor_shape)
            dtype = mybir.dt.np(alloc.dtype)
            out_avals.append(jax.core.ShapedArray(shape, dtype))
            zero_outs.append(np.zeros(shape, dtype))
    n_params = len(in_names)
    all_in = in_names + out_names
    if partition_name is not None:
        all_in.append(partition_name)

    def _body(*args):
        operands = list(args)
        if partition_name is not None:
            operands.append(bass2jax.partition_id_tensor())
        outs = _bass_exec_p.bind(
            *operands, out_avals=tuple(out_avals), in_names=tuple(all_in),
            out_names=tuple(out_names), lowering_input_output_aliases=(),
            sim_require_finite=True, sim_require_nnan=True, nc=nc)
        return tuple(outs)

    devices = jax.devices()[:n_cores]
    mesh = Mesh(np.asarray(devices), ("core",))
    specs = (PartitionSpec("core"),) * (n_params + len(out_names))
    out_specs = (PartitionSpec("core"),) * len(out_names)
    fn = jax.jit(shard_map(_body, mesh=mesh, in_specs=specs,
                           out_specs=out_specs, check_rep=False),
                 keep_unused=True)
    concat_zeros = [np.zeros((n_cores * z.shape[0], *z.shape[1:]), z.dtype)
                    for z in zero_outs]

    def run(in_maps):
        concat_in = [np.concatenate([np.asarray(in_maps[c][k]) for c in range(n_cores)],
                                    axis=0) for k in in_names]
        outs = fn(*concat_in, *concat_zeros)
        o = np.asarray(outs[out_names.index("out")])
        return o.reshape(n_cores, *zero_outs[out_names.index("out")].shape)

    return run


def kernel(x, mask, Wq, bq, Wk, bk, Wv, bv, Wo, bo):
    x = np.asarray(x, dtype=np.float32)
    mask = np.asarray(mask)
    Wq, bq = np.asarray(Wq, np.float32), np.asarray(bq, np.float32)
    Wk, bk = np.asarray(Wk, np.float32), np.asarray(bk, np.float32)
    Wv, bv = np.asarray(Wv, np.float32), np.asarray(bv, np.float32)
    Wo, bo = np.asarray(Wo, np.float32), np.asarray(bo, np.float32)

    causal_ref = np.triu(np.ones((S, S), dtype=bool), k=1)
    m2 = np.broadcast_to(mask, (B, 1, S, S))[:, 0]
    causal = all(np.array_equal(m2[b], causal_ref) for b in range(B))

    if causal not in _BUILD_CACHE:
        nc = _build(causal)
        _BUILD_CACHE[causal] = (nc, _make_runner(nc))
    nc, run = _BUILD_CACHE[causal]

    rope = _rope_tables()
    psig = _psig()
    ones = np.ones((1, 512), np.float32)
    # multiplicative 0/1 mask for the diagonal block (applied to exp(scores))
    mdiag = np.where(np.arange(128)[:, None] > np.arange(128)[None, :],
                     np.float32(0.0), np.float32(1.0)).astype(np.float32)

    xT = [np.ascontiguousarray(x[b].T) for b in range(B)]
    maskT = None
    if not causal:
        maskT = [np.ascontiguousarray(
            np.where(m2[b], np.float32(-1e30), np.float32(0.0)).T) for b in range(B)]

    in_maps = []
    for c in range(NCORES):
        b, hg = c // (NCORES // B), c % (NCORES // B)
        cs = slice(DHC * hg, DHC * (hg + 1))
        im = {
            "xT": xT[b],
            "wq": np.ascontiguousarray(Wq[:, cs] * np.float32(1.0 / math.sqrt(DK))),
            "wk": np.ascontiguousarray(Wk[:, cs]),
            "wv": np.ascontiguousarray(Wv[:, cs]),
            "wo": np.ascontiguousarray(Wo[cs, :]),
            "bqk": np.ascontiguousarray(np.stack([bq[cs] * np.float32(1.0 / math.sqrt(DK)), bk[cs]])),
            "bv": np.ascontiguousarray(bv[cs][None, :]),
            "ones": ones, "psig": psig, "rope": rope, "mdiag": mdiag,
        }
        if not causal:
            im["maskT"] = maskT[b]
        in_maps.append(im)

    try:
        partials = run(in_maps)
    except Exception:
        # fallback: canonical SPMD runner (recompiles per call)
        from concourse.bass_utils import run_bass_kernel_spmd
        res = run_bass_kernel_spmd(nc, in_maps, core_ids=list(range(NCORES)))
        partials = np.stack([res.results[c]["out"] for c in range(NCORES)])
    out = np.zeros((B, S, D), np.float32)
    for c in range(NCORES):
        out[c // (NCORES // B)] += partials[c].T
    out += bo[None, None, :]
    return out


# revision 27
# speedup vs baseline: 22132.9013x; 1.0413x over previous
"""Multi-head attention (RoPE, causal) Trainium2 Bass kernel, 8-core SPMD.

Problem: B=2, S=2048, D=1024, H=16, DK=64, fp32, causal mask.

Sharding: core c handles batch b = c//4 and head group hg = c%4 (4 heads).
Each core computes Q/K/V projections for its 4 heads (column-sliced weights),
RoPE, causal attention, and a partial output projection (row-sliced Wo).
Host sums the 4 partial outputs per batch and adds the output bias.

Layout strategy (no on-device transposes):
  x^T [D, S] is precomputed on host; Q^T/K^T computed as [dk, S] tiles
  (weights stationary, x^T moving); scores computed transposed [k, q]
  (K^T stationary, Q^T moving); PV uses V in natural layout [k, dk+1]
  (stationary) with exp(scores^T) moving, accumulating attn^T [dk(+1), q];
  the ones column of V accumulates the softmax denominator. Normalization
  multiplies attn^T rows by broadcast 1/denom. Output projection uses
  attn^T as stationary and Wo as moving, producing natural [s, D] partials.

All matmul operands are float32r (TF32-like fast mode: 1 cycle/row at
moving-dim >= 256 vs 4 cycles/row for fp32): ~1e-4 relative L2 per matmul.
"""
import sys
sys.path.insert(0, "/opt/trn_rl_repo")
import math
import numpy as np

B, S, D, H, DK = 2, 2048, 1024, 16, 64
NCORES = 8
HPC = H // (NCORES // B)     # 4 heads per core
DHC = HPC * DK               # 256 attn dims per core
NPAIR = HPC // 2             # 2 head pairs per core
KC = D // 128                # 8 contraction chunks
NSB = S // 128               # 16 s-blocks / k-blocks
NQC = S // 512               # 4 q-chunks of 512

_BUILD_CACHE = {}


def _build(causal: bool):
    import concourse.tile as tile
    from concourse import bacc, mybir

    f32, f32r = mybir.dt.float32, mybir.dt.float32r
    MULT, ADD = mybir.AluOpType.mult, mybir.AluOpType.add
    EXP = mybir.ActivationFunctionType.Exp

    nc = bacc.Bacc(target_bir_lowering=False, trn_type="TRN2", debug=False)

    xT_d = nc.dram_tensor("xT", [D, S], f32r, kind="ExternalInput")
    wq_d = nc.dram_tensor("wq", [D, DHC], f32r, kind="ExternalInput")
    wk_d = nc.dram_tensor("wk", [D, DHC], f32r, kind="ExternalInput")
    wv_d = nc.dram_tensor("wv", [D, DHC], f32r, kind="ExternalInput")
    wo_d = nc.dram_tensor("wo", [DHC, D], f32r, kind="ExternalInput")
    bqk_d = nc.dram_tensor("bqk", [2, DHC], f32r, kind="ExternalInput")
    bv_d = nc.dram_tensor("bv", [1, DHC], f32r, kind="ExternalInput")
    ones_d = nc.dram_tensor("ones", [1, 512], f32r, kind="ExternalInput")
    psig_d = nc.dram_tensor("psig", [128, 128], f32r, kind="ExternalInput")
    rope_d = nc.dram_tensor("rope", [2, 64, S], f32, kind="ExternalInput")
    mdiag_d = nc.dram_tensor("mdiag", [128, 128], f32, kind="ExternalInput")
    out_d = nc.dram_tensor("out", [D, S], f32, kind="ExternalOutput")
    if not causal:
        maskT_d = nc.dram_tensor("maskT", [S, S], f32, kind="ExternalInput")

    with tile.TileContext(nc) as tc:
        with tc.tile_pool(name="const", bufs=1) as const_p, \
             tc.tile_pool(name="persist", bufs=1) as pers_p, \
             tc.tile_pool(name="ph2sb", bufs=1) as ph2_sb, \
             tc.tile_pool(name="ph3sb", bufs=1) as ph3_sb:

            # ---------- constants (tiles now; DMAs deferred until after the
            # first-chunk weight/x preload so the first matmuls start early) ----------
            const_dmas = []
            ones_t = const_p.tile([1, 512], f32r, tag="ones")
            const_dmas.append((ones_t, ones_d[:]))
            psig_t = const_p.tile([128, 128], f32r, tag="psig")
            const_dmas.append((psig_t, psig_d[:]))
            # per-pair bias columns [128, 1] (partition = head-pair dims)
            bcol = {}
            for t_i in (0, 1):
                for p in range(NPAIR):
                    bt = const_p.tile([128, 1], f32r, tag=f"bc{t_i}{p}", name=f"bc{t_i}{p}")
                    src_row = bqk_d[t_i:t_i + 1, 128 * p:128 * (p + 1)]
                    const_dmas.append((bt, src_row.rearrange("o d -> d o")))
                    bcol[t_i, p] = bt.bitcast(f32)
            bv_t = const_p.tile([1, DHC], f32r, tag="bv")
            const_dmas.append((bv_t, bv_d[:]))
            mdiag_t = const_p.tile([128, 128], f32, tag="mdiag")
            const_dmas.append((mdiag_t, mdiag_d[:]))

            # ---------- persistent activations ----------
            qt_pair = [pers_p.tile([128, S], f32r, tag=f"qt{p}", name=f"qt{p}") for p in range(NPAIR)]
            kt_pair = [pers_p.tile([128, S], f32r, tag=f"kt{p}", name=f"kt{p}") for p in range(NPAIR)]
            v_sb = [pers_p.tile([128, HPC, DK + 1], f32r, tag=f"v{i}", name=f"v{i}") for i in range(NSB)]
            attnT_sb = [pers_p.tile([128, S], f32r, tag=f"at{p}", name=f"at{p}") for p in range(NPAIR)]

            # =========================================================
            # Phase 1: projections + RoPE + V assembly
            # =========================================================
            with tc.tile_pool(name="ph1sb", bufs=1) as ph1_sb, \
                 tc.tile_pool(name="ph1ps", bufs=1, space="PSUM") as ph1_ps:

                def load_xq(qc, tiles_only=False):
                    ql, qh = 512 * qc, 512 * (qc + 1)
                    ts = [ph1_sb.tile([128, 512], f32r, tag="xq", bufs=12, name=f"xq{kc}_{qc}")
                          for kc in range(KC)]
                    if not tiles_only:
                        for kc in range(KC):
                            nc.sync.dma_start(out=ts[kc], in_=xT_d[128 * kc:128 * (kc + 1), ql:qh])
                    return ts

                # q-chunks processed descending so that phase 2 (which walks
                # k-blocks descending) can start as soon as the tail chunk of
                # Q^T/K^T/V is ready.  First-chunk x slices and weights are
                # DMA'd interleaved per k-chunk so the first matmuls start
                # as soon as possible.
                qc_order = list(reversed(range(NQC)))
                w_t = {t_i: [ph1_sb.tile([128, DHC], f32r, tag=f"w{t_i}_{kc}", name=f"w{t_i}_{kc}")
                             for kc in range(KC)] for t_i in (0, 1)}
                wv_t = [ph1_sb.tile([128, DHC], f32r, tag=f"wv{kc}", name=f"wv{kc}")
                        for kc in range(KC)]
                def load_rope(qc):
                    # [64, 512] source pair-stacked to 128 partitions via two DMAs
                    ql = 512 * qc
                    ts = [ph1_sb.tile([128, 512], f32, tag="rope", bufs=4, name=f"rope{i}_{qc}")
                          for i in range(2)]
                    for i in range(2):
                        nc.sync.dma_start(out=ts[i][0:64, :], in_=rope_d[i][:, ql:ql + 512])
                        nc.sync.dma_start(out=ts[i][64:128, :], in_=rope_d[i][:, ql:ql + 512])
                    return ts

                xq_next = load_xq(qc_order[0], tiles_only=True)
                q3l = 512 * qc_order[0]
                for kc in range(KC):
                    nc.sync.dma_start(out=w_t[0][kc], in_=wq_d[128 * kc:128 * (kc + 1), :])
                    nc.sync.dma_start(out=w_t[1][kc], in_=wk_d[128 * kc:128 * (kc + 1), :])
                    nc.sync.dma_start(out=xq_next[kc], in_=xT_d[128 * kc:128 * (kc + 1), q3l:q3l + 512])
                for kc in range(KC):
                    nc.sync.dma_start(out=wv_t[kc], in_=wv_d[128 * kc:128 * (kc + 1), :])
                for t_, src_ in const_dmas:
                    nc.sync.dma_start(out=t_, in_=src_)
                rope_next = load_rope(qc_order[0])

                for qi, qc in enumerate(qc_order):
                    ql, qh = 512 * qc, 512 * (qc + 1)
                    xq = xq_next
                    rope_s = rope_next
                    pps = {}
                    for t_i in (0, 1):
                        for p in range(NPAIR):
                            pps[t_i, p] = ph1_ps.tile([128, 512], f32, tag="qtp",
                                                      bufs=4, name=f"pp{t_i}_{p}_{qc}")
                    for kc in range(KC):
                        for t_i in (0, 1):
                            for p in range(NPAIR):
                                nc.tensor.matmul(pps[t_i, p],
                                                 w_t[t_i][kc][:, 128 * p:128 * (p + 1)],
                                                 xq[kc], start=(kc == 0), stop=(kc == KC - 1))
                    if qi + 1 < NQC:
                        xq_next = load_xq(qc_order[qi + 1])
                        rope_next = load_rope(qc_order[qi + 1])
                    cos_t, sin_t = rope_s
                    for t_i in (0, 1):
                        dst_pair = qt_pair if t_i == 0 else kt_pair
                        for p in range(NPAIR):
                            pp = pps[t_i, p]
                            # RoPE with bias folded in:
                            #   dst = (pp+b)*cos + Psig @ ((pp+b)*sin_sig)
                            u_t = ph1_sb.tile([128, 512], f32r, tag="u", bufs=3)
                            nc.vector.scalar_tensor_tensor(
                                out=u_t, in0=pp, scalar=bcol[t_i, p], in1=sin_t,
                                op0=ADD, op1=MULT)
                            us = ph1_ps.tile([128, 512], f32, tag="usp", bufs=2)
                            nc.tensor.matmul(us, psig_t, u_t, start=True, stop=True)
                            dst = dst_pair[p][:, ql:qh]
                            nc.vector.scalar_tensor_tensor(
                                out=dst, in0=pp, scalar=bcol[t_i, p], in1=cos_t,
                                op0=ADD, op1=MULT)
                            nc.vector.tensor_tensor(out=dst, in0=us, in1=dst.bitcast(f32), op=ADD)
                    # V projection for the 4 s-blocks covered by this q-chunk
                    for r in range(4):
                        si = 4 * qc + r
                        vp = ph1_ps.tile([128, DHC + HPC], f32, tag="vp", bufs=2)
                        for kc in range(KC):
                            nc.tensor.matmul(vp[:, 0:DHC], xq[kc][:, 128 * r:128 * (r + 1)],
                                             wv_t[kc], start=(kc == 0), stop=False)
                        nc.tensor.matmul(vp[:, 0:DHC], ones_t[:, 0:128], bv_t,
                                         start=False, stop=True)
                        nc.tensor.matmul(vp[:, DHC:DHC + HPC], ones_t[:, 0:128],
                                         ones_t[:, 0:HPC], start=True, stop=True)
                        nc.scalar.copy(out=v_sb[si][:, :, 0:DK],
                                       in_=vp[:, 0:DHC].rearrange("p (h d) -> p h d", h=HPC))
                        nc.scalar.copy(out=v_sb[si][:, :, DK:DK + 1],
                                       in_=vp[:, DHC:DHC + HPC].rearrange("p (h o) -> p h o", h=HPC))

            # =========================================================
            # Phase 2: attention per head
            # =========================================================
            HALF = S // 2
            with tc.tile_pool(name="ph2ps", bufs=1, space="PSUM") as ph2_ps:
                # Per (pair, q-half): the two heads of the pair run as two
                # interleaved dependency chains (separate scores psum + attn
                # accumulator each) so PE and ACT stay busy simultaneously.
                # k-blocks walk descending (matches phase-1 production order);
                # PV trails one item behind QK^T/exp.
                def emit_pv(h, at_ps, hlo, it, pT):
                    j, base, w, a0 = it
                    a = a0
                    while a < w:
                        bnd = min((a // 512 + 1) * 512, w)
                        sbank = (base + a) // 512
                        jmax = min(NSB - 1, 4 * sbank + 3) if causal else NSB - 1
                        nc.tensor.matmul(at_ps[:, base + a - hlo:base + bnd - hlo],
                                         v_sb[j][:, h, :], pT[:, a:bnd],
                                         start=(j == jmax), stop=(j == 0))
                        a = bnd

                # Wo preloaded here so the per-half output projection (emitted
                # between halves, borrowing "sc" psum slots) never waits on DMA
                wo_t = [ph3_sb.tile([128, D], f32r, tag=f"wo{ch}", name=f"wo{ch}")
                        for ch in range(NPAIR)]
                for ch in range(NPAIR):
                    nc.sync.dma_start(out=wo_t[ch], in_=wo_d[128 * ch:128 * (ch + 1), :])

                def emit_outproj(half):
                    # out^T[dout, s] for this half; Wo stationary across s sweep.
                    # Borrows one "sc" pool slot ([128,1024] = two psum banks =
                    # two 512-wide outputs) per dout block, so it interleaves
                    # with the next half's attention on the PE.
                    scs_half = [2 * half, 2 * half + 1]
                    for do in range(D // 128):
                        op = ph2_ps.tile([128, 1024], f32, tag="sc",
                                         bufs=2, name=f"op{do}_{half}")
                        for ch in range(NPAIR):
                            for i, sc_ in enumerate(scs_half):
                                nc.tensor.matmul(op[:, 512 * i:512 * (i + 1)],
                                                 wo_t[ch][:, 128 * do:128 * (do + 1)],
                                                 attnT_sb[ch][:, 512 * sc_:512 * (sc_ + 1)],
                                                 start=(ch == 0), stop=(ch == NPAIR - 1))
                        ob = ph3_sb.tile([128, 1024], f32, tag="ob", bufs=4)
                        if do % 2 == 0:
                            nc.vector.tensor_copy(ob, op)
                        else:
                            nc.scalar.copy(out=ob, in_=op)
                        nc.sync.dma_start(
                            out=out_d[128 * do:128 * (do + 1), HALF * half:HALF * (half + 1)],
                            in_=ob)

                for half in (1, 0):
                    for p in range(NPAIR):
                        hlo, hhi = HALF * half, HALF * (half + 1)
                        at_ps = [ph2_ps.tile([DK + 1, HALF], f32, tag="atp",
                                             bufs=2, name=f"atp{p}_{half}_{hh}")
                                 for hh in range(2)]
                        items = []
                        for j in reversed(range(NSB)):
                            qlo = max(128 * j, hlo) if causal else hlo
                            if qlo >= hhi:
                                continue
                            base = (qlo // 512) * 512
                            first = True
                            while base < hhi:
                                w = min(1024, hhi - base)
                                a0 = (qlo - base) if first else 0
                                items.append((j, base, w, a0))
                                base += w
                                first = False
                        pend = []
                        for it in items:
                            j, base, w, a0 = it
                            scs = []
                            for hh in range(2):
                                off = 64 * hh
                                sc = ph2_ps.tile([128, 1024], f32, tag="sc",
                                                 bufs=2, name=f"sc{hh}")
                                a = a0
                                while a < w:
                                    bnd = min((a // 512 + 1) * 512, w)
                                    nc.tensor.matmul(
                                        sc[:, a:bnd],
                                        kt_pair[p][off:off + 64, 128 * j:128 * (j + 1)],
                                        qt_pair[p][off:off + 64, base + a:base + bnd],
                                        start=True, stop=True)
                                    a = bnd
                                scs.append(sc)
                            if pend:
                                for (pit, phh, ppT) in pend:
                                    emit_pv(2 * p + phh, at_ps[phh], hlo, pit, ppT)
                                pend = []
                            for hh in range(2):
                                sc = scs[hh]
                                if not causal:
                                    mt = ph2_sb.tile([128, 1024], f32, tag="mt", bufs=3)
                                    nc.sync.dma_start(
                                        out=mt[:, a0:w],
                                        in_=maskT_d[128 * j:128 * (j + 1), base + a0:base + w])
                                    nc.vector.tensor_tensor(
                                        out=sc[:, a0:w], in0=sc[:, a0:w],
                                        in1=mt[:, a0:w], op=ADD)
                                pT = ph2_sb.tile([128, 1024], f32r, tag="pT", bufs=6)
                                nc.scalar.activation(out=pT[:, a0:w], in_=sc[:, a0:w], func=EXP)
                                if causal and base <= 128 * j < base + w:
                                    dc = 128 * j - base
                                    nc.vector.tensor_tensor(
                                        out=pT[:, dc:dc + 128],
                                        in0=pT[:, dc:dc + 128].bitcast(f32),
                                        in1=mdiag_t, op=MULT)
                                pend.append((it, hh, pT))
                        for (pit, phh, ppT) in pend:
                            emit_pv(2 * p + phh, at_ps[phh], hlo, pit, ppT)
                        for hh in range(2):
                            off = 64 * hh
                            rec = ph2_sb.tile([1, HALF], f32, tag="rec", bufs=3)
                            nc.vector.reciprocal(rec, at_ps[hh][DK:DK + 1, :])
                            bc = ph2_sb.tile([64, HALF], f32, tag="bc", bufs=3)
                            nc.gpsimd.partition_broadcast(bc, rec)
                            nc.vector.tensor_tensor(out=attnT_sb[p][off:off + 64, hlo:hhi],
                                                    in0=at_ps[hh][0:DK, :], in1=bc, op=MULT)
                    # both pairs of this half normalized -> project this half's
                    # output now; its matmuls/copies/DMA overlap the next half
                    emit_outproj(half)

    nc.compile()
    return nc


def _rope_tables():
    # [2, 64, S]: cos and sigma-permuted signed sin, one head's worth; the
    # device pair-stacks to 128 partitions. The 1/sqrt(dk) score scale is
    # folded into Wq/bq on the host, so Q and K share these tables.
    half = DK // 2
    freqs = (10000.0 ** (-2.0 / DK * np.arange(half, dtype=np.float32))).astype(np.float64)
    ang = np.outer(np.arange(S, dtype=np.float64), freqs)           # [S, 32]
    cos1 = np.cos(ang).T.astype(np.float32)                          # [32, S]
    sin1 = np.sin(ang).T.astype(np.float32)
    c64 = np.concatenate([cos1, cos1], axis=0)                       # [64, S]
    ssig64 = np.concatenate([sin1, -sin1], axis=0)                   # s-tilde(sigma(p))
    return np.stack([c64, ssig64]).astype(np.float32)


def _psig():
    p64 = np.zeros((64, 64), np.float32)
    p64[np.arange(32) + 32, np.arange(32)] = 1.0
    p64[np.arange(32), np.arange(32) + 32] = 1.0
    p = np.zeros((128, 128), np.float32)
    p[0:64, 0:64] = p64
    p[64:128, 64:128] = p64
    return p


def _make_runner(nc, n_cores=NCORES):
    """Compile the SPMD program once into a reusable jitted shard_map callable
    (same execution path as bass_utils.run_bass_kernel_spmd under axon)."""
    import jax
    from jax.sharding import Mesh, PartitionSpec
    from jax.experimental.shard_map import shard_map
    from concourse import bass2jax, mybir
    from concourse.bass2jax import _bass_exec_p, install_neuronx_cc_hook

    install_neuronx_cc_hook()
    partition_name = nc.partition_id_tensor.name if nc.partition_id_tensor else None
    in_names, out_names, out_avals, zero_outs = [], [], [], []
    for alloc in nc.m.functions[0].allocations:
        if not isinstance(alloc, mybir.MemoryLocationSet):
            continue
        name = alloc.memorylocations[0].name
        if alloc.kind == "ExternalInput":
            if name != partition_name:
                in_names.append(name)
        elif alloc.kind == "ExternalOutput":
            out_names.append(name)
            shape = tuple(alloc.tensor_shape)
            dtype = mybir.dt.np(alloc.dtype)
            out_avals.append(jax.core.ShapedArray(shape, dtype))
            zero_outs.append(np.zeros(shape, dtype))
    n_params = len(in_names)
    all_in = in_names + out_names
    if partition_name is not None:
        all_in.append(partition_name)

    def _body(*args):
        operands = list(args)
        if partition_name is not None:
            operands.append(bass2jax.partition_id_tensor())
        outs = _bass_exec_p.bind(
            *operands, out_avals=tuple(out_avals), in_names=tuple(all_in),
            out_names=tuple(out_names), lowering_input_output_aliases=(),
            sim_require_finite=True, sim_require_nnan=True, nc=nc)
        return tuple(outs)

    devices = jax.devices()[:n_cores]
    mesh = Mesh(np.asarray(devices), ("core",))
    specs = (PartitionSpec("core"),) * (n_params + len(out_names))
    out_specs = (PartitionSpec("core"),) * len(out_names)
    fn = jax.jit(shard_map(_body, mesh=mesh, in_specs=specs,
                           out_specs=out_specs, check_rep=False),
                 keep_unused=True)
    concat_zeros = [np.zeros((n_cores * z.shape[0], *z.shape[1:]), z.dtype)
                    for z in zero_outs]

    def run(in_maps):
        concat_in = [np.concatenate([np.asarray(in_maps[c][k]) for c in range(n_cores)],
                                    axis=0) for k in in_names]
        outs = fn(*concat_in, *concat_zeros)
        o = np.asarray(outs[out_names.index("out")])
        return o.reshape(n_cores, *zero_outs[out_names.index("out")].shape)

    return run


def kernel(x, mask, Wq, bq, Wk, bk, Wv, bv, Wo, bo):
    x = np.asarray(x, dtype=np.float32)
    mask = np.asarray(mask)
    Wq, bq = np.asarray(Wq, np.float32), np.asarray(bq, np.float32)
    Wk, bk = np.asarray(Wk, np.float32), np.asarray(bk, np.float32)
    Wv, bv = np.asarray(Wv, np.float32), np.asarray(bv, np.float32)
    Wo, bo = np.asarray(Wo, np.float32), np.asarray(bo, np.float32)

    causal_ref = np.triu(np.ones((S, S), dtype=bool), k=1)
    m2 = np.broadcast_to(mask, (B, 1, S, S))[:, 0]
    causal = all(np.array_equal(m2[b], causal_ref) for b in range(B))

    if causal not in _BUILD_CACHE:
        nc = _build(causal)
        _BUILD_CACHE[causal] = (nc, _make_runner(nc))
    nc, run = _BUILD_CACHE[causal]

    rope = _rope_tables()
    psig = _psig()
    ones = np.ones((1, 512), np.float32)
    # multiplicative 0/1 mask for the diagonal block (applied to exp(scores))
    mdiag = np.where(np.arange(128)[:, None] > np.arange(128)[None, :],
                     np.float32(0.0), np.float32(1.0)).astype(np.float32)

    xT = [np.ascontiguousarray(x[b].T) for b in range(B)]
    maskT = None
    if not causal:
        maskT = [np.ascontiguousarray(
            np.where(m2[b], np.float32(-1e30), np.float32(0.0)).T) for b in range(B)]

    in_maps = []
    for c in range(NCORES):
        b, hg = c // (NCORES // B), c % (NCORES // B)
        cs = slice(DHC * hg, DHC * (hg + 1))
        im = {
            "xT": xT[b],
            "wq": np.ascontiguousarray(Wq[:, cs] * np.float32(1.0 / math.sqrt(DK))),
            "wk": np.ascontiguousarray(Wk[:, cs]),
            "wv": np.ascontiguousarray(Wv[:, cs]),
            "wo": np.ascontiguousarray(Wo[cs, :]),
            "bqk": np.ascontiguousarray(np.stack([bq[cs] * np.float32(1.0 / math.sqrt(DK)), bk[cs]])),
            "bv": np.ascontiguousarray(bv[cs][None, :]),
            "ones": ones, "psig": psig, "rope": rope, "mdiag": mdiag,
        }
        if not causal:
            im["maskT"] = maskT[b]
        in_maps.append(im)

    try:
        partials = run(in_maps)
    except Exception:
        # fallback: canonical SPMD runner (recompiles per call)
        from concourse.bass_utils import run_bass_kernel_spmd
        res = run_bass_kernel_spmd(nc, in_maps, core_ids=list(range(NCORES)))
        partials = np.stack([res.results[c]["out"] for c in range(NCORES)])
    out = np.zeros((B, S, D), np.float32)
    for c in range(NCORES):
        out[c // (NCORES // B)] += partials[c].T
    out += bo[None, None, :]
    return out


# revision 29
# speedup vs baseline: 22143.7479x; 1.0005x over previous
"""Multi-head attention (RoPE, causal) Trainium2 Bass kernel, 8-core SPMD.

Problem: B=2, S=2048, D=1024, H=16, DK=64, fp32, causal mask.

Sharding: core c handles batch b = c//4 and head group hg = c%4 (4 heads).
Each core computes Q/K/V projections for its 4 heads (column-sliced weights),
RoPE, causal attention, and a partial output projection (row-sliced Wo).
Host sums the 4 partial outputs per batch and adds the output bias.

Layout strategy (no on-device transposes):
  x^T [D, S] is precomputed on host; Q^T/K^T computed as [dk, S] tiles
  (weights stationary, x^T moving); scores computed transposed [k, q]
  (K^T stationary, Q^T moving); PV uses V in natural layout [k, dk+1]
  (stationary) with exp(scores^T) moving, accumulating attn^T [dk(+1), q];
  the ones column of V accumulates the softmax denominator. Normalization
  multiplies attn^T rows by broadcast 1/denom. Output projection uses
  attn^T as stationary and Wo as moving, producing natural [s, D] partials.

All matmul operands are float32r (TF32-like fast mode: 1 cycle/row at
moving-dim >= 256 vs 4 cycles/row for fp32): ~1e-4 relative L2 per matmul.
"""
import sys
sys.path.insert(0, "/opt/trn_rl_repo")
import math
import numpy as np

B, S, D, H, DK = 2, 2048, 1024, 16, 64
NCORES = 8
HPC = H // (NCORES // B)     # 4 heads per core
DHC = HPC * DK               # 256 attn dims per core
NPAIR = HPC // 2             # 2 head pairs per core
KC = D // 128                # 8 contraction chunks
NSB = S // 128               # 16 s-blocks / k-blocks
NQC = S // 512               # 4 q-chunks of 512

_BUILD_CACHE = {}


def _build(causal: bool):
    import concourse.tile as tile
    from concourse import bacc, mybir

    f32, f32r = mybir.dt.float32, mybir.dt.float32r
    MULT, ADD = mybir.AluOpType.mult, mybir.AluOpType.add
    EXP = mybir.ActivationFunctionType.Exp

    nc = bacc.Bacc(target_bir_lowering=False, trn_type="TRN2", debug=False)

    xT_d = nc.dram_tensor("xT", [D, S], f32r, kind="ExternalInput")
    wq_d = nc.dram_tensor("wq", [D, DHC], f32r, kind="ExternalInput")
    wk_d = nc.dram_tensor("wk", [D, DHC], f32r, kind="ExternalInput")
    wv_d = nc.dram_tensor("wv", [D, DHC], f32r, kind="ExternalInput")
    wo_d = nc.dram_tensor("wo", [DHC, D], f32r, kind="ExternalInput")
    bqk_d = nc.dram_tensor("bqk", [2, DHC], f32r, kind="ExternalInput")
    bv_d = nc.dram_tensor("bv", [1, DHC], f32r, kind="ExternalInput")
    ones_d = nc.dram_tensor("ones", [1, 512], f32r, kind="ExternalInput")
    psig_d = nc.dram_tensor("psig", [128, 128], f32r, kind="ExternalInput")
    rope_d = nc.dram_tensor("rope", [2, 64, S], f32, kind="ExternalInput")
    mdiag_d = nc.dram_tensor("mdiag", [128, 128], f32, kind="ExternalInput")
    out_d = nc.dram_tensor("out", [D, S], f32, kind="ExternalOutput")
    if not causal:
        maskT_d = nc.dram_tensor("maskT", [S, S], f32, kind="ExternalInput")

    with tile.TileContext(nc) as tc:
        with tc.tile_pool(name="const", bufs=1) as const_p, \
             tc.tile_pool(name="persist", bufs=1) as pers_p, \
             tc.tile_pool(name="ph2sb", bufs=1) as ph2_sb, \
             tc.tile_pool(name="ph3sb", bufs=1) as ph3_sb:

            # ---------- constants (tiles now; DMAs deferred until after the
            # first-chunk weight/x preload so the first matmuls start early) ----------
            const_dmas = []
            ones_t = const_p.tile([1, 512], f32r, tag="ones")
            const_dmas.append((ones_t, ones_d[:]))
            psig_t = const_p.tile([128, 128], f32r, tag="psig")
            const_dmas.append((psig_t, psig_d[:]))
            # per-pair bias columns [128, 1] (partition = head-pair dims)
            bcol = {}
            for t_i in (0, 1):
                for p in range(NPAIR):
                    bt = const_p.tile([128, 1], f32r, tag=f"bc{t_i}{p}", name=f"bc{t_i}{p}")
                    src_row = bqk_d[t_i:t_i + 1, 128 * p:128 * (p + 1)]
                    const_dmas.append((bt, src_row.rearrange("o d -> d o")))
                    bcol[t_i, p] = bt.bitcast(f32)
            bv_t = const_p.tile([1, DHC], f32r, tag="bv")
            const_dmas.append((bv_t, bv_d[:]))
            mdiag_t = const_p.tile([128, 128], f32, tag="mdiag")
            const_dmas.append((mdiag_t, mdiag_d[:]))

            # ---------- persistent activations ----------
            qt_pair = [pers_p.tile([128, S], f32r, tag=f"qt{p}", name=f"qt{p}") for p in range(NPAIR)]
            kt_pair = [pers_p.tile([128, S], f32r, tag=f"kt{p}", name=f"kt{p}") for p in range(NPAIR)]
            v_sb = [pers_p.tile([128, HPC, DK + 1], f32r, tag=f"v{i}", name=f"v{i}") for i in range(NSB)]
            attnT_sb = [pers_p.tile([128, S], f32r, tag=f"at{p}", name=f"at{p}") for p in range(NPAIR)]

            # =========================================================
            # Phase 1: projections + RoPE + V assembly
            # =========================================================
            with tc.tile_pool(name="ph1sb", bufs=1) as ph1_sb, \
                 tc.tile_pool(name="ph1ps", bufs=1, space="PSUM") as ph1_ps:

                def load_xq(qc, tiles_only=False):
                    ql, qh = 512 * qc, 512 * (qc + 1)
                    ts = [ph1_sb.tile([128, 512], f32r, tag="xq", bufs=14, name=f"xq{kc}_{qc}")
                          for kc in range(KC)]
                    if not tiles_only:
                        for kc in range(KC):
                            nc.sync.dma_start(out=ts[kc], in_=xT_d[128 * kc:128 * (kc + 1), ql:qh])
                    return ts

                # q-chunks processed descending so that phase 2 (which walks
                # k-blocks descending) can start as soon as the tail chunk of
                # Q^T/K^T/V is ready.  First-chunk x slices and weights are
                # DMA'd interleaved per k-chunk so the first matmuls start
                # as soon as possible.
                qc_order = list(reversed(range(NQC)))
                w_t = {t_i: [ph1_sb.tile([128, DHC], f32r, tag=f"w{t_i}_{kc}", name=f"w{t_i}_{kc}")
                             for kc in range(KC)] for t_i in (0, 1)}
                wv_t = [ph1_sb.tile([128, DHC], f32r, tag=f"wv{kc}", name=f"wv{kc}")
                        for kc in range(KC)]
                def load_rope(qc):
                    # [64, 512] source pair-stacked to 128 partitions via two DMAs
                    ql = 512 * qc
                    ts = [ph1_sb.tile([128, 512], f32, tag="rope", bufs=4, name=f"rope{i}_{qc}")
                          for i in range(2)]
                    for i in range(2):
                        nc.sync.dma_start(out=ts[i][0:64, :], in_=rope_d[i][:, ql:ql + 512])
                        nc.sync.dma_start(out=ts[i][64:128, :], in_=rope_d[i][:, ql:ql + 512])
                    return ts

                xq_next = load_xq(qc_order[0], tiles_only=True)
                q3l = 512 * qc_order[0]
                for kc in range(KC):
                    nc.sync.dma_start(out=w_t[0][kc], in_=wq_d[128 * kc:128 * (kc + 1), :])
                    nc.sync.dma_start(out=w_t[1][kc], in_=wk_d[128 * kc:128 * (kc + 1), :])
                    nc.sync.dma_start(out=xq_next[kc], in_=xT_d[128 * kc:128 * (kc + 1), q3l:q3l + 512])
                rope_next = load_rope(qc_order[0])
                for t_, src_ in const_dmas:
                    nc.sync.dma_start(out=t_, in_=src_)
                for kc in range(KC):
                    nc.sync.dma_start(out=wv_t[kc], in_=wv_d[128 * kc:128 * (kc + 1), :])

                for qi, qc in enumerate(qc_order):
                    ql, qh = 512 * qc, 512 * (qc + 1)
                    xq = xq_next
                    rope_s = rope_next
                    pps = {}
                    for t_i in (0, 1):
                        for p in range(NPAIR):
                            pps[t_i, p] = ph1_ps.tile([128, 512], f32, tag="qtp",
                                                      bufs=4, name=f"pp{t_i}_{p}_{qc}")
                    for kc in range(KC):
                        for t_i in (0, 1):
                            for p in range(NPAIR):
                                nc.tensor.matmul(pps[t_i, p],
                                                 w_t[t_i][kc][:, 128 * p:128 * (p + 1)],
                                                 xq[kc], start=(kc == 0), stop=(kc == KC - 1))
                    if qi + 1 < NQC:
                        xq_next = load_xq(qc_order[qi + 1])
                        rope_next = load_rope(qc_order[qi + 1])
                    cos_t, sin_t = rope_s
                    for t_i in (0, 1):
                        dst_pair = qt_pair if t_i == 0 else kt_pair
                        for p in range(NPAIR):
                            pp = pps[t_i, p]
                            # RoPE with bias folded in:
                            #   dst = (pp+b)*cos + Psig @ ((pp+b)*sin_sig)
                            u_t = ph1_sb.tile([128, 512], f32r, tag="u", bufs=3)
                            nc.vector.scalar_tensor_tensor(
                                out=u_t, in0=pp, scalar=bcol[t_i, p], in1=sin_t,
                                op0=ADD, op1=MULT)
                            us = ph1_ps.tile([128, 512], f32, tag="usp", bufs=2)
                            nc.tensor.matmul(us, psig_t, u_t, start=True, stop=True)
                            dst = dst_pair[p][:, ql:qh]
                            nc.vector.scalar_tensor_tensor(
                                out=dst, in0=pp, scalar=bcol[t_i, p], in1=cos_t,
                                op0=ADD, op1=MULT)
                            nc.vector.tensor_tensor(out=dst, in0=us, in1=dst.bitcast(f32), op=ADD)
                    # V projection for the 4 s-blocks covered by this q-chunk
                    for r in range(4):
                        si = 4 * qc + r
                        vp = ph1_ps.tile([128, DHC + HPC], f32, tag="vp", bufs=2)
                        for kc in range(KC):
                            nc.tensor.matmul(vp[:, 0:DHC], xq[kc][:, 128 * r:128 * (r + 1)],
                                             wv_t[kc], start=(kc == 0), stop=False)
                        nc.tensor.matmul(vp[:, 0:DHC], ones_t[:, 0:128], bv_t,
                                         start=False, stop=True)
                        nc.tensor.matmul(vp[:, DHC:DHC + HPC], ones_t[:, 0:128],
                                         ones_t[:, 0:HPC], start=True, stop=True)
                        nc.scalar.copy(out=v_sb[si][:, :, 0:DK],
                                       in_=vp[:, 0:DHC].rearrange("p (h d) -> p h d", h=HPC))
                        nc.scalar.copy(out=v_sb[si][:, :, DK:DK + 1],
                                       in_=vp[:, DHC:DHC + HPC].rearrange("p (h o) -> p h o", h=HPC))

            # =========================================================
            # Phase 2: attention per head
            # =========================================================
            HALF = S // 2
            with tc.tile_pool(name="ph2ps", bufs=1, space="PSUM") as ph2_ps:
                # Per (pair, q-half): the two heads of the pair run as two
                # interleaved dependency chains (separate scores psum + attn
                # accumulator each) so PE and ACT stay busy simultaneously.
                # k-blocks walk descending (matches phase-1 production order);
                # PV trails one item behind QK^T/exp.
                def emit_pv(h, at_ps, hlo, it, pT):
                    j, base, w, a0 = it
                    a = a0
                    while a < w:
                        bnd = min((a // 512 + 1) * 512, w)
                        sbank = (base + a) // 512
                        jmax = min(NSB - 1, 4 * sbank + 3) if causal else NSB - 1
                        nc.tensor.matmul(at_ps[:, base + a - hlo:base + bnd - hlo],
                                         v_sb[j][:, h, :], pT[:, a:bnd],
                                         start=(j == jmax), stop=(j == 0))
                        a = bnd

                # Wo preloaded here so the per-half output projection (emitted
                # between halves, borrowing "sc" psum slots) never waits on DMA
                wo_t = [ph3_sb.tile([128, D], f32r, tag=f"wo{ch}", name=f"wo{ch}")
                        for ch in range(NPAIR)]
                for ch in range(NPAIR):
                    nc.sync.dma_start(out=wo_t[ch], in_=wo_d[128 * ch:128 * (ch + 1), :])

                def emit_outproj(half):
                    # out^T[dout, s] for this half; Wo stationary across s sweep.
                    # Borrows one "sc" pool slot ([128,1024] = two psum banks =
                    # two 512-wide outputs) per dout block, so it interleaves
                    # with the next half's attention on the PE.
                    scs_half = [2 * half, 2 * half + 1]
                    for do in range(D // 128):
                        op = ph2_ps.tile([128, 1024], f32, tag="sc",
                                         bufs=2, name=f"op{do}_{half}")
                        for ch in range(NPAIR):
                            for i, sc_ in enumerate(scs_half):
                                nc.tensor.matmul(op[:, 512 * i:512 * (i + 1)],
                                                 wo_t[ch][:, 128 * do:128 * (do + 1)],
                                                 attnT_sb[ch][:, 512 * sc_:512 * (sc_ + 1)],
                                                 start=(ch == 0), stop=(ch == NPAIR - 1))
                        ob = ph3_sb.tile([128, 1024], f32, tag="ob", bufs=4)
                        if do % 2 == 0:
                            nc.vector.tensor_copy(ob, op)
                        else:
                            nc.scalar.copy(out=ob, in_=op)
                        nc.sync.dma_start(
                            out=out_d[128 * do:128 * (do + 1), HALF * half:HALF * (half + 1)],
                            in_=ob)

                for half in (1, 0):
                    for p in range(NPAIR):
                        hlo, hhi = HALF * half, HALF * (half + 1)
                        at_ps = [ph2_ps.tile([DK + 1, HALF], f32, tag="atp",
                                             bufs=2, name=f"atp{p}_{half}_{hh}")
                                 for hh in range(2)]
                        items = []
                        for j in reversed(range(NSB)):
                            qlo = max(128 * j, hlo) if causal else hlo
                            if qlo >= hhi:
                                continue
                            base = (qlo // 512) * 512
                            first = True
                            while base < hhi:
                                w = min(1024, hhi - base)
                                a0 = (qlo - base) if first else 0
                                items.append((j, base, w, a0))
                                base += w
                                first = False
                        pend = []
                        for it in items:
                            j, base, w, a0 = it
                            scs = []
                            for hh in range(2):
                                off = 64 * hh
                                sc = ph2_ps.tile([128, 1024], f32, tag="sc",
                                                 bufs=2, name=f"sc{hh}")
                                a = a0
                                while a < w:
                                    bnd = min((a // 512 + 1) * 512, w)
                                    nc.tensor.matmul(
                                        sc[:, a:bnd],
                                        kt_pair[p][off:off + 64, 128 * j:128 * (j + 1)],
                                        qt_pair[p][off:off + 64, base + a:base + bnd],
                                        start=True, stop=True)
                                    a = bnd
                                scs.append(sc)
                            if pend:
                                for (pit, phh, ppT) in pend:
                                    emit_pv(2 * p + phh, at_ps[phh], hlo, pit, ppT)
                                pend = []
                            for hh in range(2):
                                sc = scs[hh]
                                if not causal:
                                    mt = ph2_sb.tile([128, 1024], f32, tag="mt", bufs=3)
                                    nc.sync.dma_start(
                                        out=mt[:, a0:w],
                                        in_=maskT_d[128 * j:128 * (j + 1), base + a0:base + w])
                                    nc.vector.tensor_tensor(
                                        out=sc[:, a0:w], in0=sc[:, a0:w],
                                        in1=mt[:, a0:w], op=ADD)
                                pT = ph2_sb.tile([128, 1024], f32r, tag="pT", bufs=6)
                                nc.scalar.activation(out=pT[:, a0:w], in_=sc[:, a0:w], func=EXP)
                                if causal and base <= 128 * j < base + w:
                                    dc = 128 * j - base
                                    nc.vector.tensor_tensor(
                                        out=pT[:, dc:dc + 128],
                                        in0=pT[:, dc:dc + 128].bitcast(f32),
                                        in1=mdiag_t, op=MULT)
                                pend.append((it, hh, pT))
                        for (pit, phh, ppT) in pend:
                            emit_pv(2 * p + phh, at_ps[phh], hlo, pit, ppT)
                        for hh in range(2):
                            off = 64 * hh
                            rec = ph2_sb.tile([1, HALF], f32, tag="rec", bufs=3)
                            nc.vector.reciprocal(rec, at_ps[hh][DK:DK + 1, :])
                            bc = ph2_sb.tile([64, HALF], f32, tag="bc", bufs=3)
                            nc.gpsimd.partition_broadcast(bc, rec)
                            nc.vector.tensor_tensor(out=attnT_sb[p][off:off + 64, hlo:hhi],
                                                    in0=at_ps[hh][0:DK, :], in1=bc, op=MULT)
                    # both pairs of this half normalized -> project this half's
                    # output now; its matmuls/copies/DMA overlap the next half
                    emit_outproj(half)

    nc.compile()
    return nc


def _rope_tables():
    # [2, 64, S]: cos and sigma-permuted signed sin, one head's worth; the
    # device pair-stacks to 128 partitions. The 1/sqrt(dk) score scale is
    # folded into Wq/bq on the host, so Q and K share these tables.
    half = DK // 2
    freqs = (10000.0 ** (-2.0 / DK * np.arange(half, dtype=np.float32))).astype(np.float64)
    ang = np.outer(np.arange(S, dtype=np.float64), freqs)           # [S, 32]
    cos1 = np.cos(ang).T.astype(np.float32)                          # [32, S]
    sin1 = np.sin(ang).T.astype(np.float32)
    c64 = np.concatenate([cos1, cos1], axis=0)                       # [64, S]
    ssig64 = np.concatenate([sin1, -sin1], axis=0)                   # s-tilde(sigma(p))
    return np.stack([c64, ssig64]).astype(np.float32)


def _psig():
    p64 = np.zeros((64, 64), np.float32)
    p64[np.arange(32) + 32, np.arange(32)] = 1.0
    p64[np.arange(32), np.arange(32) + 32] = 1.0
    p = np.zeros((128, 128), np.float32)
    p[0:64, 0:64] = p64
    p[64:128, 64:128] = p64
    return p


def _make_runner(nc, n_cores=NCORES):
    """Compile the SPMD program once into a reusable jitted shard_map callable
    (same execution path as bass_utils.run_bass_kernel_spmd under axon)."""
    import jax
    from jax.sharding import Mesh, PartitionSpec
    from jax.experimental.shard_map import shard_map
    from concourse import bass2jax, mybir
    from concourse.bass2jax import _bass_exec_p, install_neuronx_cc_hook

    install_neuronx_cc_hook()
    partition_name = nc.partition_id_tensor.name if nc.partition_id_tensor else None
    in_names, out_names, out_avals, zero_outs = [], [], [], []
    for alloc in nc.m.functions[0].allocations:
        if not isinstance(alloc, mybir.MemoryLocationSet):
            continue
        name = alloc.memorylocations[0].name
        if alloc.kind == "ExternalInput":
            if name != partition_name:
                in_names.append(name)
        elif alloc.kind == "ExternalOutput":
            out_names.append(name)
            shape = tuple(alloc.tensor_shape)
            dtype = mybir.dt.np(alloc.dtype)
            out_avals.append(jax.core.ShapedArray(shape, dtype))
            zero_outs.append(np.zeros(shape, dtype))
    n_params = len(in_names)
    all_in = in_names + out_names
    if partition_name is not None:
        all_in.append(partition_name)

    def _body(*args):
        operands = list(args)
        if partition_name is not None:
            operands.append(bass2jax.partition_id_tensor())
        outs = _bass_exec_p.bind(
            *operands, out_avals=tuple(out_avals), in_names=tuple(all_in),
            out_names=tuple(out_names), lowering_input_output_aliases=(),
            sim_require_finite=True, sim_require_nnan=True, nc=nc)
        return tuple(outs)

    devices = jax.devices()[:n_cores]
    mesh = Mesh(np.asarray(devices), ("core",))
    specs = (PartitionSpec("core"),) * (n_params + len(out_names))
    out_specs = (PartitionSpec("core"),) * len(out_names)
    fn = jax.jit(shard_map(_body, mesh=mesh, in_specs=specs,
                           out_specs=out_specs, check_rep=False),
                 keep_unused=True)
    concat_zeros = [np.zeros((n_cores * z.shape[0], *z.shape[1:]), z.dtype)
                    for z in zero_outs]

    def run(in_maps):
        concat_in = [np.concatenate([np.asarray(in_maps[c][k]) for c in range(n_cores)],
                                    axis=0) for k in in_names]
        outs = fn(*concat_in, *concat_zeros)
        o = np.asarray(outs[out_names.index("out")])
        return o.reshape(n_cores, *zero_outs[out_names.index("out")].shape)

    return run


def kernel(x, mask, Wq, bq, Wk, bk, Wv, bv, Wo, bo):
    x = np.asarray(x, dtype=np.float32)
    mask = np.asarray(mask)
    Wq, bq = np.asarray(Wq, np.float32), np.asarray(bq, np.float32)
    Wk, bk = np.asarray(Wk, np.float32), np.asarray(bk, np.float32)
    Wv, bv = np.asarray(Wv, np.float32), np.asarray(bv, np.float32)
    Wo, bo = np.asarray(Wo, np.float32), np.asarray(bo, np.float32)

    causal_ref = np.triu(np.ones((S, S), dtype=bool), k=1)
    m2 = np.broadcast_to(mask, (B, 1, S, S))[:, 0]
    causal = all(np.array_equal(m2[b], causal_ref) for b in range(B))

    if causal not in _BUILD_CACHE:
        nc = _build(causal)
        _BUILD_CACHE[causal] = (nc, _make_runner(nc))
    nc, run = _BUILD_CACHE[causal]

    rope = _rope_tables()
    psig = _psig()
    ones = np.ones((1, 512), np.float32)
    # multiplicative 0/1 mask for the diagonal block (applied to exp(scores))
    mdiag = np.where(np.arange(128)[:, None] > np.arange(128)[None, :],
                     np.float32(0.0), np.float32(1.0)).astype(np.float32)

    xT = [np.ascontiguousarray(x[b].T) for b in range(B)]
    maskT = None
    if not causal:
        maskT = [np.ascontiguousarray(
            np.where(m2[b], np.float32(-1e30), np.float32(0.0)).T) for b in range(B)]

    in_maps = []
    for c in range(NCORES):
        b, hg = c // (NCORES // B), c % (NCORES // B)
        cs = slice(DHC * hg, DHC * (hg + 1))
        im = {
            "xT": xT[b],
            "wq": np.ascontiguousarray(Wq[:, cs] * np.float32(1.0 / math.sqrt(DK))),
            "wk": np.ascontiguousarray(Wk[:, cs]),
            "wv": np.ascontiguousarray(Wv[:, cs]),
            "wo": np.ascontiguousarray(Wo[cs, :]),
            "bqk": np.ascontiguousarray(np.stack([bq[cs] * np.float32(1.0 / math.sqrt(DK)), bk[cs]])),
            "bv": np.ascontiguousarray(bv[cs][None, :]),
            "ones": ones, "psig": psig, "rope": rope, "mdiag": mdiag,
        }
        if not causal:
            im["maskT"] = maskT[b]
        in_maps.append(im)

    try:
        partials = run(in_maps)
    except Exception:
        # fallback: canonical SPMD runner (recompiles per call)
        from concourse.bass_utils import run_bass_kernel_spmd
        res = run_bass_kernel_spmd(nc, in_maps, core_ids=list(range(NCORES)))
        partials = np.stack([res.results[c]["out"] for c in range(NCORES)])
    out = np.zeros((B, S, D), np.float32)
    for c in range(NCORES):
        out[c // (NCORES // B)] += partials[c].T
    out += bo[None, None, :]
    return out


# revision 30
# speedup vs baseline: 22156.9816x; 1.0006x over previous
"""Multi-head attention (RoPE, causal) Trainium2 Bass kernel, 8-core SPMD.

Problem: B=2, S=2048, D=1024, H=16, DK=64, fp32, causal mask.

Sharding: core c handles batch b = c//4 and head group hg = c%4 (4 heads).
Each core computes Q/K/V projections for its 4 heads (column-sliced weights),
RoPE, causal attention, and a partial output projection (row-sliced Wo).
Host sums the 4 partial outputs per batch and adds the output bias.

Layout strategy (no on-device transposes):
  x^T [D, S] is precomputed on host; Q^T/K^T computed as [dk, S] tiles
  (weights stationary, x^T moving); scores computed transposed [k, q]
  (K^T stationary, Q^T moving); PV uses V in natural layout [k, dk+1]
  (stationary) with exp(scores^T) moving, accumulating attn^T [dk(+1), q];
  the ones column of V accumulates the softmax denominator. Normalization
  multiplies attn^T rows by broadcast 1/denom. Output projection uses
  attn^T as stationary and Wo as moving, producing natural [s, D] partials.

All matmul operands are float32r (TF32-like fast mode: 1 cycle/row at
moving-dim >= 256 vs 4 cycles/row for fp32): ~1e-4 relative L2 per matmul.
"""
import sys
sys.path.insert(0, "/opt/trn_rl_repo")
import math
import numpy as np

B, S, D, H, DK = 2, 2048, 1024, 16, 64
NCORES = 8
HPC = H // (NCORES // B)     # 4 heads per core
DHC = HPC * DK               # 256 attn dims per core
NPAIR = HPC // 2             # 2 head pairs per core
KC = D // 128                # 8 contraction chunks
NSB = S // 128               # 16 s-blocks / k-blocks
NQC = S // 512               # 4 q-chunks of 512

_BUILD_CACHE = {}


def _build(causal: bool):
    import concourse.tile as tile
    from concourse import bacc, mybir

    f32, f32r = mybir.dt.float32, mybir.dt.float32r
    MULT, ADD = mybir.AluOpType.mult, mybir.AluOpType.add
    EXP = mybir.ActivationFunctionType.Exp

    nc = bacc.Bacc(target_bir_lowering=False, trn_type="TRN2", debug=False)

    xT_d = nc.dram_tensor("xT", [D, S], f32r, kind="ExternalInput")
    wq_d = nc.dram_tensor("wq", [D, DHC], f32r, kind="ExternalInput")
    wk_d = nc.dram_tensor("wk", [D, DHC], f32r, kind="ExternalInput")
    wv_d = nc.dram_tensor("wv", [D, DHC], f32r, kind="ExternalInput")
    wo_d = nc.dram_tensor("wo", [DHC, D], f32r, kind="ExternalInput")
    bqk_d = nc.dram_tensor("bqk", [2, DHC], f32r, kind="ExternalInput")
    bv_d = nc.dram_tensor("bv", [1, DHC], f32r, kind="ExternalInput")
    ones_d = nc.dram_tensor("ones", [1, 512], f32r, kind="ExternalInput")
    psig_d = nc.dram_tensor("psig", [128, 128], f32r, kind="ExternalInput")
    rope_d = nc.dram_tensor("rope", [2, 64, S], f32, kind="ExternalInput")
    mdiag_d = nc.dram_tensor("mdiag", [128, 128], f32, kind="ExternalInput")
    out_d = nc.dram_tensor("out", [D, S], f32, kind="ExternalOutput")
    if not causal:
        maskT_d = nc.dram_tensor("maskT", [S, S], f32, kind="ExternalInput")

    with tile.TileContext(nc) as tc:
        with tc.tile_pool(name="const", bufs=1) as const_p, \
             tc.tile_pool(name="persist", bufs=1) as pers_p, \
             tc.tile_pool(name="ph2sb", bufs=1) as ph2_sb, \
             tc.tile_pool(name="ph3sb", bufs=1) as ph3_sb:

            # ---------- constants (tiles now; DMAs deferred until after the
            # first-chunk weight/x preload so the first matmuls start early) ----------
            const_dmas = []
            ones_t = const_p.tile([1, 512], f32r, tag="ones")
            const_dmas.append((ones_t, ones_d[:]))
            psig_t = const_p.tile([128, 128], f32r, tag="psig")
            const_dmas.append((psig_t, psig_d[:]))
            # per-pair bias columns [128, 1] (partition = head-pair dims)
            bcol = {}
            for t_i in (0, 1):
                for p in range(NPAIR):
                    bt = const_p.tile([128, 1], f32r, tag=f"bc{t_i}{p}", name=f"bc{t_i}{p}")
                    src_row = bqk_d[t_i:t_i + 1, 128 * p:128 * (p + 1)]
                    const_dmas.append((bt, src_row.rearrange("o d -> d o")))
                    bcol[t_i, p] = bt.bitcast(f32)
            bv_t = const_p.tile([1, DHC], f32r, tag="bv")
            const_dmas.append((bv_t, bv_d[:]))
            mdiag_t = const_p.tile([128, 128], f32, tag="mdiag")
            const_dmas.append((mdiag_t, mdiag_d[:]))

            # ---------- persistent activations ----------
            qt_pair = [pers_p.tile([128, S], f32r, tag=f"qt{p}", name=f"qt{p}") for p in range(NPAIR)]
            kt_pair = [pers_p.tile([128, S], f32r, tag=f"kt{p}", name=f"kt{p}") for p in range(NPAIR)]
            v_sb = [pers_p.tile([128, HPC, DK + 1], f32r, tag=f"v{i}", name=f"v{i}") for i in range(NSB)]
            attnT_sb = [pers_p.tile([128, S], f32r, tag=f"at{p}", name=f"at{p}") for p in range(NPAIR)]

            # =========================================================
            # Phase 1: projections + RoPE + V assembly
            # =========================================================
            with tc.tile_pool(name="ph1sb", bufs=1) as ph1_sb, \
                 tc.tile_pool(name="ph1ps", bufs=1, space="PSUM") as ph1_ps:

                def load_xq(qc, tiles_only=False):
                    ql, qh = 512 * qc, 512 * (qc + 1)
                    ts = [ph1_sb.tile([128, 512], f32r, tag="xq", bufs=14, name=f"xq{kc}_{qc}")
                          for kc in range(KC)]
                    if not tiles_only:
                        for kc in range(KC):
                            nc.sync.dma_start(out=ts[kc], in_=xT_d[128 * kc:128 * (kc + 1), ql:qh])
                    return ts

                # q-chunks processed descending so that phase 2 (which walks
                # k-blocks descending) can start as soon as the tail chunk of
                # Q^T/K^T/V is ready.  First-chunk x slices and weights are
                # DMA'd interleaved per k-chunk so the first matmuls start
                # as soon as possible.
                qc_order = list(reversed(range(NQC)))
                w_t = {t_i: [ph1_sb.tile([128, DHC], f32r, tag=f"w{t_i}_{kc}", name=f"w{t_i}_{kc}")
                             for kc in range(KC)] for t_i in (0, 1)}
                wv_t = [ph1_sb.tile([128, DHC], f32r, tag=f"wv{kc}", name=f"wv{kc}")
                        for kc in range(KC)]
                def load_rope(qc):
                    # [64, 512] source pair-stacked to 128 partitions via two DMAs
                    ql = 512 * qc
                    ts = [ph1_sb.tile([128, 512], f32, tag="rope", bufs=4, name=f"rope{i}_{qc}")
                          for i in range(2)]
                    for i in range(2):
                        nc.sync.dma_start(out=ts[i][0:64, :], in_=rope_d[i][:, ql:ql + 512])
                        nc.sync.dma_start(out=ts[i][64:128, :], in_=rope_d[i][:, ql:ql + 512])
                    return ts

                xq_next = load_xq(qc_order[0], tiles_only=True)
                q3l = 512 * qc_order[0]
                for kc in range(KC):
                    nc.sync.dma_start(out=w_t[0][kc], in_=wq_d[128 * kc:128 * (kc + 1), :])
                    nc.sync.dma_start(out=w_t[1][kc], in_=wk_d[128 * kc:128 * (kc + 1), :])
                    nc.sync.dma_start(out=xq_next[kc], in_=xT_d[128 * kc:128 * (kc + 1), q3l:q3l + 512])
                rope_next = load_rope(qc_order[0])
                for t_, src_ in const_dmas:
                    nc.sync.dma_start(out=t_, in_=src_)
                for kc in range(KC):
                    nc.sync.dma_start(out=wv_t[kc], in_=wv_d[128 * kc:128 * (kc + 1), :])

                for qi, qc in enumerate(qc_order):
                    ql, qh = 512 * qc, 512 * (qc + 1)
                    xq = xq_next
                    rope_s = rope_next
                    pps = {}
                    for t_i in (0, 1):
                        for p in range(NPAIR):
                            pps[t_i, p] = ph1_ps.tile([128, 512], f32, tag="qtp",
                                                      bufs=4, name=f"pp{t_i}_{p}_{qc}")
                    for kc in range(KC):
                        for t_i in (0, 1):
                            for p in range(NPAIR):
                                nc.tensor.matmul(pps[t_i, p],
                                                 w_t[t_i][kc][:, 128 * p:128 * (p + 1)],
                                                 xq[kc], start=(kc == 0), stop=(kc == KC - 1))
                    if qi + 1 < NQC:
                        xq_next = load_xq(qc_order[qi + 1])
                        rope_next = load_rope(qc_order[qi + 1])
                    cos_t, sin_t = rope_s
                    for t_i in (0, 1):
                        dst_pair = qt_pair if t_i == 0 else kt_pair
                        for p in range(NPAIR):
                            pp = pps[t_i, p]
                            # RoPE with bias folded in:
                            #   dst = (pp+b)*cos + Psig @ ((pp+b)*sin_sig)
                            u_t = ph1_sb.tile([128, 512], f32r, tag="u", bufs=3)
                            nc.vector.scalar_tensor_tensor(
                                out=u_t, in0=pp, scalar=bcol[t_i, p], in1=sin_t,
                                op0=ADD, op1=MULT)
                            us = ph1_ps.tile([128, 512], f32, tag="usp", bufs=2)
                            nc.tensor.matmul(us, psig_t, u_t, start=True, stop=True)
                            dst = dst_pair[p][:, ql:qh]
                            nc.vector.scalar_tensor_tensor(
                                out=dst, in0=pp, scalar=bcol[t_i, p], in1=cos_t,
                                op0=ADD, op1=MULT)
                            nc.vector.tensor_tensor(out=dst, in0=us, in1=dst.bitcast(f32), op=ADD)
                    # V projection for the 4 s-blocks covered by this q-chunk
                    for r in range(4):
                        si = 4 * qc + r
                        vp = ph1_ps.tile([128, DHC + HPC], f32, tag="vp", bufs=2)
                        for kc in range(KC):
                            nc.tensor.matmul(vp[:, 0:DHC], xq[kc][:, 128 * r:128 * (r + 1)],
                                             wv_t[kc], start=(kc == 0), stop=False)
                        nc.tensor.matmul(vp[:, 0:DHC], ones_t[:, 0:128], bv_t,
                                         start=False, stop=True)
                        nc.tensor.matmul(vp[:, DHC:DHC + HPC], ones_t[:, 0:128],
                                         ones_t[:, 0:HPC], start=True, stop=True)
                        nc.scalar.copy(out=v_sb[si][:, :, 0:DK],
                                       in_=vp[:, 0:DHC].rearrange("p (h d) -> p h d", h=HPC))
                        nc.scalar.copy(out=v_sb[si][:, :, DK:DK + 1],
                                       in_=vp[:, DHC:DHC + HPC].rearrange("p (h o) -> p h o", h=HPC))

            # =========================================================
            # Phase 2: attention per head
            # =========================================================
            HALF = S // 2
            with tc.tile_pool(name="ph2ps", bufs=1, space="PSUM") as ph2_ps:
                # Per (pair, q-half): the two heads of the pair run as two
                # interleaved dependency chains (separate scores psum + attn
                # accumulator each) so PE and ACT stay busy simultaneously.
                # k-blocks walk descending (matches phase-1 production order);
                # PV trails one item behind QK^T/exp.
                def emit_pv(h, at_ps, hlo, it, pT):
                    j, base, w, a0 = it
                    a = a0
                    while a < w:
                        bnd = min((a // 512 + 1) * 512, w)
                        sbank = (base + a) // 512
                        jmax = min(NSB - 1, 4 * sbank + 3) if causal else NSB - 1
                        nc.tensor.matmul(at_ps[:, base + a - hlo:base + bnd - hlo],
                                         v_sb[j][:, h, :], pT[:, a:bnd],
                                         start=(j == jmax), stop=(j == 0))
                        a = bnd

                # Wo preloaded here so the per-half output projection (emitted
                # between halves, borrowing "sc" psum slots) never waits on DMA
                wo_t = [ph3_sb.tile([128, D], f32r, tag=f"wo{ch}", name=f"wo{ch}")
                        for ch in range(NPAIR)]
                for ch in range(NPAIR):
                    nc.sync.dma_start(out=wo_t[ch], in_=wo_d[128 * ch:128 * (ch + 1), :])

                def emit_outproj(half, tag):
                    # out^T[dout, s] for this half; Wo stationary across s sweep.
                    # Borrows pool slots ([128,1024] = two psum banks = two
                    # 512-wide outputs) per dout block: "sc" slots when the
                    # next half's attention still needs the "atp" accumulators,
                    # "atp" slots for the final half (they free right after
                    # each normalize, before the trailing exps release "sc").
                    scs_half = [2 * half, 2 * half + 1]
                    for do in range(D // 128):
                        op = ph2_ps.tile([128, 1024], f32, tag=tag,
                                         bufs=2, name=f"op{do}_{half}")
                        for ch in range(NPAIR):
                            for i, sc_ in enumerate(scs_half):
                                nc.tensor.matmul(op[:, 512 * i:512 * (i + 1)],
                                                 wo_t[ch][:, 128 * do:128 * (do + 1)],
                                                 attnT_sb[ch][:, 512 * sc_:512 * (sc_ + 1)],
                                                 start=(ch == 0), stop=(ch == NPAIR - 1))
                        ob = ph3_sb.tile([128, 1024], f32, tag="ob", bufs=4)
                        if do % 2 == 0:
                            nc.vector.tensor_copy(ob, op)
                        else:
                            nc.scalar.copy(out=ob, in_=op)
                        nc.sync.dma_start(
                            out=out_d[128 * do:128 * (do + 1), HALF * half:HALF * (half + 1)],
                            in_=ob)

                for half in (1, 0):
                    for p in range(NPAIR):
                        hlo, hhi = HALF * half, HALF * (half + 1)
                        at_ps = [ph2_ps.tile([DK + 1, HALF], f32, tag="atp",
                                             bufs=2, name=f"atp{p}_{half}_{hh}")
                                 for hh in range(2)]
                        items = []
                        for j in reversed(range(NSB)):
                            qlo = max(128 * j, hlo) if causal else hlo
                            if qlo >= hhi:
                                continue
                            base = (qlo // 512) * 512
                            first = True
                            while base < hhi:
                                w = min(1024, hhi - base)
                                a0 = (qlo - base) if first else 0
                                items.append((j, base, w, a0))
                                base += w
                                first = False
                        pend = []
                        for it in items:
                            j, base, w, a0 = it
                            scs = []
                            for hh in range(2):
                                off = 64 * hh
                                sc = ph2_ps.tile([128, 1024], f32, tag="sc",
                                                 bufs=2, name=f"sc{hh}")
                                a = a0
                                while a < w:
                                    bnd = min((a // 512 + 1) * 512, w)
                                    nc.tensor.matmul(
                                        sc[:, a:bnd],
                                        kt_pair[p][off:off + 64, 128 * j:128 * (j + 1)],
                                        qt_pair[p][off:off + 64, base + a:base + bnd],
                                        start=True, stop=True)
                                    a = bnd
                                scs.append(sc)
                            if pend:
                                for (pit, phh, ppT) in pend:
                                    emit_pv(2 * p + phh, at_ps[phh], hlo, pit, ppT)
                                pend = []
                            for hh in range(2):
                                sc = scs[hh]
                                if not causal:
                                    mt = ph2_sb.tile([128, 1024], f32, tag="mt", bufs=3)
                                    nc.sync.dma_start(
                                        out=mt[:, a0:w],
                                        in_=maskT_d[128 * j:128 * (j + 1), base + a0:base + w])
                                    nc.vector.tensor_tensor(
                                        out=sc[:, a0:w], in0=sc[:, a0:w],
                                        in1=mt[:, a0:w], op=ADD)
                                pT = ph2_sb.tile([128, 1024], f32r, tag="pT", bufs=6)
                                nc.scalar.activation(out=pT[:, a0:w], in_=sc[:, a0:w], func=EXP)
                                if causal and base <= 128 * j < base + w:
                                    dc = 128 * j - base
                                    nc.vector.tensor_tensor(
                                        out=pT[:, dc:dc + 128],
                                        in0=pT[:, dc:dc + 128].bitcast(f32),
                                        in1=mdiag_t, op=MULT)
                                pend.append((it, hh, pT))
                        for (pit, phh, ppT) in pend:
                            emit_pv(2 * p + phh, at_ps[phh], hlo, pit, ppT)
                        for hh in range(2):
                            off = 64 * hh
                            rec = ph2_sb.tile([1, HALF], f32, tag="rec", bufs=3)
                            nc.vector.reciprocal(rec, at_ps[hh][DK:DK + 1, :])
                            bc = ph2_sb.tile([64, HALF], f32, tag="bc", bufs=3)
                            nc.gpsimd.partition_broadcast(bc, rec)
                            nc.vector.tensor_tensor(out=attnT_sb[p][off:off + 64, hlo:hhi],
                                                    in0=at_ps[hh][0:DK, :], in1=bc, op=MULT)
                    # both pairs of this half normalized -> project this half's
                    # output now; its matmuls/copies/DMA overlap the next half
                    emit_outproj(half, "sc" if half == 1 else "atp")

    nc.compile()
    return nc


def _rope_tables():
    # [2, 64, S]: cos and sigma-permuted signed sin, one head's worth; the
    # device pair-stacks to 128 partitions. The 1/sqrt(dk) score scale is
    # folded into Wq/bq on the host, so Q and K share these tables.
    half = DK // 2
    freqs = (10000.0 ** (-2.0 / DK * np.arange(half, dtype=np.float32))).astype(np.float64)
    ang = np.outer(np.arange(S, dtype=np.float64), freqs)           # [S, 32]
    cos1 = np.cos(ang).T.astype(np.float32)                          # [32, S]
    sin1 = np.sin(ang).T.astype(np.float32)
    c64 = np.concatenate([cos1, cos1], axis=0)                       # [64, S]
    ssig64 = np.concatenate([sin1, -sin1], axis=0)                   # s-tilde(sigma(p))
    return np.stack([c64, ssig64]).astype(np.float32)


def _psig():
    p64 = np.zeros((64, 64), np.float32)
    p64[np.arange(32) + 32, np.arange(32)] = 1.0
    p64[np.arange(32), np.arange(32) + 32] = 1.0
    p = np.zeros((128, 128), np.float32)
    p[0:64, 0:64] = p64
    p[64:128, 64:128] = p64
    return p


def _make_runner(nc, n_cores=NCORES):
    """Compile the SPMD program once into a reusable jitted shard_map callable
    (same execution path as bass_utils.run_bass_kernel_spmd under axon)."""
    import jax
    from jax.sharding import Mesh, PartitionSpec
    from jax.experimental.shard_map import shard_map
    from concourse import bass2jax, mybir
    from concourse.bass2jax import _bass_exec_p, install_neuronx_cc_hook

    install_neuronx_cc_hook()
    partition_name = nc.partition_id_tensor.name if nc.partition_id_tensor else None
    in_names, out_names, out_avals, zero_outs = [], [], [], []
    for alloc in nc.m.functions[0].allocations:
        if not isinstance(alloc, mybir.MemoryLocationSet):
            continue
        name = alloc.memorylocations[0].name
        if alloc.kind == "ExternalInput":
            if name != partition_name:
                in_names.append(name)
        elif alloc.kind == "ExternalOutput":
            out_names.append(name)
            shape = tuple(alloc.tensor_shape)
            dtype = mybir.dt.np(alloc.dtype)
            out_avals.append(jax.core.ShapedArray(shape, dtype))
            zero_outs.append(np.zeros(shape, dtype))
    n_params = len(in_names)
    all_in = in_names + out_names
    if partition_name is not None:
        all_in.append(partition_name)

    def _body(*args):
        operands = list(args)
        if partition_name is not None:
            operands.append(bass2jax.partition_id_tensor())
        outs = _bass_exec_p.bind(
            *operands, out_avals=tuple(out_avals), in_names=tuple(all_in),
            out_names=tuple(out_names), lowering_input_output_aliases=(),
            sim_require_finite=True, sim_require_nnan=True, nc=nc)
        return tuple(outs)

    devices = jax.devices()[:n_cores]
    mesh = Mesh(np.asarray(devices), ("core",))
    specs = (PartitionSpec("core"),) * (n_params + len(out_names))
    out_specs = (PartitionSpec("core"),) * len(out_names)
    fn = jax.jit(shard_map(_body, mesh=mesh, in_specs=specs,
                           out_specs=out_specs, check_rep=False),
                 keep_unused=True)
    concat_zeros = [np.zeros((n_cores * z.shape[0], *z.shape[1:]), z.dtype)
                    for z in zero_outs]

    def run(in_maps):
        concat_in = [np.concatenate([np.asarray(in_maps[c][k]) for c in range(n_cores)],
                                    axis=0) for k in in_names]
        outs = fn(*concat_in, *concat_zeros)
        o = np.asarray(outs[out_names.index("out")])
        return o.reshape(n_cores, *zero_outs[out_names.index("out")].shape)

    return run


def kernel(x, mask, Wq, bq, Wk, bk, Wv, bv, Wo, bo):
    x = np.asarray(x, dtype=np.float32)
    mask = np.asarray(mask)
    Wq, bq = np.asarray(Wq, np.float32), np.asarray(bq, np.float32)
    Wk, bk = np.asarray(Wk, np.float32), np.asarray(bk, np.float32)
    Wv, bv = np.asarray(Wv, np.float32), np.asarray(bv, np.float32)
    Wo, bo = np.asarray(Wo, np.float32), np.asarray(bo, np.float32)

    causal_ref = np.triu(np.ones((S, S), dtype=bool), k=1)
    m2 = np.broadcast_to(mask, (B, 1, S, S))[:, 0]
    causal = all(np.array_equal(m2[b], causal_ref) for b in range(B))

    if causal not in _BUILD_CACHE:
        nc = _build(causal)
        _BUILD_CACHE[causal] = (nc, _make_runner(nc))
    nc, run = _BUILD_CACHE[causal]

    rope = _rope_tables()
    psig = _psig()
    ones = np.ones((1, 512), np.float32)
    # multiplicative 0/1 mask for the diagonal block (applied to exp(scores))
    mdiag = np.where(np.arange(128)[:, None] > np.arange(128)[None, :],
                     np.float32(0.0), np.float32(1.0)).astype(np.float32)

    xT = [np.ascontiguousarray(x[b].T) for b in range(B)]
    maskT = None
    if not causal:
        maskT = [np.ascontiguousarray(
            np.where(m2[b], np.float32(-1e30), np.float32(0.0)).T) for b in range(B)]

    in_maps = []
    for c in range(NCORES):
        b, hg = c // (NCORES // B), c % (NCORES // B)
        cs = slice(DHC * hg, DHC * (hg + 1))
        im = {
            "xT": xT[b],
            "wq": np.ascontiguousarray(Wq[:, cs] * np.float32(1.0 / math.sqrt(DK))),
            "wk": np.ascontiguousarray(Wk[:, cs]),
            "wv": np.ascontiguousarray(Wv[:, cs]),
            "wo": np.ascontiguousarray(Wo[cs, :]),
            "bqk": np.ascontiguousarray(np.stack([bq[cs] * np.float32(1.0 / math.sqrt(DK)), bk[cs]])),
            "bv": np.ascontiguousarray(bv[cs][None, :]),
            "ones": ones, "psig": psig, "rope": rope, "mdiag": mdiag,
        }
        if not causal:
            im["maskT"] = maskT[b]
        in_maps.append(im)

    try:
        partials = run(in_maps)
    except Exception:
        # fallback: canonical SPMD runner (recompiles per call)
        from concourse.bass_utils import run_bass_kernel_spmd
        res = run_bass_kernel_spmd(nc, in_maps, core_ids=list(range(NCORES)))
        partials = np.stack([res.results[c]["out"] for c in range(NCORES)])
    out = np.zeros((B, S, D), np.float32)
    for c in range(NCORES):
        out[c // (NCORES // B)] += partials[c].T
    out += bo[None, None, :]
    return out
